# revision 39
# baseline (speedup 1.0000x reference)
"""Bass/Trainium2 kernel for nn_KPlexPool (GCN blocks + cover pooling), 8 NeuronCores.

v3: message-passing gathers use dma_gather (one SWDGE instruction per <=4096
slots instead of one per 128 rows), with slot streams sorted by (src_core,
dst) so each instruction's int16 indices fit one core's 32K-row window of the
AllGathered bf16 table.  Segment sums are done by TensorE: per 128-slot block
a one-hot matrix M[slot, dst_partition] = (iota==dstp)*ew is built in one DVE
tensor_scalar op and matmul'd against the gathered rows, accumulating into a
f32 SBUF accumulator per tile.  Cover pooling keeps dst-aligned CSR (needed
for max) bucketed by source core, gathered with dma_gather + DVE reduces.
Activations stay f32 in SBUF (dense matmuls fp32); only the gathered hs/x1
tables, their AllGathers, and the one-hot ew are bf16.  The program is
SPMD-uniform: all slot counts are padded to per-(tile, src_core) maxima over
cores; per-core data (indices, dstp, ew) carries the differences.
"""

import sys
import numpy as np
import ml_dtypes

sys.path.insert(0, "/opt/trn_rl_repo")

PART = 128
NCORES = 8
EPS = 1e-5
NIDX_MAX = 1024      # slots per dma_gather (ring-limited: 2048+ wedges SWDGE)
EPI_CH = 16          # tiles per epilogue chunk
STRIPE = 32          # tiles per hs DMA stripe
STRIPE_X = 8         # tiles per x lhsT load stripe

BF16 = ml_dtypes.bfloat16
WIDE = 128           # gathered-table row width (bf16 -> 256B rows)


# ----------------------------------------------------------------- host prep

def _shard_items(batch, sortkey, B):
    n = batch.shape[0]
    counts = np.bincount(batch, minlength=B)
    starts = np.concatenate([[0], np.cumsum(counts)[:-1]])
    perm_cores = [[] for _ in range(NCORES)]
    tile_graph = []
    for g in range(B):
        cnt = int(counts[g])
        st = int(starts[g])
        base, rem = divmod(cnt, NCORES)
        sizes = [base + (1 if c < rem else 0) for c in range(NCORES)]
        Tg = max(1, -(-max(sizes) // PART))
        tile_graph += [g] * Tg
        off = st
        for c in range(NCORES):
            s = sizes[c]
            ids = np.arange(off, off + s)
            off += s
            order = np.argsort(-sortkey[ids], kind="stable")
            padded = np.full(Tg * PART, -1, dtype=np.int64)
            padded[:s] = ids[order]
            perm_cores[c].append(padded)
    perm = [np.concatenate(p) for p in perm_cores]
    rows = perm[0].shape[0]
    pos = np.full(n, -1, dtype=np.int64)
    for c in range(NCORES):
        real = perm[c] >= 0
        pos[perm[c][real]] = c * rows + np.nonzero(real)[0]
    return perm, pos, rows, np.asarray(tile_graph)


def _wrap16(flat):
    """idx stream [n] -> dma_gather idx layout [128, n/16] int16."""
    n = flat.shape[0]
    assert n % 16 == 0
    w = np.zeros((16, n // 16), dtype=np.int16)
    w[np.arange(n) % 16, np.arange(n) // 16] = flat
    return np.tile(w, (8, 1))


def _mp_stream(dst_loc, src_pos, w, rows, T):
    """(src_core, dst)-sorted slot stream, chunked into tile-major
    super-chunks of 8 per-core sub-gathers so each tile's segment-sum is one
    PSUM accumulation group.

    Returns program-uniform meta + per-core data (idx16, dstpn, ewb, ewbn).
    """
    cnt = np.zeros((NCORES, T, NCORES), np.int64)
    percore = []
    for me in range(NCORES):
        dl, sp, ww = dst_loc[me], src_pos[me], w[me]
        sc = sp // rows
        np.add.at(cnt[me], (dl // PART, sc), 1)
        percore.append((dl, sp, ww, sc))
    n_tc = cnt.max(axis=0)  # [T, NCORES]
    n_tc = ((n_tc + 15) // 16) * 16          # x16 so idx slices stay aligned

    sec_len = n_tc.sum(axis=0) + 128         # +128 slack for x128 rounding
    sec_len_p = ((sec_len + 127) // 128) * 128
    sec_off = np.concatenate([[0], np.cumsum(sec_len_p)])
    S = int(sec_off[-1])

    tc_off = np.zeros((T, NCORES), np.int64)
    for c in range(NCORES):
        off = int(sec_off[c])
        for t in range(T):
            tc_off[t, c] = off
            off += int(n_tc[t, c])

    # super-chunks: tile ranges where every core's sub-range <= NIDX_MAX-127
    lim = NIDX_MAX - 127
    tranges = []
    t0 = 0
    while t0 < T:
        t1 = t0 + 1
        while (t1 < T and int(n_tc[t0:t1 + 1].sum(axis=0).max()) <= lim):
            t1 += 1
        tranges.append((t0, t1))
        t0 = t1

    # emission: per super-chunk, per core: sub-gather (a, npad); blocks are
    # instruction-local; runs = (sub, blk_local, tile, lo, hi) -> emission ids
    schunks = []
    nblk_em = 0
    runs_all = []  # run id -> (blk_em, tile, lo, hi, abs_lo)
    for (t0, t1) in tranges:
        subs = []
        tile_ops = {t: [] for t in range(t0, t1)}
        r0, b0 = len(runs_all), nblk_em
        for c in range(NCORES):
            a = int(tc_off[t0, c])
            b = (int(tc_off[t1, c]) if t1 < T
                 else int(tc_off[t1 - 1, c] + n_tc[t1 - 1, c]))
            npad = ((b - a + 127) // 128) * 128
            assert a + npad <= int(sec_off[c + 1]), "section slack exceeded"
            sub_id = len(subs)
            subs.append((c, a, npad))
            for t in range(t0, t1):
                lo = int(tc_off[t, c]) - a
                hi = lo + int(n_tc[t, c])
                s = lo
                while s < hi:
                    bl = s // 128
                    e = min(hi, (bl + 1) * 128)
                    runs_all.append((nblk_em + bl, t, s - bl * 128,
                                     e - bl * 128, a + s))
                    tile_ops[t].append((sub_id, bl, len(runs_all) - 1))
                    s = e
            nblk_em += npad // 128
        schunks.append(dict(t0=t0, t1=t1, subs=subs, tile_ops=tile_ops,
                            r0=r0, r1=len(runs_all), b0=b0, b1=nblk_em))
    nruns_em = len(runs_all)
    mrun = max(1, max(s["r1"] - s["r0"] for s in schunks))
    mblk = max(1, max(s["b1"] - s["b0"] for s in schunks))

    # per-core data tables in emission layout
    idx16s, dstpns, ewbs, ewbns = [], [], [], []
    for me in range(NCORES):
        dl, sp, ww, sc = percore[me]
        order = np.lexsort((dl, sc))
        dls, sps, wws, scs = (a[order] for a in (dl, sp, ww, sc))
        idx = np.zeros(S, np.int16)
        dstp_sl = np.full(S, 255.0, np.float32)
        ew_sl = np.zeros(S, np.float32)
        ptr = 0
        for c in range(NCORES):
            for t in range(T):
                k = int(cnt[me, t, c])
                if k:
                    o = int(tc_off[t, c])
                    sl = slice(ptr, ptr + k)
                    idx[o:o + k] = (sps[sl] % rows).astype(np.int16)
                    dstp_sl[o:o + k] = (dls[sl] % PART).astype(np.float32)
                    ew_sl[o:o + k] = wws[sl]
                    ptr += k
        assert ptr == dls.shape[0]
        dstpn = np.full((PART, max(nruns_em, 1)), -255.0, np.float32)
        for r, (be, t, lo, hi, abs_lo) in enumerate(runs_all):
            dstpn[lo:hi, r] = -dstp_sl[abs_lo:abs_lo + (hi - lo)]
        # per-emission-block ew columns
        ewb = np.zeros((PART, max(nblk_em, 1)), np.float32)
        for sch in schunks:
            be = sch["b0"]
            for (c, a, npad) in sch["subs"]:
                for bl in range(npad // 128):
                    ewb[:, be] = ew_sl[a + bl * 128:a + (bl + 1) * 128]
                    be += 1
        idx16s.append(_wrap16(idx))
        dstpns.append(dstpn)
        ewbs.append(ewb)
        ewbns.append(-ewb)
    meta = dict(S=S, nblk=nblk_em, nruns=nruns_em, schunks=schunks,
                runs_all=runs_all, mrun=mrun, mblk=mblk)
    return meta, idx16s, dstpns, ewbs, ewbns


def _cover_stream(cl_loc, src_pos, rows1, rows2, T2, pad_local):
    """Dst-aligned per-src-core-bucketed CSR for cover sum+max.

    Returns meta (k_tc [T2, NCORES], chunks, col layout) + per-core idx16."""
    cnt_pc = np.zeros((NCORES, T2 * PART, NCORES), np.int32)
    percore = []
    for me in range(NCORES):
        cl, sp = cl_loc[me], src_pos[me]
        sc = sp // rows1
        np.add.at(cnt_pc[me], (cl, sc), 1)
        percore.append((cl, sp, sc))
    # k per (tile, src core): max over partitions and cores
    k_tc = cnt_pc.reshape(NCORES, T2, PART, NCORES).max(axis=(0, 2))  # [T2, C]
    sec_cols = k_tc.sum(axis=0)  # columns per section
    col_off = np.zeros((T2, NCORES), np.int64)
    acc = 0
    sec_col0 = np.zeros(NCORES + 1, np.int64)
    for c in range(NCORES):
        sec_col0[c] = acc
        for t in range(T2):
            col_off[t, c] = acc
            acc += int(k_tc[t, c])
    sec_col0[NCORES] = acc
    Scols = acc
    S = Scols * 128

    # chunks cut only at whole (t, c) range boundaries so a tile's columns
    # never straddle two gathers
    chunks = []  # (c, col0, col1)
    maxcols = NIDX_MAX // 128
    for c in range(NCORES):
        c0 = int(sec_col0[c])
        cur = c0
        for t in range(T2):
            k = int(k_tc[t, c])
            if cur + k - c0 > maxcols and cur > c0:
                chunks.append((c, c0, cur))
                c0 = cur
            cur += k
        if cur > c0:
            chunks.append((c, c0, cur))

    idx16s = []
    for me in range(NCORES):
        cl, sp, sc = percore[me]
        idx_cols = np.full((PART, Scols), -1, np.int64)
        for c in range(NCORES):
            m = sc == c
            cls, sps = cl[m], sp[m]
            order = np.argsort(cls, kind="stable")
            cls, sps = cls[order], sps[order]
            ccount = np.bincount(cls, minlength=T2 * PART)
            first = np.concatenate([[0], np.cumsum(ccount)[:-1]])
            rank = np.arange(cls.shape[0]) - first[cls]
            tt = cls // PART
            p = cls % PART
            col = col_off[tt, c] + rank
            idx_cols[p, col] = sps % rows1
            # pads for this section -> core-local zero row
            secsl = slice(int(sec_col0[c]), int(sec_col0[c + 1]))
            sub = idx_cols[:, secsl]
            sub[sub < 0] = pad_local[c]
            idx_cols[:, secsl] = sub
        # slot stream: col-major (slot i = col*128 + p)
        flat = idx_cols.T.reshape(-1).astype(np.int16)
        idx16s.append(_wrap16(flat))
    meta = dict(k_tc=k_tc, col_off=col_off, chunks=chunks, Scols=Scols, S=S)
    return meta, idx16s


def _prep(inputs):
    f32 = np.float32
    x = np.asarray(inputs["x"], f32)
    ei = np.asarray(inputs["edge_index"], np.int64)
    wts = np.asarray(inputs["weights"], f32)
    batch = np.asarray(inputs["batch"], np.int64)
    cover_n = np.asarray(inputs["cover_n"], np.int64)
    cover_c = np.asarray(inputs["cover_c"], np.int64)
    ei2 = np.asarray(inputs["edge_index2"], np.int64)
    wts2 = np.asarray(inputs["weights2"], f32)
    batch2 = np.asarray(inputs["batch2"], np.int64)
    N = x.shape[0]
    C = batch2.shape[0]
    B = int(batch.max()) + 1 if batch.size else 1
    B = max(B, int(batch2.max()) + 1)

    indeg = np.bincount(ei[1], minlength=N)
    perm1, pos1, rows1, tg1 = _shard_items(batch, indeg, B)
    covercnt = np.bincount(cover_c, minlength=C)
    perm2, pos2, rows2, tg2 = _shard_items(batch2, covercnt, B)
    T1, T2 = rows1 // PART, rows2 // PART

    # per-core pad (zero x1) local rows
    pad_local = []
    for c in range(NCORES):
        pads = np.nonzero(perm1[c] < 0)[0]
        assert pads.size, f"core {c} has no pad node row"
        pad_local.append(int(pads[0]))

    # mp level 1: edges grouped by dst core
    dpos = pos1[ei[1]]
    spos = pos1[ei[0]]
    dl1, sp1, w1 = [], [], []
    for me in range(NCORES):
        m = (dpos // rows1) == me
        dl1.append(dpos[m] % rows1)
        sp1.append(spos[m])
        w1.append(wts[m])
    mp1, idx16_1, dstp_1, ewb_1, ewbn_1 = _mp_stream(dl1, sp1, w1, rows1, T1)

    # mp level 2
    dpos2 = pos2[ei2[1]]
    spos2 = pos2[ei2[0]]
    dl2, sp2, w2 = [], [], []
    for me in range(NCORES):
        m = (dpos2 // rows2) == me
        dl2.append(dpos2[m] % rows2)
        sp2.append(spos2[m])
        w2.append(wts2[m])
    mp2, idx16_2, dstp_2, ewb_2, ewbn_2 = _mp_stream(dl2, sp2, w2, rows2, T2)

    # cover
    cpos = pos2[cover_c]
    npos = pos1[cover_n]
    clc, spc = [], []
    for me in range(NCORES):
        m = (cpos // rows2) == me
        clc.append(cpos[m] % rows2)
        spc.append(npos[m])
    cov, idx16_c = _cover_stream(clc, spc, rows1, rows2, T2, pad_local)

    # per-core dense transposed inputs (f32) and masks
    xTs, m1s, m2s = [], [], []
    for c in range(NCORES):
        pc = perm1[c]
        xc = np.zeros((rows1, x.shape[1]), f32)
        xc[pc >= 0] = x[pc[pc >= 0]]
        xTs.append(np.ascontiguousarray(xc.T))
        m1s.append(np.ascontiguousarray(
            (pc >= 0).astype(f32).reshape(T1, PART).T))
        p2 = perm2[c]
        m2s.append(np.ascontiguousarray(
            (p2 >= 0).astype(f32).reshape(T2, PART).T))

    # degree tables (host-side: exact f32) -> dis = rsqrt(deg+1)*mask
    deg1 = np.zeros(NCORES * rows1, f32)
    np.add.at(deg1, dpos, wts)
    deg2 = np.zeros(NCORES * rows2, f32)
    np.add.at(deg2, dpos2, wts2)
    dis1s, dis2s = [], []
    for c in range(NCORES):
        d1 = 1.0 / np.sqrt(deg1[c * rows1:(c + 1) * rows1] + 1.0)
        d1 = d1.reshape(T1, PART).T * m1s[c]
        dis1s.append(np.ascontiguousarray(d1).astype(f32))
        d2 = 1.0 / np.sqrt(deg2[c * rows2:(c + 1) * rows2] + 1.0)
        d2 = d2.reshape(T2, PART).T * m2s[c]
        dis2s.append(np.ascontiguousarray(d2).astype(f32))

    meta = dict(B=B, T1=T1, T2=T2, rows1=rows1, rows2=rows2,
                mp1=mp1, mp2=mp2, cov=cov, tg1=tg1, tg2=tg2, FIN=x.shape[1])

    rep = lambda v: np.ascontiguousarray(
        np.broadcast_to(np.asarray(v, f32).reshape(1, -1), (PART, v.shape[-1])))
    g = np.asarray(inputs["bn_gamma"], f32)
    bb = np.asarray(inputs["bn_beta"], f32)
    l1w = np.asarray(inputs["lin1_W"], f32)
    H = np.asarray(inputs["W_in0"], f32).shape[1]
    selS = np.r_[0:H, 2 * H:3 * H]
    selM = np.r_[H:2 * H, 3 * H:4 * H]
    shared = {
        "W_in0": np.asarray(inputs["W_in0"], f32),
        "W_in1": np.asarray(inputs["W_in1"], f32),
        "Wl_in": np.asarray(inputs["Wl_in"], f32),
        "W_b0": np.asarray(inputs["W_b0"], f32),
        "W_b1": np.asarray(inputs["W_b1"], f32),
        "Wl_b": np.asarray(inputs["Wl_b"], f32),
        "b_in0": rep(inputs["b_in0"]), "b_in1": rep(inputs["b_in1"]),
        "bl_in": rep(inputs["bl_in"]), "b_b0": rep(inputs["b_b0"]),
        "b_b1": rep(inputs["b_b1"]), "bl_b": rep(inputs["bl_b"]),
        "gammaS": np.ascontiguousarray(g[selS].reshape(PART, 1)),
        "gammaM": np.ascontiguousarray(g[selM].reshape(PART, 1)),
        "betaS": np.ascontiguousarray(bb[selS].reshape(PART, 1)),
        "betaM": np.ascontiguousarray(bb[selM].reshape(PART, 1)),
        "l1WS": np.ascontiguousarray(l1w[selS]),
        "l1WM": np.ascontiguousarray(l1w[selM]),
        "l1b": rep(inputs["lin1_b"]),
        "l2W": np.asarray(inputs["lin2_W"], f32),
        "l2b": rep(inputs["lin2_b"]),
        "iota": np.ascontiguousarray(
            np.broadcast_to(np.arange(PART, dtype=f32)[None, :],
                            (PART, PART))),
        "iotan": np.ascontiguousarray(
            np.broadcast_to(-np.arange(PART, dtype=f32)[None, :],
                            (PART, PART))),
    }
    in_maps = []
    for c in range(NCORES):
        m = dict(shared)
        m["x_cT"] = xTs[c]
        m["mask1"] = m1s[c]
        m["mask2"] = m2s[c]
        m["dis1"] = dis1s[c]
        m["dis2"] = dis2s[c]
        m["idx16_1"] = idx16_1[c]
        m["dstp_1"] = dstp_1[c]
        m["ewb_1"] = ewb_1[c]
        m["ewbn_1"] = ewbn_1[c]
        m["idx16_2"] = idx16_2[c]
        m["dstp_2"] = dstp_2[c]
        m["ewb_2"] = ewb_2[c]
        m["ewbn_2"] = ewbn_2[c]
        m["idx16_c"] = idx16_c[c]
        in_maps.append(m)
    return meta, in_maps


# ------------------------------------------------------------- device kernel

def _build(meta, NCLS=10, H=64):
    import concourse.bass as bass
    import concourse.bacc as bacc
    import concourse.mybir as mybir
    import concourse.tile as tile
    from concourse.masks import make_identity
    from concourse import library_config

    f32 = mybir.dt.float32
    bf16 = mybir.dt.bfloat16
    i16 = mybir.dt.int16
    ALU = mybir.AluOpType
    ACTF = mybir.ActivationFunctionType
    AX = mybir.AxisListType

    B = meta["B"]
    T1, T2 = meta["T1"], meta["T2"]
    rows1, rows2 = meta["rows1"], meta["rows2"]
    FIN = meta["FIN"]
    mp1, mp2, cov = meta["mp1"], meta["mp2"], meta["cov"]
    RG = [list(range(NCORES))]

    nc = bacc.Bacc("TRN2", target_bir_lowering=False, debug=False,
                   num_devices=NCORES, num_swdge_queues=4)

    ein = lambda n, s, d=f32: nc.dram_tensor(n, s, d, kind="ExternalInput")
    x_cT = ein("x_cT", [FIN, rows1])
    mask1 = ein("mask1", [PART, T1]); mask2 = ein("mask2", [PART, T2])
    dis1_d = ein("dis1", [PART, T1]); dis2_d = ein("dis2", [PART, T2])
    idx16_1 = ein("idx16_1", [PART, mp1["S"] // 16], i16)
    dstp_1 = ein("dstp_1", [PART, max(mp1["nruns"], 1)])
    ewb_1 = ein("ewb_1", [PART, max(mp1["nblk"], 1)])
    ewbn_1 = ein("ewbn_1", [PART, max(mp1["nblk"], 1)])
    idx16_2 = ein("idx16_2", [PART, mp2["S"] // 16], i16)
    dstp_2 = ein("dstp_2", [PART, max(mp2["nruns"], 1)])
    ewb_2 = ein("ewb_2", [PART, max(mp2["nblk"], 1)])
    ewbn_2 = ein("ewbn_2", [PART, max(mp2["nblk"], 1)])
    idx16_c = ein("idx16_c", [PART, cov["S"] // 16], i16)
    iota_d = ein("iota", [PART, PART])
    iotan_d = ein("iotan", [PART, PART])
    wshapes = {"W_in0": [FIN, H], "W_in1": [H, H], "Wl_in": [2 * H, H],
               "W_b0": [2 * H, H], "W_b1": [H, H], "Wl_b": [2 * H, H]}
    Ws = {n: ein(n, s) for n, s in wshapes.items()}
    bs = {n: ein(n, [PART, H]) for n in
          ("b_in0", "b_in1", "bl_in", "b_b0", "b_b1", "bl_b")}
    gammaS = ein("gammaS", [PART, 1]); gammaM = ein("gammaM", [PART, 1])
    betaS = ein("betaS", [PART, 1]); betaM = ein("betaM", [PART, 1])
    l1WS = ein("l1WS", [PART, H]); l1WM = ein("l1WM", [PART, H])
    l1b = ein("l1b", [PART, H])
    l2W = ein("l2W", [H, NCLS]); l2b = ein("l2b", [PART, NCLS])
    out_ext = nc.dram_tensor("out", [B, NCLS], f32, kind="ExternalOutput")

    # internal DRAM: wide bf16 tables (upper half junk, never read)
    hs_c1 = nc.dram_tensor("hs_c1", [rows1, WIDE], bf16)
    hs_full1 = nc.dram_tensor("hs_full1", [NCORES * rows1, WIDE], bf16, addr_space="Shared")
    hs_c1b = nc.dram_tensor("hs_c1b", [rows1, WIDE], bf16)
    hs_full1b = nc.dram_tensor("hs_full1b", [NCORES * rows1, WIDE], bf16, addr_space="Shared")
    x1_c = nc.dram_tensor("x1_c", [rows1, WIDE], bf16)
    x1_full = nc.dram_tensor("x1_full", [NCORES * rows1, WIDE], bf16, addr_space="Shared")
    hs_c2 = nc.dram_tensor("hs_c2", [rows2, WIDE], bf16)
    hs_full2 = nc.dram_tensor("hs_full2", [NCORES * rows2, WIDE], bf16, addr_space="Shared")
    hs_c2b = nc.dram_tensor("hs_c2b", [rows2, WIDE], bf16)
    hs_full2b = nc.dram_tensor("hs_full2b", [NCORES * rows2, WIDE], bf16, addr_space="Shared")
    arS_in = nc.dram_tensor("arS_in", [PART, B], f32)
    arS_out = nc.dram_tensor("arS_out", [PART, B], f32, addr_space="Shared")
    arM_in = nc.dram_tensor("arM_in", [PART, B], f32)
    arM_out = nc.dram_tensor("arM_out", [PART, B], f32, addr_space="Shared")

    with tile.TileContext(nc) as tc:
        nc.gpsimd.load_library(library_config.mlp)
        with (tc.tile_pool(name="const", bufs=1) as cpool,
              tc.tile_pool(name="res", bufs=1) as rpool,
              tc.tile_pool(name="gtp", bufs=3) as gtpool,
              tc.tile_pool(name="stg", bufs=3) as stgpool,
              tc.tile_pool(name="work", bufs=2) as wpool,
              tc.tile_pool(name="ps", bufs=3, space="PSUM") as pspool,
              tc.tile_pool(name="psacc", bufs=1, space="PSUM") as papool):

            ident = cpool.tile([PART, PART], f32, tag="ident")
            make_identity(nc, ident[:])

            def load2d(dram, shape, dt=f32, tag=None):
                t = cpool.tile(list(shape), dt, tag=tag or dram.name)
                nc.sync.dma_start(t[:], dram[:, :])
                return t

            identB = cpool.tile([PART, PART], bf16, tag="identB")
            make_identity(nc, identB[:])
            mask1_s = load2d(mask1, (PART, T1))
            mask2_s = load2d(mask2, (PART, T2))
            dis1 = load2d(dis1_d, (PART, T1), tag="dis1s")
            dis2 = load2d(dis2_d, (PART, T2), tag="dis2s")
            iota_s = load2d(iota_d, (PART, PART))
            iotan_s = load2d(iotan_d, (PART, PART), tag="iotan")
            W_s = {n: load2d(Ws[n], Ws[n].shape) for n in Ws}
            b_s = {n: load2d(bs[n], (PART, H)) for n in bs}
            l1WS_s = load2d(l1WS, (PART, H)); l1WM_s = load2d(l1WM, (PART, H))
            l1b_s = load2d(l1b, (PART, H))
            l2W_s = load2d(l2W, (H, NCLS)); l2b_s = load2d(l2b, (PART, NCLS))
            gS_s = load2d(gammaS, (PART, 1)); gM_s = load2d(gammaM, (PART, 1))
            bS_s = load2d(betaS, (PART, 1)); bM_s = load2d(betaM, (PART, 1))

            # f32 activation accumulators: [..., 0, :] = layer a / cover sum,
            # [..., 1, :] = layer b / cover max
            acc1 = rpool.tile([PART, T1, 2, H], f32, tag="acc1")
            acc2 = rpool.tile([PART, T2, 2, H], f32, tag="acc2")
            hs1_sb = rpool.tile([PART, T1, H], bf16, tag="hs1_sb")
            hs2_sb = rpool.tile([PART, T2, H], bf16, tag="hs2_sb")
            rm1 = rpool.tile([PART, B, H], f32, tag="rm1")
            rm2 = rpool.tile([PART, B, H], f32, tag="rm2")
            oneh = rpool.tile([PART, B, B], f32, tag="oneh")
            nc.vector.memset(rm1[:], 0.0)
            nc.vector.memset(rm2[:], 0.0)
            nc.vector.memset(oneh[:], 0.0)
            for g in range(B):
                nc.vector.memset(oneh[:, g, g:g + 1], 1.0)

            def bc_mid(ap2d, G):
                a = ap2d.ap
                return bass.AP(ap2d.tensor, ap2d.offset,
                               [a[0], [0, G], a[-1]])

            ps_sum1 = papool.tile([B, H], f32, tag="sum1")
            ps_sum2 = papool.tile([B, H], f32, tag="sum2")

            def stripes(T, step):
                return [(s, min(s + step, T)) for s in range(0, T, step)]

            # ---- dense matmul phase: hs = dis * (act @ W) -> SBUF + DRAM ----
            def mm_phase(lhsT_fn, Tn, W, dis_t, hs_sb, hs_dram):
                hsd = hs_dram.ap().rearrange("(t p) f -> p t f", p=PART)
                for (s0, s1) in stripes(Tn, STRIPE):
                    for t in range(s0, s1):
                        lhsT = lhsT_fn(t)
                        mm = pspool.tile([PART, H], f32, tag="mm")
                        nc.tensor.matmul(out=mm[:], lhsT=lhsT, rhs=W[:],
                                         start=True, stop=True)
                        nc.vector.tensor_scalar(
                            out=hs_sb[:, t, :], in0=mm[:],
                            scalar1=dis_t[:, t:t + 1], scalar2=None,
                            op0=ALU.mult)
                    nc.sync.dma_start(hsd[:, s0:s1, 0:H], hs_sb[:, s0:s1, :])

            def lhsT_transpose(src_fn, kdim):
                def fn(t):
                    tp = pspool.tile([PART, PART], f32, tag="tp")
                    nc.tensor.transpose(tp[:kdim, :], src_fn(t), ident[:])
                    tsb = wpool.tile([PART, PART], f32, tag="tsb", bufs=5)
                    nc.scalar.copy(out=tsb[:kdim, :], in_=tp[:kdim, :])
                    return tsb[:kdim, :]
                return fn

            xTv = x_cT.ap()
            _xc = {}

            def lhsT_x(t):
                s0 = (t // STRIPE_X) * STRIPE_X
                if s0 not in _xc:
                    xstg = stgpool.tile([FIN, STRIPE_X * PART], f32,
                                        tag="xstg")
                    s1 = min(s0 + STRIPE_X, T1)
                    nc.sync.dma_start(xstg[:, :(s1 - s0) * PART],
                                      xTv[:, s0 * PART:s1 * PART])
                    _xc[s0] = xstg
                return _xc[s0][:, (t - s0) * PART:(t - s0 + 1) * PART]

            def allgather(src, dst):
                nc.gpsimd.collective_compute(
                    "AllGather", ALU.bypass, ins=[src.ap().opt()],
                    outs=[dst.ap().opt()], replica_groups=RG)

            # ---- mp phase: tile-major super-chunks; per tile one PSUM
            # accumulation group (self matmul + one matmul per run), fused
            # epilogue acc = relu((sum + hs_self)*dis + bias) ----
            _qrot = [0]

            def mp_phase(mp, hs_full, idx16_d, dstp_d, ewb_d, ewbn_d, rows,
                         acc, half, hs_sb, dis_t, bias):
                runs_all = mp["runs_all"]
                mrun, mblk = mp["mrun"], mp["mblk"]
                mctr = 0
                for sch in mp["schunks"]:
                    r0, r1 = sch["r0"], sch["r1"]
                    b0, b1 = sch["b0"], sch["b1"]
                    dst_t = stgpool.tile([PART, mrun], f32, tag="dstpstg")
                    nc.sync.dma_start(dst_t[:, :r1 - r0], dstp_d[:, r0:r1])
                    ew_t = stgpool.tile([PART, mblk], f32, tag="ewstg")
                    nc.sync.dma_start(ew_t[:, :b1 - b0], ewb_d[:, b0:b1])
                    ewn_t = stgpool.tile([PART, mblk], f32, tag="ewnstg")
                    nc.sync.dma_start(ewn_t[:, :b1 - b0], ewbn_d[:, b0:b1])
                    gts = []
                    for (c, a, npad) in sch["subs"]:
                        idxt = stgpool.tile([PART, NIDX_MAX // 16], i16,
                                            tag="idxstg", name="idxt",
                                            bufs=12)
                        nc.sync.dma_start(
                            idxt[:, :npad // 16],
                            idx16_d[:, a // 16:(a + npad) // 16])
                        gt = gtpool.tile([PART, NIDX_MAX // 128, WIDE],
                                         bf16, tag="gt", name="gt", bufs=8)
                        nc.gpsimd.dma_gather(
                            gt[:, :npad // 128, :],
                            hs_full[c * rows:(c + 1) * rows, :],
                            idxt[:, :npad // 16], npad, npad, WIDE,
                            queue_num=_qrot[0] % 4)
                        _qrot[0] += 1
                        gts.append(gt)
                    for t in range(sch["t0"], sch["t1"]):
                        ops = sch["tile_ops"][t]
                        ps = pspool.tile([PART, H], f32, tag="mm")
                        nc.tensor.matmul(out=ps[:], lhsT=identB[:],
                                         rhs=hs_sb[:, t, :], start=True,
                                         stop=(len(ops) == 0),
                                         skip_group_check=True)
                        for j, (sub_id, bl, rid) in enumerate(ops):
                            be = runs_all[rid][0]
                            M = wpool.tile([PART, PART], bf16, tag="M", bufs=6)
                            if mctr % 2 == 0:
                                nc.vector.tensor_scalar(
                                    out=M[:], in0=iotan_s[:],
                                    scalar1=dst_t[:, rid - r0:rid - r0 + 1],
                                    scalar2=ew_t[:, be - b0:be - b0 + 1],
                                    op0=ALU.is_equal, op1=ALU.mult)
                            else:
                                msq = wpool.tile([PART, PART], f32,
                                                 tag="msq", bufs=2)
                                nc.scalar.activation(
                                    msq[:], iota_s[:], ACTF.Square,
                                    bias=dst_t[:, rid - r0:rid - r0 + 1])
                                nc.scalar.activation(
                                    M[:], msq[:], ACTF.Relu,
                                    bias=ew_t[:, be - b0:be - b0 + 1],
                                    scale=ewn_t[:, be - b0:be - b0 + 1])
                            mctr += 1
                            nc.tensor.matmul(out=ps[:], lhsT=M[:],
                                             rhs=gts[sub_id][:, bl, 0:H],
                                             start=False,
                                             stop=(j == len(ops) - 1),
                                             skip_group_check=True)
                        ept = wpool.tile([PART, H], f32, tag="ept", bufs=5)
                        nc.vector.tensor_scalar(
                            out=ept[:], in0=ps[:],
                            scalar1=dis_t[:, t:t + 1], scalar2=None,
                            op0=ALU.mult)
                        nc.vector.tensor_tensor(out=ept[:], in0=ept[:],
                                                in1=bias[:], op=ALU.add)
                        nc.scalar.activation(acc[:, t, half, :], ept[:],
                                             ACTF.Relu)

            # ---- jk: cat(a,b) @ Wl + bias, relu*mask, readouts ----
            def jk_phase(acc, Tn, Wl, bias, mask_s, tg, ps_sum, rm, x_dram):
                lfn = lhsT_transpose(
                    lambda t: acc[:, t, :, :].rearrange("p a b -> p (a b)"),
                    PART)
                xd = (x_dram.ap().rearrange("(t p) f -> p t f", p=PART)
                      if x_dram is not None else None)
                for (s0, s1) in stripes(Tn, EPI_CH):
                    stg = (stgpool.tile([PART, EPI_CH, H], bf16, tag="x1stg",
                                        name="stg")
                           if x_dram is not None else None)
                    for t in range(s0, s1):
                        lhsT = lfn(t)
                        mm = pspool.tile([PART, H], f32, tag="mm")
                        nc.tensor.matmul(out=mm[:], lhsT=lhsT, rhs=Wl[:],
                                         start=True, stop=True)
                        xt = wpool.tile([PART, H], f32, tag="xt", bufs=5)
                        nc.vector.tensor_tensor(out=xt[:], in0=mm[:],
                                                in1=bias[:], op=ALU.add)
                        nc.scalar.activation(xt[:], xt[:], ACTF.Relu,
                                             scale=mask_s[:, t:t + 1])
                        g = int(tg[t])
                        nc.tensor.matmul(out=ps_sum[:], lhsT=oneh[:, g, :],
                                         rhs=xt[:], start=(t == 0),
                                         stop=(t == Tn - 1),
                                         skip_group_check=True)
                        nc.vector.tensor_tensor(out=rm[:, g, :],
                                                in0=rm[:, g, :],
                                                in1=xt[:], op=ALU.max)
                        if stg is not None:
                            nc.scalar.copy(out=stg[:, t - s0, :], in_=xt[:])
                    if stg is not None:
                        nc.sync.dma_start(xd[:, s0:s1, 0:H],
                                          stg[:, :s1 - s0, :])

            # ================= pipeline =================
            with nc.named_scope("mm1a"):
                mm_phase(lhsT_x, T1, W_s["W_in0"], dis1, hs1_sb, hs_c1)
            with nc.named_scope("ag1a"):
                allgather(hs_c1, hs_full1)
            with nc.named_scope("mp1a"):
                mp_phase(mp1, hs_full1, idx16_1, dstp_1, ewb_1, ewbn_1,
                         rows1, acc1, 0, hs1_sb, dis1, b_s["b_in0"])
            with nc.named_scope("mm1b"):
                mm_phase(lhsT_transpose(lambda t: acc1[:, t, 0, :], H),
                         T1, W_s["W_in1"], dis1, hs1_sb, hs_c1b)
            with nc.named_scope("ag1b"):
                allgather(hs_c1b, hs_full1b)
            with nc.named_scope("mp1b"):
                mp_phase(mp1, hs_full1b, idx16_1, dstp_1, ewb_1, ewbn_1,
                         rows1, acc1, 1, hs1_sb, dis1, b_s["b_in1"])
            with nc.named_scope("jk1"):
                jk_phase(acc1, T1, W_s["Wl_in"], b_s["bl_in"], mask1_s,
                         meta["tg1"], ps_sum1, rm1, x1_c)
            with nc.named_scope("agx1"):
                allgather(x1_c, x1_full)

            # ---------- cover pooling: sum -> acc1[...,0], max -> [...,1]
            # (acc1 is free after jk1; reuse its first T2 tiles)
            with nc.named_scope("cover"):
                k_tc = cov["k_tc"]; col_off = cov["col_off"]
                written = set()
                for (c, c0, c1) in cov["chunks"]:
                    ncols = c1 - c0
                    n = ncols * 128
                    idxt = stgpool.tile([PART, NIDX_MAX // 16], i16,
                                        tag="idxstg", bufs=12)
                    nc.sync.dma_start(idxt[:, :n // 16],
                                      idx16_c[:, c0 * 8:c1 * 8])
                    gt = gtpool.tile([PART, NIDX_MAX // 128, WIDE], bf16,
                                     tag="gt", bufs=8)
                    nc.gpsimd.dma_gather(
                        gt[:, :ncols, :],
                        x1_full[c * rows1:(c + 1) * rows1, :],
                        idxt[:, :n // 16], n, n, WIDE,
                        queue_num=_qrot[0] % 4)
                    _qrot[0] += 1
                    for t in range(T2):
                        k = int(k_tc[t, c])
                        if k == 0:
                            continue
                        ca = int(col_off[t, c]) - c0
                        if ca < 0 or ca + k > ncols:
                            continue
                        view = gt[:, ca:ca + k, 0:H].rearrange(
                            "p k f -> p f k")
                        zs = wpool.tile([PART, H], f32, tag="zs", bufs=5)
                        nc.vector.tensor_reduce(out=zs[:], in_=view,
                                                axis=AX.X, op=ALU.add)
                        zm = wpool.tile([PART, H], f32, tag="zm", bufs=5)
                        nc.vector.tensor_reduce(out=zm[:], in_=view,
                                                axis=AX.X, op=ALU.max)
                        if t in written:
                            nc.vector.tensor_tensor(
                                out=acc1[:, t, 0, :], in0=acc1[:, t, 0, :],
                                in1=zs[:], op=ALU.add)
                            nc.vector.tensor_tensor(
                                out=acc1[:, t, 1, :], in0=acc1[:, t, 1, :],
                                in1=zm[:], op=ALU.max)
                        else:
                            nc.vector.tensor_copy(acc1[:, t, 0, :], zs[:])
                            nc.vector.tensor_copy(acc1[:, t, 1, :], zm[:])
                            written.add(t)
                for t in range(T2):
                    if t not in written:
                        nc.vector.memset(acc1[:, t, 0, :], 0.0)
                        nc.vector.memset(acc1[:, t, 1, :], 0.0)

            # ---------- pooled block ----------
            with nc.named_scope("mm2a"):
                mm_phase(lhsT_transpose(
                    lambda t: acc1[:, t, :, :].rearrange("p a b -> p (a b)"),
                    PART), T2, W_s["W_b0"], dis2, hs2_sb, hs_c2)
            with nc.named_scope("ag2a"):
                allgather(hs_c2, hs_full2)
            with nc.named_scope("mp2a"):
                mp_phase(mp2, hs_full2, idx16_2, dstp_2, ewb_2, ewbn_2,
                         rows2, acc2, 0, hs2_sb, dis2, b_s["b_b0"])
            with nc.named_scope("mm2b"):
                mm_phase(lhsT_transpose(lambda t: acc2[:, t, 0, :], H),
                         T2, W_s["W_b1"], dis2, hs2_sb, hs_c2b)
            with nc.named_scope("ag2b"):
                allgather(hs_c2b, hs_full2b)
            with nc.named_scope("mp2b"):
                mp_phase(mp2, hs_full2b, idx16_2, dstp_2, ewb_2, ewbn_2,
                         rows2, acc2, 1, hs2_sb, dis2, b_s["b_b1"])
            with nc.named_scope("jk2"):
                jk_phase(acc2, T2, W_s["Wl_b"], b_s["bl_b"], mask2_s,
                         meta["tg2"], ps_sum2, rm2, None)

            # ---------- readout combine + head ----------
            sc_head = nc.named_scope("head"); sc_head.__enter__()
            sum1_sb = wpool.tile([B, H], f32, tag="s1sb")
            nc.scalar.copy(out=sum1_sb[:], in_=ps_sum1[:])
            sum2_sb = wpool.tile([B, H], f32, tag="s2sb")
            nc.scalar.copy(out=sum2_sb[:], in_=ps_sum2[:])
            sT = pspool.tile([H, B], f32, tag="tp")
            nc.tensor.matmul(out=sT[:], lhsT=sum1_sb[:], rhs=ident[:B, :B],
                             start=True, stop=True)
            sT1 = wpool.tile([H, B], f32, tag="sT1")
            nc.scalar.copy(out=sT1[:], in_=sT[:])
            sT_2 = pspool.tile([H, B], f32, tag="tp")
            nc.tensor.matmul(out=sT_2[:], lhsT=sum2_sb[:], rhs=ident[:B, :B],
                             start=True, stop=True)
            sT2 = wpool.tile([H, B], f32, tag="sT2")
            nc.scalar.copy(out=sT2[:], in_=sT_2[:])
            nc.sync.dma_start(arS_in[0:H, :], sT1[:])
            nc.sync.dma_start(arS_in[H:2 * H, :], sT2[:])

            mT1 = wpool.tile([H, B], f32, tag="mT1")
            mT2 = wpool.tile([H, B], f32, tag="mT2")
            for g in range(B):
                for rm, mt in ((rm1, mT1), (rm2, mT2)):
                    tpm = pspool.tile([H, PART], f32, tag="tp")
                    nc.tensor.transpose(tpm[:], rm[:, g, :], ident[:])
                    msb = wpool.tile([H, PART], f32, tag="msb")
                    nc.scalar.copy(out=msb[:], in_=tpm[:])
                    nc.vector.tensor_reduce(out=mt[:, g:g + 1], in_=msb[:],
                                            axis=AX.X, op=ALU.max)
            nc.sync.dma_start(arM_in[0:H, :], mT1[:])
            nc.sync.dma_start(arM_in[H:2 * H, :], mT2[:])

            nc.gpsimd.collective_compute(
                "AllReduce", ALU.add, ins=[arS_in.ap().opt()],
                outs=[arS_out.ap().opt()], replica_groups=RG)
            nc.gpsimd.collective_compute(
                "AllReduce", ALU.max, ins=[arM_in.ap().opt()],
                outs=[arM_out.ap().opt()], replica_groups=RG)

            S_sb = wpool.tile([PART, B], f32, tag="Ssb")
            M_sb = wpool.tile([PART, B], f32, tag="Msb")
            nc.sync.dma_start(S_sb[:], arS_out[:, :])
            nc.sync.dma_start(M_sb[:], arM_out[:, :])

            def bn(t_sb, gam, bet):
                mu = wpool.tile([PART, 1], f32, tag="mu")
                nc.vector.tensor_reduce(out=mu[:], in_=t_sb[:], axis=AX.X,
                                        op=ALU.add)
                nc.vector.tensor_scalar_mul(mu[:], mu[:], 1.0 / B)
                nc.vector.tensor_scalar(out=t_sb[:], in0=t_sb[:],
                                        scalar1=mu[:], scalar2=None,
                                        op0=ALU.subtract)
                sq = wpool.tile([PART, B], f32, tag="sq")
                nc.vector.tensor_tensor(out=sq[:], in0=t_sb[:], in1=t_sb[:],
                                        op=ALU.mult)
                var = wpool.tile([PART, 1], f32, tag="var")
                nc.vector.tensor_reduce(out=var[:], in_=sq[:], axis=AX.X,
                                        op=ALU.add)
                nc.vector.tensor_scalar(out=var[:], in0=var[:],
                                        scalar1=1.0 / B, scalar2=EPS,
                                        op0=ALU.mult, op1=ALU.add)
                nc.scalar.activation(var[:], var[:], ACTF.Sqrt)
                nc.vector.reciprocal(var[:], var[:])
                nc.vector.tensor_scalar(out=t_sb[:], in0=t_sb[:],
                                        scalar1=var[:], scalar2=gam[:],
                                        op0=ALU.mult, op1=ALU.mult)
                nc.vector.tensor_scalar(out=t_sb[:], in0=t_sb[:],
                                        scalar1=bet[:], scalar2=None,
                                        op0=ALU.add)

            bn(S_sb, gS_s, bS_s)
            bn(M_sb, gM_s, bM_s)

            pl1 = pspool.tile([B, H], f32, tag="mm")
            nc.tensor.matmul(out=pl1[:], lhsT=S_sb[:], rhs=l1WS_s[:],
                             start=True, stop=False)
            nc.tensor.matmul(out=pl1[:], lhsT=M_sb[:], rhs=l1WM_s[:],
                             start=False, stop=True)
            y = wpool.tile([B, H], f32, tag="y")
            nc.vector.tensor_tensor(out=y[:], in0=pl1[:], in1=l1b_s[:B, :],
                                    op=ALU.add)
            nc.scalar.activation(y[:], y[:], ACTF.Relu)
            yT_ps = pspool.tile([H, B], f32, tag="tp")
            nc.tensor.matmul(out=yT_ps[:], lhsT=y[:], rhs=ident[:B, :B],
                             start=True, stop=True)
            yT = wpool.tile([H, B], f32, tag="yTs")
            nc.scalar.copy(out=yT[:], in_=yT_ps[:])
            pl2 = pspool.tile([B, NCLS], f32, tag="mm")
            nc.tensor.matmul(out=pl2[:], lhsT=yT[:], rhs=l2W_s[:],
                             start=True, stop=True)
            z = wpool.tile([B, NCLS], f32, tag="z")
            nc.vector.tensor_tensor(out=z[:], in0=pl2[:], in1=l2b_s[:B, :],
                                    op=ALU.add)
            zmax = wpool.tile([B, 1], f32, tag="zmax")
            nc.vector.tensor_reduce(out=zmax[:], in_=z[:], axis=AX.X,
                                    op=ALU.max)
            nc.vector.tensor_scalar(out=z[:], in0=z[:], scalar1=zmax[:],
                                    scalar2=None, op0=ALU.subtract)
            nc.scalar.activation(z[:], z[:], ACTF.Exp)
            zsum = wpool.tile([B, 1], f32, tag="zsum")
            nc.vector.tensor_reduce(out=zsum[:], in_=z[:], axis=AX.X,
                                    op=ALU.add)
            nc.vector.reciprocal(zsum[:], zsum[:])
            nc.vector.tensor_scalar(out=z[:], in0=z[:], scalar1=zsum[:],
                                    scalar2=None, op0=ALU.mult)
            nc.sync.dma_start(out_ext[:, :], z[:])
            sc_head.__exit__(None, None, None)

    nc.compile()
    return nc


def kernel(**inputs):
    from concourse import bass_utils
    meta, in_maps = _prep(inputs)
    nc = _build(meta)
    res = bass_utils.run_bass_kernel_spmd(
        nc, in_maps, core_ids=list(range(NCORES)))
    return np.asarray(res.results[0]["out"])


# revision 45
# speedup vs baseline: 1.0509x; 1.0509x over previous
"""Bass/Trainium2 kernel for nn_KPlexPool (GCN blocks + cover pooling), 8 NeuronCores.

v3: message-passing gathers use dma_gather (one SWDGE instruction per <=4096
slots instead of one per 128 rows), with slot streams sorted by (src_core,
dst) so each instruction's int16 indices fit one core's 32K-row window of the
AllGathered bf16 table.  Segment sums are done by TensorE: per 128-slot block
a one-hot matrix M[slot, dst_partition] = (iota==dstp)*ew is built in one DVE
tensor_scalar op and matmul'd against the gathered rows, accumulating into a
f32 SBUF accumulator per tile.  Cover pooling keeps dst-aligned CSR (needed
for max) bucketed by source core, gathered with dma_gather + DVE reduces.
Activations stay f32 in SBUF (dense matmuls fp32); only the gathered hs/x1
tables, their AllGathers, and the one-hot ew are bf16.  The program is
SPMD-uniform: all slot counts are padded to per-(tile, src_core) maxima over
cores; per-core data (indices, dstp, ew) carries the differences.
"""

import sys
import numpy as np
import ml_dtypes

sys.path.insert(0, "/opt/trn_rl_repo")

PART = 128
NCORES = 8
EPS = 1e-5
NIDX_MAX = 1024      # slots per dma_gather (ring-limited: 2048+ wedges SWDGE)
EPI_CH = 16          # tiles per epilogue chunk
STRIPE = 32          # tiles per hs DMA stripe
STRIPE_X = 8         # tiles per x lhsT load stripe

BF16 = ml_dtypes.bfloat16
WIDE = 128           # gathered-table row width (bf16 -> 256B rows)


# ----------------------------------------------------------------- host prep

def _shard_items(batch, sortkey, B):
    n = batch.shape[0]
    counts = np.bincount(batch, minlength=B)
    starts = np.concatenate([[0], np.cumsum(counts)[:-1]])
    perm_cores = [[] for _ in range(NCORES)]
    tile_graph = []
    for g in range(B):
        cnt = int(counts[g])
        st = int(starts[g])
        base, rem = divmod(cnt, NCORES)
        sizes = [base + (1 if c < rem else 0) for c in range(NCORES)]
        Tg = max(1, -(-max(sizes) // PART))
        tile_graph += [g] * Tg
        off = st
        for c in range(NCORES):
            s = sizes[c]
            ids = np.arange(off, off + s)
            off += s
            order = np.argsort(-sortkey[ids], kind="stable")
            padded = np.full(Tg * PART, -1, dtype=np.int64)
            padded[:s] = ids[order]
            perm_cores[c].append(padded)
    perm = [np.concatenate(p) for p in perm_cores]
    rows = perm[0].shape[0]
    pos = np.full(n, -1, dtype=np.int64)
    for c in range(NCORES):
        real = perm[c] >= 0
        pos[perm[c][real]] = c * rows + np.nonzero(real)[0]
    return perm, pos, rows, np.asarray(tile_graph)


def _wrap16(flat):
    """idx stream [n] -> dma_gather idx layout [128, n/16] int16."""
    n = flat.shape[0]
    assert n % 16 == 0
    w = np.zeros((16, n // 16), dtype=np.int16)
    w[np.arange(n) % 16, np.arange(n) // 16] = flat
    return np.tile(w, (8, 1))


def _mp_stream(dst_loc, src_pos, w, rows, T):
    """(src_core, dst)-sorted slot stream, chunked into tile-major
    super-chunks of 8 per-core sub-gathers so each tile's segment-sum is one
    PSUM accumulation group.

    Returns program-uniform meta + per-core data (idx16, dstpn, ewb, ewbn).
    """
    cnt = np.zeros((NCORES, T, NCORES), np.int64)
    percore = []
    for me in range(NCORES):
        dl, sp, ww = dst_loc[me], src_pos[me], w[me]
        sc = sp // rows
        np.add.at(cnt[me], (dl // PART, sc), 1)
        percore.append((dl, sp, ww, sc))
    n_tc = cnt.max(axis=0)  # [T, NCORES]
    n_tc = ((n_tc + 15) // 16) * 16          # x16 so idx slices stay aligned

    sec_len = n_tc.sum(axis=0) + 128         # +128 slack for x128 rounding
    sec_len_p = ((sec_len + 127) // 128) * 128
    sec_off = np.concatenate([[0], np.cumsum(sec_len_p)])
    S = int(sec_off[-1])

    tc_off = np.zeros((T, NCORES), np.int64)
    for c in range(NCORES):
        off = int(sec_off[c])
        for t in range(T):
            tc_off[t, c] = off
            off += int(n_tc[t, c])

    # super-chunks: tile ranges where every core's sub-range <= NIDX_MAX-127
    lim = NIDX_MAX - 127
    tranges = []
    t0 = 0
    while t0 < T:
        t1 = t0 + 1
        while (t1 < T and t1 - t0 < EPI_CH
               and int(n_tc[t0:t1 + 1].sum(axis=0).max()) <= lim):
            t1 += 1
        tranges.append((t0, t1))
        t0 = t1

    # emission: per super-chunk, per core: sub-gather (a, npad); blocks are
    # instruction-local; runs = (sub, blk_local, tile, lo, hi) -> emission ids
    schunks = []
    nblk_em = 0
    runs_all = []  # run id -> (blk_em, tile, lo, hi, abs_lo)
    for (t0, t1) in tranges:
        subs = []
        tile_ops = {t: [] for t in range(t0, t1)}
        r0, b0 = len(runs_all), nblk_em
        for c in range(NCORES):
            a = int(tc_off[t0, c])
            b = (int(tc_off[t1, c]) if t1 < T
                 else int(tc_off[t1 - 1, c] + n_tc[t1 - 1, c]))
            npad = ((b - a + 127) // 128) * 128
            assert a + npad <= int(sec_off[c + 1]), "section slack exceeded"
            sub_id = len(subs)
            subs.append((c, a, npad))
            for t in range(t0, t1):
                lo = int(tc_off[t, c]) - a
                hi = lo + int(n_tc[t, c])
                s = lo
                while s < hi:
                    bl = s // 128
                    e = min(hi, (bl + 1) * 128)
                    runs_all.append((nblk_em + bl, t, s - bl * 128,
                                     e - bl * 128, a + s))
                    tile_ops[t].append((sub_id, bl, len(runs_all) - 1))
                    s = e
            nblk_em += npad // 128
        schunks.append(dict(t0=t0, t1=t1, subs=subs, tile_ops=tile_ops,
                            r0=r0, r1=len(runs_all), b0=b0, b1=nblk_em))
    nruns_em = len(runs_all)
    mrun = max(1, max(s["r1"] - s["r0"] for s in schunks))
    mblk = max(1, max(s["b1"] - s["b0"] for s in schunks))

    # per-core data tables in emission layout
    idx16s, dstpns, ewbs, ewbns = [], [], [], []
    for me in range(NCORES):
        dl, sp, ww, sc = percore[me]
        order = np.lexsort((dl, sc))
        dls, sps, wws, scs = (a[order] for a in (dl, sp, ww, sc))
        idx = np.zeros(S, np.int16)
        dstp_sl = np.full(S, 255.0, np.float32)
        ew_sl = np.zeros(S, np.float32)
        ptr = 0
        for c in range(NCORES):
            for t in range(T):
                k = int(cnt[me, t, c])
                if k:
                    o = int(tc_off[t, c])
                    sl = slice(ptr, ptr + k)
                    idx[o:o + k] = (sps[sl] % rows).astype(np.int16)
                    dstp_sl[o:o + k] = (dls[sl] % PART).astype(np.float32)
                    ew_sl[o:o + k] = wws[sl]
                    ptr += k
        assert ptr == dls.shape[0]
        dstpn = np.full((PART, max(nruns_em, 1)), -255.0, np.float32)
        for r, (be, t, lo, hi, abs_lo) in enumerate(runs_all):
            dstpn[lo:hi, r] = -dstp_sl[abs_lo:abs_lo + (hi - lo)]
        # per-emission-block ew columns
        ewb = np.zeros((PART, max(nblk_em, 1)), np.float32)
        for sch in schunks:
            be = sch["b0"]
            for (c, a, npad) in sch["subs"]:
                for bl in range(npad // 128):
                    ewb[:, be] = ew_sl[a + bl * 128:a + (bl + 1) * 128]
                    be += 1
        idx16s.append(_wrap16(idx))
        dstpns.append(dstpn)
        ewbs.append(ewb)
        ewbns.append(-ewb)
    meta = dict(S=S, nblk=nblk_em, nruns=nruns_em, schunks=schunks,
                runs_all=runs_all, mrun=mrun, mblk=mblk)
    return meta, idx16s, dstpns, ewbs, ewbns


def _cover_stream(cl_loc, src_pos, rows1, rows2, T2, pad_local):
    """Dst-aligned per-src-core-bucketed CSR for cover sum+max.

    Returns meta (k_tc [T2, NCORES], chunks, col layout) + per-core idx16."""
    cnt_pc = np.zeros((NCORES, T2 * PART, NCORES), np.int32)
    percore = []
    for me in range(NCORES):
        cl, sp = cl_loc[me], src_pos[me]
        sc = sp // rows1
        np.add.at(cnt_pc[me], (cl, sc), 1)
        percore.append((cl, sp, sc))
    # k per (tile, src core): max over partitions and cores
    k_tc = cnt_pc.reshape(NCORES, T2, PART, NCORES).max(axis=(0, 2))  # [T2, C]
    sec_cols = k_tc.sum(axis=0)  # columns per section
    col_off = np.zeros((T2, NCORES), np.int64)
    acc = 0
    sec_col0 = np.zeros(NCORES + 1, np.int64)
    for c in range(NCORES):
        sec_col0[c] = acc
        for t in range(T2):
            col_off[t, c] = acc
            acc += int(k_tc[t, c])
    sec_col0[NCORES] = acc
    Scols = acc
    S = Scols * 128

    # chunks cut only at whole (t, c) range boundaries so a tile's columns
    # never straddle two gathers
    chunks = []  # (c, col0, col1)
    maxcols = NIDX_MAX // 128
    for c in range(NCORES):
        c0 = int(sec_col0[c])
        cur = c0
        for t in range(T2):
            k = int(k_tc[t, c])
            if cur + k - c0 > maxcols and cur > c0:
                chunks.append((c, c0, cur))
                c0 = cur
            cur += k
        if cur > c0:
            chunks.append((c, c0, cur))

    idx16s = []
    for me in range(NCORES):
        cl, sp, sc = percore[me]
        idx_cols = np.full((PART, Scols), -1, np.int64)
        for c in range(NCORES):
            m = sc == c
            cls, sps = cl[m], sp[m]
            order = np.argsort(cls, kind="stable")
            cls, sps = cls[order], sps[order]
            ccount = np.bincount(cls, minlength=T2 * PART)
            first = np.concatenate([[0], np.cumsum(ccount)[:-1]])
            rank = np.arange(cls.shape[0]) - first[cls]
            tt = cls // PART
            p = cls % PART
            col = col_off[tt, c] + rank
            idx_cols[p, col] = sps % rows1
            # pads for this section -> core-local zero row
            secsl = slice(int(sec_col0[c]), int(sec_col0[c + 1]))
            sub = idx_cols[:, secsl]
            sub[sub < 0] = pad_local[c]
            idx_cols[:, secsl] = sub
        # slot stream: col-major (slot i = col*128 + p)
        flat = idx_cols.T.reshape(-1).astype(np.int16)
        idx16s.append(_wrap16(flat))
    meta = dict(k_tc=k_tc, col_off=col_off, chunks=chunks, Scols=Scols, S=S)
    return meta, idx16s


def _prep(inputs):
    f32 = np.float32
    x = np.asarray(inputs["x"], f32)
    ei = np.asarray(inputs["edge_index"], np.int64)
    wts = np.asarray(inputs["weights"], f32)
    batch = np.asarray(inputs["batch"], np.int64)
    cover_n = np.asarray(inputs["cover_n"], np.int64)
    cover_c = np.asarray(inputs["cover_c"], np.int64)
    ei2 = np.asarray(inputs["edge_index2"], np.int64)
    wts2 = np.asarray(inputs["weights2"], f32)
    batch2 = np.asarray(inputs["batch2"], np.int64)
    N = x.shape[0]
    C = batch2.shape[0]
    B = int(batch.max()) + 1 if batch.size else 1
    B = max(B, int(batch2.max()) + 1)

    indeg = np.bincount(ei[1], minlength=N)
    perm1, pos1, rows1, tg1 = _shard_items(batch, indeg, B)
    covercnt = np.bincount(cover_c, minlength=C)
    perm2, pos2, rows2, tg2 = _shard_items(batch2, covercnt, B)
    T1, T2 = rows1 // PART, rows2 // PART

    # per-core pad (zero x1) local rows
    pad_local = []
    for c in range(NCORES):
        pads = np.nonzero(perm1[c] < 0)[0]
        assert pads.size, f"core {c} has no pad node row"
        pad_local.append(int(pads[0]))

    # mp level 1: edges grouped by dst core
    dpos = pos1[ei[1]]
    spos = pos1[ei[0]]
    dl1, sp1, w1 = [], [], []
    for me in range(NCORES):
        m = (dpos // rows1) == me
        dl1.append(dpos[m] % rows1)
        sp1.append(spos[m])
        w1.append(wts[m])
    mp1, idx16_1, dstp_1, ewb_1, ewbn_1 = _mp_stream(dl1, sp1, w1, rows1, T1)

    # mp level 2
    dpos2 = pos2[ei2[1]]
    spos2 = pos2[ei2[0]]
    dl2, sp2, w2 = [], [], []
    for me in range(NCORES):
        m = (dpos2 // rows2) == me
        dl2.append(dpos2[m] % rows2)
        sp2.append(spos2[m])
        w2.append(wts2[m])
    mp2, idx16_2, dstp_2, ewb_2, ewbn_2 = _mp_stream(dl2, sp2, w2, rows2, T2)

    # cover
    cpos = pos2[cover_c]
    npos = pos1[cover_n]
    clc, spc = [], []
    for me in range(NCORES):
        m = (cpos // rows2) == me
        clc.append(cpos[m] % rows2)
        spc.append(npos[m])
    cov, idx16_c = _cover_stream(clc, spc, rows1, rows2, T2, pad_local)

    # per-core dense transposed inputs (f32) and masks
    xTs, m1s, m2s = [], [], []
    for c in range(NCORES):
        pc = perm1[c]
        xc = np.zeros((rows1, x.shape[1]), f32)
        xc[pc >= 0] = x[pc[pc >= 0]]
        xTs.append(np.ascontiguousarray(xc.T))
        m1s.append(np.ascontiguousarray(
            (pc >= 0).astype(f32).reshape(T1, PART).T))
        p2 = perm2[c]
        m2s.append(np.ascontiguousarray(
            (p2 >= 0).astype(f32).reshape(T2, PART).T))

    # degree tables (host-side: exact f32) -> dis = rsqrt(deg+1)*mask
    deg1 = np.zeros(NCORES * rows1, f32)
    np.add.at(deg1, dpos, wts)
    deg2 = np.zeros(NCORES * rows2, f32)
    np.add.at(deg2, dpos2, wts2)
    dis1s, dis2s = [], []
    for c in range(NCORES):
        d1 = 1.0 / np.sqrt(deg1[c * rows1:(c + 1) * rows1] + 1.0)
        d1 = d1.reshape(T1, PART).T * m1s[c]
        dis1s.append(np.ascontiguousarray(d1).astype(f32))
        d2 = 1.0 / np.sqrt(deg2[c * rows2:(c + 1) * rows2] + 1.0)
        d2 = d2.reshape(T2, PART).T * m2s[c]
        dis2s.append(np.ascontiguousarray(d2).astype(f32))

    meta = dict(B=B, T1=T1, T2=T2, rows1=rows1, rows2=rows2,
                mp1=mp1, mp2=mp2, cov=cov, tg1=tg1, tg2=tg2, FIN=x.shape[1])

    rep = lambda v: np.ascontiguousarray(
        np.broadcast_to(np.asarray(v, f32).reshape(1, -1), (PART, v.shape[-1])))
    g = np.asarray(inputs["bn_gamma"], f32)
    bb = np.asarray(inputs["bn_beta"], f32)
    l1w = np.asarray(inputs["lin1_W"], f32)
    H = np.asarray(inputs["W_in0"], f32).shape[1]
    selS = np.r_[0:H, 2 * H:3 * H]
    selM = np.r_[H:2 * H, 3 * H:4 * H]
    shared = {
        "W_in0": np.asarray(inputs["W_in0"], f32),
        "W_in1": np.asarray(inputs["W_in1"], f32),
        "Wl_in": np.asarray(inputs["Wl_in"], f32),
        "W_b0": np.asarray(inputs["W_b0"], f32),
        "W_b1": np.asarray(inputs["W_b1"], f32),
        "Wl_b": np.asarray(inputs["Wl_b"], f32),
        "b_in0": rep(inputs["b_in0"]), "b_in1": rep(inputs["b_in1"]),
        "bl_in": rep(inputs["bl_in"]), "b_b0": rep(inputs["b_b0"]),
        "b_b1": rep(inputs["b_b1"]), "bl_b": rep(inputs["bl_b"]),
        "gammaS": np.ascontiguousarray(g[selS].reshape(PART, 1)),
        "gammaM": np.ascontiguousarray(g[selM].reshape(PART, 1)),
        "betaS": np.ascontiguousarray(bb[selS].reshape(PART, 1)),
        "betaM": np.ascontiguousarray(bb[selM].reshape(PART, 1)),
        "l1WS": np.ascontiguousarray(l1w[selS]),
        "l1WM": np.ascontiguousarray(l1w[selM]),
        "l1b": rep(inputs["lin1_b"]),
        "l2W": np.asarray(inputs["lin2_W"], f32),
        "l2b": rep(inputs["lin2_b"]),
        "iota": np.ascontiguousarray(
            np.broadcast_to(np.arange(PART, dtype=f32)[None, :],
                            (PART, PART))),
        "iotan": np.ascontiguousarray(
            np.broadcast_to(-np.arange(PART, dtype=f32)[None, :],
                            (PART, PART))),
    }
    in_maps = []
    for c in range(NCORES):
        m = dict(shared)
        m["x_cT"] = xTs[c]
        m["mask1"] = m1s[c]
        m["mask2"] = m2s[c]
        m["dis1"] = dis1s[c]
        m["dis2"] = dis2s[c]
        m["idx16_1"] = idx16_1[c]
        m["dstp_1"] = dstp_1[c]
        m["ewb_1"] = ewb_1[c]
        m["ewbn_1"] = ewbn_1[c]
        m["idx16_2"] = idx16_2[c]
        m["dstp_2"] = dstp_2[c]
        m["ewb_2"] = ewb_2[c]
        m["ewbn_2"] = ewbn_2[c]
        m["idx16_c"] = idx16_c[c]
        in_maps.append(m)
    return meta, in_maps


# ------------------------------------------------------------- device kernel

def _build(meta, NCLS=10, H=64):
    import concourse.bass as bass
    import concourse.bacc as bacc
    import concourse.mybir as mybir
    import concourse.tile as tile
    from concourse.masks import make_identity
    from concourse import library_config

    f32 = mybir.dt.float32
    bf16 = mybir.dt.bfloat16
    i16 = mybir.dt.int16
    ALU = mybir.AluOpType
    ACTF = mybir.ActivationFunctionType
    AX = mybir.AxisListType

    B = meta["B"]
    T1, T2 = meta["T1"], meta["T2"]
    rows1, rows2 = meta["rows1"], meta["rows2"]
    FIN = meta["FIN"]
    mp1, mp2, cov = meta["mp1"], meta["mp2"], meta["cov"]
    RG = [list(range(NCORES))]

    nc = bacc.Bacc("TRN2", target_bir_lowering=False, debug=False,
                   num_devices=NCORES, num_swdge_queues=4)

    ein = lambda n, s, d=f32: nc.dram_tensor(n, s, d, kind="ExternalInput")
    x_cT = ein("x_cT", [FIN, rows1])
    mask1 = ein("mask1", [PART, T1]); mask2 = ein("mask2", [PART, T2])
    dis1_d = ein("dis1", [PART, T1]); dis2_d = ein("dis2", [PART, T2])
    idx16_1 = ein("idx16_1", [PART, mp1["S"] // 16], i16)
    dstp_1 = ein("dstp_1", [PART, max(mp1["nruns"], 1)])
    ewb_1 = ein("ewb_1", [PART, max(mp1["nblk"], 1)])
    ewbn_1 = ein("ewbn_1", [PART, max(mp1["nblk"], 1)])
    idx16_2 = ein("idx16_2", [PART, mp2["S"] // 16], i16)
    dstp_2 = ein("dstp_2", [PART, max(mp2["nruns"], 1)])
    ewb_2 = ein("ewb_2", [PART, max(mp2["nblk"], 1)])
    ewbn_2 = ein("ewbn_2", [PART, max(mp2["nblk"], 1)])
    idx16_c = ein("idx16_c", [PART, cov["S"] // 16], i16)
    iota_d = ein("iota", [PART, PART])
    iotan_d = ein("iotan", [PART, PART])
    wshapes = {"W_in0": [FIN, H], "W_in1": [H, H], "Wl_in": [2 * H, H],
               "W_b0": [2 * H, H], "W_b1": [H, H], "Wl_b": [2 * H, H]}
    Ws = {n: ein(n, s) for n, s in wshapes.items()}
    bs = {n: ein(n, [PART, H]) for n in
          ("b_in0", "b_in1", "bl_in", "b_b0", "b_b1", "bl_b")}
    gammaS = ein("gammaS", [PART, 1]); gammaM = ein("gammaM", [PART, 1])
    betaS = ein("betaS", [PART, 1]); betaM = ein("betaM", [PART, 1])
    l1WS = ein("l1WS", [PART, H]); l1WM = ein("l1WM", [PART, H])
    l1b = ein("l1b", [PART, H])
    l2W = ein("l2W", [H, NCLS]); l2b = ein("l2b", [PART, NCLS])
    out_ext = nc.dram_tensor("out", [B, NCLS], f32, kind="ExternalOutput")

    # internal DRAM: wide bf16 tables (upper half junk, never read)
    hs_c1 = nc.dram_tensor("hs_c1", [rows1, WIDE], bf16)
    hs_full1 = nc.dram_tensor("hs_full1", [NCORES * rows1, WIDE], bf16, addr_space="Shared")
    hs_c1b = nc.dram_tensor("hs_c1b", [rows1, WIDE], bf16)
    hs_full1b = nc.dram_tensor("hs_full1b", [NCORES * rows1, WIDE], bf16, addr_space="Shared")
    x1_c = nc.dram_tensor("x1_c", [rows1, WIDE], bf16)
    x1_full = nc.dram_tensor("x1_full", [NCORES * rows1, WIDE], bf16, addr_space="Shared")
    hs_c2 = nc.dram_tensor("hs_c2", [rows2, WIDE], bf16)
    hs_full2 = nc.dram_tensor("hs_full2", [NCORES * rows2, WIDE], bf16, addr_space="Shared")
    hs_c2b = nc.dram_tensor("hs_c2b", [rows2, WIDE], bf16)
    hs_full2b = nc.dram_tensor("hs_full2b", [NCORES * rows2, WIDE], bf16, addr_space="Shared")
    arS_in = nc.dram_tensor("arS_in", [PART, B], f32)
    arS_out = nc.dram_tensor("arS_out", [PART, B], f32, addr_space="Shared")
    arM_in = nc.dram_tensor("arM_in", [PART, B], f32)
    arM_out = nc.dram_tensor("arM_out", [PART, B], f32, addr_space="Shared")

    with tile.TileContext(nc) as tc:
        nc.gpsimd.load_library(library_config.mlp)
        with (tc.tile_pool(name="const", bufs=1) as cpool,
              tc.tile_pool(name="res", bufs=1) as rpool,
              tc.tile_pool(name="gtp", bufs=3) as gtpool,
              tc.tile_pool(name="stg", bufs=3) as stgpool,
              tc.tile_pool(name="work", bufs=2) as wpool,
              tc.tile_pool(name="ps", bufs=3, space="PSUM") as pspool,
              tc.tile_pool(name="psacc", bufs=1, space="PSUM") as papool):

            ident = cpool.tile([PART, PART], f32, tag="ident")
            make_identity(nc, ident[:])

            def load2d(dram, shape, dt=f32, tag=None):
                t = cpool.tile(list(shape), dt, tag=tag or dram.name)
                nc.sync.dma_start(t[:], dram[:, :])
                return t

            identB = cpool.tile([PART, PART], bf16, tag="identB")
            make_identity(nc, identB[:])
            mask1_s = load2d(mask1, (PART, T1))
            mask2_s = load2d(mask2, (PART, T2))
            dis1 = load2d(dis1_d, (PART, T1), tag="dis1s")
            dis2 = load2d(dis2_d, (PART, T2), tag="dis2s")
            iota_s = load2d(iota_d, (PART, PART))
            iotan_s = load2d(iotan_d, (PART, PART), tag="iotan")
            W_s = {n: load2d(Ws[n], Ws[n].shape) for n in Ws}
            b_s = {n: load2d(bs[n], (PART, H)) for n in bs}
            l1WS_s = load2d(l1WS, (PART, H)); l1WM_s = load2d(l1WM, (PART, H))
            l1b_s = load2d(l1b, (PART, H))
            l2W_s = load2d(l2W, (H, NCLS)); l2b_s = load2d(l2b, (PART, NCLS))
            gS_s = load2d(gammaS, (PART, 1)); gM_s = load2d(gammaM, (PART, 1))
            bS_s = load2d(betaS, (PART, 1)); bM_s = load2d(betaM, (PART, 1))

            # f32 activation accumulators: [..., 0, :] = layer a / cover sum,
            # [..., 1, :] = layer b / cover max
            acc1 = rpool.tile([PART, T1, 2, H], f32, tag="acc1")
            acc2 = rpool.tile([PART, T2, 2, H], f32, tag="acc2")
            hs1_sb = rpool.tile([PART, T1, H], bf16, tag="hs1_sb")
            hs2_sb = rpool.tile([PART, T2, H], bf16, tag="hs2_sb")
            rm1 = rpool.tile([PART, B, H], f32, tag="rm1")
            rm2 = rpool.tile([PART, B, H], f32, tag="rm2")
            oneh = rpool.tile([PART, B, B], f32, tag="oneh")
            nc.vector.memset(rm1[:], 0.0)
            nc.vector.memset(rm2[:], 0.0)
            nc.vector.memset(oneh[:], 0.0)
            for g in range(B):
                nc.vector.memset(oneh[:, g, g:g + 1], 1.0)

            def bc_mid(ap2d, G):
                a = ap2d.ap
                return bass.AP(ap2d.tensor, ap2d.offset,
                               [a[0], [0, G], a[-1]])

            ps_sum1 = papool.tile([B, H], f32, tag="sum1")
            ps_sum2 = papool.tile([B, H], f32, tag="sum2")

            def stripes(T, step):
                return [(s, min(s + step, T)) for s in range(0, T, step)]

            # ---- dense matmul phase: hs = dis * (act @ W) -> SBUF + DRAM ----
            def mm_phase(lhsT_fn, Tn, W, dis_t, hs_sb, hs_dram):
                hsd = hs_dram.ap().rearrange("(t p) f -> p t f", p=PART)
                for (s0, s1) in stripes(Tn, STRIPE):
                    for t in range(s0, s1):
                        lhsT = lhsT_fn(t)
                        mm = pspool.tile([PART, H], f32, tag="mm")
                        nc.tensor.matmul(out=mm[:], lhsT=lhsT, rhs=W[:],
                                         start=True, stop=True)
                        nc.vector.tensor_scalar(
                            out=hs_sb[:, t, :], in0=mm[:],
                            scalar1=dis_t[:, t:t + 1], scalar2=None,
                            op0=ALU.mult)
                    nc.sync.dma_start(hsd[:, s0:s1, 0:H], hs_sb[:, s0:s1, :])

            def lhsT_transpose(src_fn, kdim):
                def fn(t):
                    tp = pspool.tile([PART, PART], f32, tag="tp")
                    nc.tensor.transpose(tp[:kdim, :], src_fn(t), ident[:])
                    tsb = wpool.tile([PART, PART], f32, tag="tsb", bufs=5)
                    nc.scalar.copy(out=tsb[:kdim, :], in_=tp[:kdim, :])
                    return tsb[:kdim, :]
                return fn

            xTv = x_cT.ap()
            _xc = {}

            def lhsT_x(t):
                s0 = (t // STRIPE_X) * STRIPE_X
                if s0 not in _xc:
                    xstg = stgpool.tile([FIN, STRIPE_X * PART], f32,
                                        tag="xstg")
                    s1 = min(s0 + STRIPE_X, T1)
                    nc.sync.dma_start(xstg[:, :(s1 - s0) * PART],
                                      xTv[:, s0 * PART:s1 * PART])
                    _xc[s0] = xstg
                return _xc[s0][:, (t - s0) * PART:(t - s0 + 1) * PART]

            def allgather(src, dst):
                nc.gpsimd.collective_compute(
                    "AllGather", ALU.bypass, ins=[src.ap().opt()],
                    outs=[dst.ap().opt()], replica_groups=RG)

            # ---- mp phase: tile-major super-chunks; per tile one PSUM
            # accumulation group (self matmul + one matmul per run), fused
            # epilogue acc = relu((sum + hs_self)*dis + bias) ----
            _qrot = [0]

            def mp_phase(mp, hs_full, idx16_d, dstp_d, ewb_d, ewbn_d, rows,
                         acc, half, hs_sb, dis_t, bias, tile_cb=None):
                runs_all = mp["runs_all"]
                mrun, mblk = mp["mrun"], mp["mblk"]
                mctr = 0
                for sch in mp["schunks"]:
                    r0, r1 = sch["r0"], sch["r1"]
                    b0, b1 = sch["b0"], sch["b1"]
                    dst_t = stgpool.tile([PART, mrun], f32, tag="dstpstg")
                    nc.sync.dma_start(dst_t[:, :r1 - r0], dstp_d[:, r0:r1])
                    ew_t = stgpool.tile([PART, mblk], f32, tag="ewstg")
                    nc.sync.dma_start(ew_t[:, :b1 - b0], ewb_d[:, b0:b1])
                    ewn_t = stgpool.tile([PART, mblk], f32, tag="ewnstg")
                    nc.sync.dma_start(ewn_t[:, :b1 - b0], ewbn_d[:, b0:b1])
                    gts = []
                    for (c, a, npad) in sch["subs"]:
                        idxt = stgpool.tile([PART, NIDX_MAX // 16], i16,
                                            tag="idxstg", name="idxt",
                                            bufs=12)
                        nc.sync.dma_start(
                            idxt[:, :npad // 16],
                            idx16_d[:, a // 16:(a + npad) // 16])
                        gt = gtpool.tile([PART, NIDX_MAX // 128, WIDE],
                                         bf16, tag="gt", name="gt", bufs=8)
                        nc.gpsimd.dma_gather(
                            gt[:, :npad // 128, :],
                            hs_full[c * rows:(c + 1) * rows, :],
                            idxt[:, :npad // 16], npad, npad, WIDE,
                            queue_num=_qrot[0] % 4)
                        _qrot[0] += 1
                        gts.append(gt)
                    for t in range(sch["t0"], sch["t1"]):
                        ops = sch["tile_ops"][t]
                        ps = pspool.tile([PART, H], f32, tag="mm")
                        nc.tensor.matmul(out=ps[:], lhsT=identB[:],
                                         rhs=hs_sb[:, t, :], start=True,
                                         stop=(len(ops) == 0),
                                         skip_group_check=True)
                        for j, (sub_id, bl, rid) in enumerate(ops):
                            be = runs_all[rid][0]
                            M = wpool.tile([PART, PART], bf16, tag="M", bufs=6)
                            if mctr % 2 == 0:
                                nc.vector.tensor_scalar(
                                    out=M[:], in0=iotan_s[:],
                                    scalar1=dst_t[:, rid - r0:rid - r0 + 1],
                                    scalar2=ew_t[:, be - b0:be - b0 + 1],
                                    op0=ALU.is_equal, op1=ALU.mult)
                            else:
                                msq = wpool.tile([PART, PART], f32,
                                                 tag="msq", bufs=2)
                                nc.scalar.activation(
                                    msq[:], iota_s[:], ACTF.Square,
                                    bias=dst_t[:, rid - r0:rid - r0 + 1])
                                nc.scalar.activation(
                                    M[:], msq[:], ACTF.Relu,
                                    bias=ew_t[:, be - b0:be - b0 + 1],
                                    scale=ewn_t[:, be - b0:be - b0 + 1])
                            mctr += 1
                            nc.tensor.matmul(out=ps[:], lhsT=M[:],
                                             rhs=gts[sub_id][:, bl, 0:H],
                                             start=False,
                                             stop=(j == len(ops) - 1),
                                             skip_group_check=True)
                        ept = wpool.tile([PART, H], f32, tag="ept", bufs=5)
                        nc.vector.tensor_scalar(
                            out=ept[:], in0=ps[:],
                            scalar1=dis_t[:, t:t + 1], scalar2=None,
                            op0=ALU.mult)
                        nc.vector.tensor_tensor(out=ept[:], in0=ept[:],
                                                in1=bias[:], op=ALU.add)
                        nc.scalar.activation(acc[:, t, half, :], ept[:],
                                             ACTF.Relu)
                    if tile_cb is not None:
                        tile_cb(sch["t0"], sch["t1"])

            # per-tile mm work folded into a preceding mp phase (emission
            # interleaving hides the dense chains under gather drains)
            def mm_tile_cb(lhsT_fn, W, dis_t, hs_sb, hs_dram):
                hsd = hs_dram.ap().rearrange("(t p) f -> p t f", p=PART)

                def cb(t0, t1):
                    for t in range(t0, t1):
                        lhsT = lhsT_fn(t)
                        mm = pspool.tile([PART, H], f32, tag="mm")
                        nc.tensor.matmul(out=mm[:], lhsT=lhsT, rhs=W[:],
                                         start=True, stop=True,
                                         skip_group_check=True)
                        nc.vector.tensor_scalar(
                            out=hs_sb[:, t, :], in0=mm[:],
                            scalar1=dis_t[:, t:t + 1], scalar2=None,
                            op0=ALU.mult)
                    nc.sync.dma_start(hsd[:, t0:t1, 0:H], hs_sb[:, t0:t1, :])
                return cb

            def jk_tile_cb(acc, Tn, Wl, bias, mask_s, tg, ps_sum, rm,
                           x_dram):
                lfn = lhsT_transpose(
                    lambda t: acc[:, t, :, :].rearrange("p a b -> p (a b)"),
                    PART)
                xd = (x_dram.ap().rearrange("(t p) f -> p t f", p=PART)
                      if x_dram is not None else None)

                def cb(t0, t1):
                    stg = (stgpool.tile([PART, EPI_CH, H], bf16, tag="x1stg",
                                        name="stg")
                           if xd is not None else None)
                    for t in range(t0, t1):
                        lhsT = lfn(t)
                        mm = pspool.tile([PART, H], f32, tag="mm")
                        nc.tensor.matmul(out=mm[:], lhsT=lhsT, rhs=Wl[:],
                                         start=True, stop=True,
                                         skip_group_check=True)
                        xt = wpool.tile([PART, H], f32, tag="xt", bufs=5)
                        nc.vector.tensor_tensor(out=xt[:], in0=mm[:],
                                                in1=bias[:], op=ALU.add)
                        nc.scalar.activation(xt[:], xt[:], ACTF.Relu,
                                             scale=mask_s[:, t:t + 1])
                        g = int(tg[t])
                        nc.tensor.matmul(out=ps_sum[:], lhsT=oneh[:, g, :],
                                         rhs=xt[:], start=(t == 0),
                                         stop=(t == Tn - 1),
                                         skip_group_check=True)
                        nc.vector.tensor_tensor(out=rm[:, g, :],
                                                in0=rm[:, g, :],
                                                in1=xt[:], op=ALU.max)
                        if stg is not None:
                            nc.scalar.copy(out=stg[:, t - t0, :], in_=xt[:])
                    if stg is not None:
                        nc.sync.dma_start(xd[:, t0:t1, 0:H],
                                          stg[:, :t1 - t0, :])
                return cb

            # ---- jk: cat(a,b) @ Wl + bias, relu*mask, readouts ----
            def jk_phase(acc, Tn, Wl, bias, mask_s, tg, ps_sum, rm, x_dram):
                lfn = lhsT_transpose(
                    lambda t: acc[:, t, :, :].rearrange("p a b -> p (a b)"),
                    PART)
                xd = (x_dram.ap().rearrange("(t p) f -> p t f", p=PART)
                      if x_dram is not None else None)
                for (s0, s1) in stripes(Tn, EPI_CH):
                    stg = (stgpool.tile([PART, EPI_CH, H], bf16, tag="x1stg",
                                        name="stg")
                           if x_dram is not None else None)
                    for t in range(s0, s1):
                        lhsT = lfn(t)
                        mm = pspool.tile([PART, H], f32, tag="mm")
                        nc.tensor.matmul(out=mm[:], lhsT=lhsT, rhs=Wl[:],
                                         start=True, stop=True)
                        xt = wpool.tile([PART, H], f32, tag="xt", bufs=5)
                        nc.vector.tensor_tensor(out=xt[:], in0=mm[:],
                                                in1=bias[:], op=ALU.add)
                        nc.scalar.activation(xt[:], xt[:], ACTF.Relu,
                                             scale=mask_s[:, t:t + 1])
                        g = int(tg[t])
                        nc.tensor.matmul(out=ps_sum[:], lhsT=oneh[:, g, :],
                                         rhs=xt[:], start=(t == 0),
                                         stop=(t == Tn - 1),
                                         skip_group_check=True)
                        nc.vector.tensor_tensor(out=rm[:, g, :],
                                                in0=rm[:, g, :],
                                                in1=xt[:], op=ALU.max)
                        if stg is not None:
                            nc.scalar.copy(out=stg[:, t - s0, :], in_=xt[:])
                    if stg is not None:
                        nc.sync.dma_start(xd[:, s0:s1, 0:H],
                                          stg[:, :s1 - s0, :])

            # ================= pipeline =================
            with nc.named_scope("mm1a"):
                mm_phase(lhsT_x, T1, W_s["W_in0"], dis1, hs1_sb, hs_c1)
            with nc.named_scope("ag1a"):
                allgather(hs_c1, hs_full1)
            with nc.named_scope("mp1a"):
                mp_phase(mp1, hs_full1, idx16_1, dstp_1, ewb_1, ewbn_1,
                         rows1, acc1, 0, hs1_sb, dis1, b_s["b_in0"],
                         tile_cb=mm_tile_cb(
                             lhsT_transpose(lambda t: acc1[:, t, 0, :], H),
                             W_s["W_in1"], dis1, hs1_sb, hs_c1b))
            with nc.named_scope("ag1b"):
                allgather(hs_c1b, hs_full1b)
            with nc.named_scope("mp1b"):
                mp_phase(mp1, hs_full1b, idx16_1, dstp_1, ewb_1, ewbn_1,
                         rows1, acc1, 1, hs1_sb, dis1, b_s["b_in1"],
                         tile_cb=jk_tile_cb(acc1, T1, W_s["Wl_in"],
                                            b_s["bl_in"], mask1_s,
                                            meta["tg1"], ps_sum1, rm1,
                                            x1_c))
            with nc.named_scope("agx1"):
                allgather(x1_c, x1_full)

            # ---------- cover pooling: sum -> acc1[...,0], max -> [...,1]
            # (acc1 is free after jk1; reuse its first T2 tiles)
            with nc.named_scope("cover"):
                k_tc = cov["k_tc"]; col_off = cov["col_off"]
                written = set()
                for (c, c0, c1) in cov["chunks"]:
                    ncols = c1 - c0
                    n = ncols * 128
                    idxt = stgpool.tile([PART, NIDX_MAX // 16], i16,
                                        tag="idxstg", bufs=12)
                    nc.sync.dma_start(idxt[:, :n // 16],
                                      idx16_c[:, c0 * 8:c1 * 8])
                    gt = gtpool.tile([PART, NIDX_MAX // 128, WIDE], bf16,
                                     tag="gt", bufs=8)
                    nc.gpsimd.dma_gather(
                        gt[:, :ncols, :],
                        x1_full[c * rows1:(c + 1) * rows1, :],
                        idxt[:, :n // 16], n, n, WIDE,
                        queue_num=_qrot[0] % 4)
                    _qrot[0] += 1
                    for t in range(T2):
                        k = int(k_tc[t, c])
                        if k == 0:
                            continue
                        ca = int(col_off[t, c]) - c0
                        if ca < 0 or ca + k > ncols:
                            continue
                        view = gt[:, ca:ca + k, 0:H].rearrange(
                            "p k f -> p f k")
                        zs = wpool.tile([PART, H], f32, tag="zs", bufs=5)
                        nc.vector.tensor_reduce(out=zs[:], in_=view,
                                                axis=AX.X, op=ALU.add)
                        zm = wpool.tile([PART, H], f32, tag="zm", bufs=5)
                        nc.vector.tensor_reduce(out=zm[:], in_=view,
                                                axis=AX.X, op=ALU.max)
                        if t in written:
                            nc.vector.tensor_tensor(
                                out=acc1[:, t, 0, :], in0=acc1[:, t, 0, :],
                                in1=zs[:], op=ALU.add)
                            nc.vector.tensor_tensor(
                                out=acc1[:, t, 1, :], in0=acc1[:, t, 1, :],
                                in1=zm[:], op=ALU.max)
                        else:
                            nc.vector.tensor_copy(acc1[:, t, 0, :], zs[:])
                            nc.vector.tensor_copy(acc1[:, t, 1, :], zm[:])
                            written.add(t)
                for t in range(T2):
                    if t not in written:
                        nc.vector.memset(acc1[:, t, 0, :], 0.0)
                        nc.vector.memset(acc1[:, t, 1, :], 0.0)

            # ---------- pooled block ----------
            with nc.named_scope("mm2a"):
                mm_phase(lhsT_transpose(
                    lambda t: acc1[:, t, :, :].rearrange("p a b -> p (a b)"),
                    PART), T2, W_s["W_b0"], dis2, hs2_sb, hs_c2)
            with nc.named_scope("ag2a"):
                allgather(hs_c2, hs_full2)
            with nc.named_scope("mp2a"):
                mp_phase(mp2, hs_full2, idx16_2, dstp_2, ewb_2, ewbn_2,
                         rows2, acc2, 0, hs2_sb, dis2, b_s["b_b0"],
                         tile_cb=mm_tile_cb(
                             lhsT_transpose(lambda t: acc2[:, t, 0, :], H),
                             W_s["W_b1"], dis2, hs2_sb, hs_c2b))
            with nc.named_scope("ag2b"):
                allgather(hs_c2b, hs_full2b)
            with nc.named_scope("mp2b"):
                mp_phase(mp2, hs_full2b, idx16_2, dstp_2, ewb_2, ewbn_2,
                         rows2, acc2, 1, hs2_sb, dis2, b_s["b_b1"],
                         tile_cb=jk_tile_cb(acc2, T2, W_s["Wl_b"],
                                            b_s["bl_b"], mask2_s,
                                            meta["tg2"], ps_sum2, rm2,
                                            None))

            # ---------- readout combine + head ----------
            sc_head = nc.named_scope("head"); sc_head.__enter__()
            sum1_sb = wpool.tile([B, H], f32, tag="s1sb")
            nc.scalar.copy(out=sum1_sb[:], in_=ps_sum1[:])
            sum2_sb = wpool.tile([B, H], f32, tag="s2sb")
            nc.scalar.copy(out=sum2_sb[:], in_=ps_sum2[:])
            sT = pspool.tile([H, B], f32, tag="tp")
            nc.tensor.matmul(out=sT[:], lhsT=sum1_sb[:], rhs=ident[:B, :B],
                             start=True, stop=True)
            sT1 = wpool.tile([H, B], f32, tag="sT1")
            nc.scalar.copy(out=sT1[:], in_=sT[:])
            sT_2 = pspool.tile([H, B], f32, tag="tp")
            nc.tensor.matmul(out=sT_2[:], lhsT=sum2_sb[:], rhs=ident[:B, :B],
                             start=True, stop=True)
            sT2 = wpool.tile([H, B], f32, tag="sT2")
            nc.scalar.copy(out=sT2[:], in_=sT_2[:])
            nc.sync.dma_start(arS_in[0:H, :], sT1[:])
            nc.sync.dma_start(arS_in[H:2 * H, :], sT2[:])

            mT1 = wpool.tile([H, B], f32, tag="mT1")
            mT2 = wpool.tile([H, B], f32, tag="mT2")
            for g in range(B):
                for rm, mt in ((rm1, mT1), (rm2, mT2)):
                    tpm = pspool.tile([H, PART], f32, tag="tp")
                    nc.tensor.transpose(tpm[:], rm[:, g, :], ident[:])
                    msb = wpool.tile([H, PART], f32, tag="msb")
                    nc.scalar.copy(out=msb[:], in_=tpm[:])
                    nc.vector.tensor_reduce(out=mt[:, g:g + 1], in_=msb[:],
                                            axis=AX.X, op=ALU.max)
            nc.sync.dma_start(arM_in[0:H, :], mT1[:])
            nc.sync.dma_start(arM_in[H:2 * H, :], mT2[:])

            nc.gpsimd.collective_compute(
                "AllReduce", ALU.add, ins=[arS_in.ap().opt()],
                outs=[arS_out.ap().opt()], replica_groups=RG)
            nc.gpsimd.collective_compute(
                "AllReduce", ALU.max, ins=[arM_in.ap().opt()],
                outs=[arM_out.ap().opt()], replica_groups=RG)

            S_sb = wpool.tile([PART, B], f32, tag="Ssb")
            M_sb = wpool.tile([PART, B], f32, tag="Msb")
            nc.sync.dma_start(S_sb[:], arS_out[:, :])
            nc.sync.dma_start(M_sb[:], arM_out[:, :])

            def bn(t_sb, gam, bet):
                mu = wpool.tile([PART, 1], f32, tag="mu")
                nc.vector.tensor_reduce(out=mu[:], in_=t_sb[:], axis=AX.X,
                                        op=ALU.add)
                nc.vector.tensor_scalar_mul(mu[:], mu[:], 1.0 / B)
                nc.vector.tensor_scalar(out=t_sb[:], in0=t_sb[:],
                                        scalar1=mu[:], scalar2=None,
                                        op0=ALU.subtract)
                sq = wpool.tile([PART, B], f32, tag="sq")
                nc.vector.tensor_tensor(out=sq[:], in0=t_sb[:], in1=t_sb[:],
                                        op=ALU.mult)
                var = wpool.tile([PART, 1], f32, tag="var")
                nc.vector.tensor_reduce(out=var[:], in_=sq[:], axis=AX.X,
                                        op=ALU.add)
                nc.vector.tensor_scalar(out=var[:], in0=var[:],
                                        scalar1=1.0 / B, scalar2=EPS,
                                        op0=ALU.mult, op1=ALU.add)
                nc.scalar.activation(var[:], var[:], ACTF.Sqrt)
                nc.vector.reciprocal(var[:], var[:])
                nc.vector.tensor_scalar(out=t_sb[:], in0=t_sb[:],
                                        scalar1=var[:], scalar2=gam[:],
                                        op0=ALU.mult, op1=ALU.mult)
                nc.vector.tensor_scalar(out=t_sb[:], in0=t_sb[:],
                                        scalar1=bet[:], scalar2=None,
                                        op0=ALU.add)

            bn(S_sb, gS_s, bS_s)
            bn(M_sb, gM_s, bM_s)

            pl1 = pspool.tile([B, H], f32, tag="mm")
            nc.tensor.matmul(out=pl1[:], lhsT=S_sb[:], rhs=l1WS_s[:],
                             start=True, stop=False)
            nc.tensor.matmul(out=pl1[:], lhsT=M_sb[:], rhs=l1WM_s[:],
                             start=False, stop=True)
            y = wpool.tile([B, H], f32, tag="y")
            nc.vector.tensor_tensor(out=y[:], in0=pl1[:], in1=l1b_s[:B, :],
                                    op=ALU.add)
            nc.scalar.activation(y[:], y[:], ACTF.Relu)
            yT_ps = pspool.tile([H, B], f32, tag="tp")
            nc.tensor.matmul(out=yT_ps[:], lhsT=y[:], rhs=ident[:B, :B],
                             start=True, stop=True)
            yT = wpool.tile([H, B], f32, tag="yTs")
            nc.scalar.copy(out=yT[:], in_=yT_ps[:])
            pl2 = pspool.tile([B, NCLS], f32, tag="mm")
            nc.tensor.matmul(out=pl2[:], lhsT=yT[:], rhs=l2W_s[:],
                             start=True, stop=True)
            z = wpool.tile([B, NCLS], f32, tag="z")
            nc.vector.tensor_tensor(out=z[:], in0=pl2[:], in1=l2b_s[:B, :],
                                    op=ALU.add)
            zmax = wpool.tile([B, 1], f32, tag="zmax")
            nc.vector.tensor_reduce(out=zmax[:], in_=z[:], axis=AX.X,
                                    op=ALU.max)
            nc.vector.tensor_scalar(out=z[:], in0=z[:], scalar1=zmax[:],
                                    scalar2=None, op0=ALU.subtract)
            nc.scalar.activation(z[:], z[:], ACTF.Exp)
            zsum = wpool.tile([B, 1], f32, tag="zsum")
            nc.vector.tensor_reduce(out=zsum[:], in_=z[:], axis=AX.X,
                                    op=ALU.add)
            nc.vector.reciprocal(zsum[:], zsum[:])
            nc.vector.tensor_scalar(out=z[:], in0=z[:], scalar1=zsum[:],
                                    scalar2=None, op0=ALU.mult)
            nc.sync.dma_start(out_ext[:, :], z[:])
            sc_head.__exit__(None, None, None)

    nc.compile()
    return nc


def kernel(**inputs):
    from concourse import bass_utils
    meta, in_maps = _prep(inputs)
    nc = _build(meta)
    res = bass_utils.run_bass_kernel_spmd(
        nc, in_maps, core_ids=list(range(NCORES)))
    return np.asarray(res.results[0]["out"])


# revision 48
# speedup vs baseline: 1.5990x; 1.5215x over previous
"""Bass/Trainium2 kernel for nn_KPlexPool (GCN blocks + cover pooling), 8 NeuronCores.

v3: message-passing gathers use dma_gather (one SWDGE instruction per <=4096
slots instead of one per 128 rows), with slot streams sorted by (src_core,
dst) so each instruction's int16 indices fit one core's 32K-row window of the
AllGathered bf16 table.  Segment sums are done by TensorE: per 128-slot block
a one-hot matrix M[slot, dst_partition] = (iota==dstp)*ew is built in one DVE
tensor_scalar op and matmul'd against the gathered rows, accumulating into a
f32 SBUF accumulator per tile.  Cover pooling keeps dst-aligned CSR (needed
for max) bucketed by source core, gathered with dma_gather + DVE reduces.
Activations stay f32 in SBUF (dense matmuls fp32); only the gathered hs/x1
tables, their AllGathers, and the one-hot ew are bf16.  The program is
SPMD-uniform: all slot counts are padded to per-(tile, src_core) maxima over
cores; per-core data (indices, dstp, ew) carries the differences.
"""

import sys
import numpy as np
import ml_dtypes

sys.path.insert(0, "/opt/trn_rl_repo")

PART = 128
NCORES = 8
EPS = 1e-5
NIDX_MAX = 1024      # slots per dma_gather (ring-limited: 2048+ wedges SWDGE)
EPI_CH = 16          # tiles per epilogue chunk
STRIPE = 32          # tiles per hs DMA stripe
STRIPE_X = 8         # tiles per x lhsT load stripe

BF16 = ml_dtypes.bfloat16
WIDE = 128           # gathered-table row width (bf16 -> 256B rows)


# ----------------------------------------------------------------- host prep

def _shard_items(batch, sortkey, B):
    n = batch.shape[0]
    counts = np.bincount(batch, minlength=B)
    starts = np.concatenate([[0], np.cumsum(counts)[:-1]])
    perm_cores = [[] for _ in range(NCORES)]
    tile_graph = []
    for g in range(B):
        cnt = int(counts[g])
        st = int(starts[g])
        base, rem = divmod(cnt, NCORES)
        sizes = [base + (1 if c < rem else 0) for c in range(NCORES)]
        Tg = max(1, -(-max(sizes) // PART))
        tile_graph += [g] * Tg
        off = st
        for c in range(NCORES):
            s = sizes[c]
            ids = np.arange(off, off + s)
            off += s
            order = np.argsort(-sortkey[ids], kind="stable")
            padded = np.full(Tg * PART, -1, dtype=np.int64)
            padded[:s] = ids[order]
            perm_cores[c].append(padded)
    perm = [np.concatenate(p) for p in perm_cores]
    rows = perm[0].shape[0]
    pos = np.full(n, -1, dtype=np.int64)
    for c in range(NCORES):
        real = perm[c] >= 0
        pos[perm[c][real]] = c * rows + np.nonzero(real)[0]
    return perm, pos, rows, np.asarray(tile_graph)


def _wrap16(flat):
    """idx stream [n] -> dma_gather idx layout [128, n/16] int16."""
    n = flat.shape[0]
    assert n % 16 == 0
    w = np.zeros((16, n // 16), dtype=np.int16)
    w[np.arange(n) % 16, np.arange(n) // 16] = flat
    return np.tile(w, (8, 1))


def _mp_stream(dst_loc, src_pos, w, rows, T):
    """(src_core, dst)-sorted slot stream, chunked into tile-major
    super-chunks of 8 per-core sub-gathers so each tile's segment-sum is one
    PSUM accumulation group.

    Returns program-uniform meta + per-core data (idx16, dstpn, ewb, ewbn).
    """
    cnt = np.zeros((NCORES, T, NCORES), np.int64)
    percore = []
    for me in range(NCORES):
        dl, sp, ww = dst_loc[me], src_pos[me], w[me]
        sc = sp // rows
        np.add.at(cnt[me], (dl // PART, sc), 1)
        percore.append((dl, sp, ww, sc))
    n_tc = cnt.max(axis=0)  # [T, NCORES]
    n_tc = ((n_tc + 15) // 16) * 16          # x16 so idx slices stay aligned

    sec_len = n_tc.sum(axis=0) + 128         # +128 slack for x128 rounding
    sec_len_p = ((sec_len + 127) // 128) * 128
    sec_off = np.concatenate([[0], np.cumsum(sec_len_p)])
    S = int(sec_off[-1])

    tc_off = np.zeros((T, NCORES), np.int64)
    for c in range(NCORES):
        off = int(sec_off[c])
        for t in range(T):
            tc_off[t, c] = off
            off += int(n_tc[t, c])

    # super-chunks: tile ranges where every core's sub-range <= NIDX_MAX-127
    lim = NIDX_MAX - 127
    tranges = []
    t0 = 0
    while t0 < T:
        t1 = t0 + 1
        while (t1 < T and t1 - t0 < EPI_CH
               and int(n_tc[t0:t1 + 1].sum(axis=0).max()) <= lim):
            t1 += 1
        tranges.append((t0, t1))
        t0 = t1

    # emission: per super-chunk, per core: sub-gather (a, npad); blocks are
    # instruction-local; runs = (sub, blk_local, tile, lo, hi) -> emission ids
    schunks = []
    nblk_em = 0
    runs_all = []  # run id -> (blk_em, tile, lo, hi, abs_lo)
    for (t0, t1) in tranges:
        subs = []
        tile_ops = {t: [] for t in range(t0, t1)}
        r0, b0 = len(runs_all), nblk_em
        for c in range(NCORES):
            a = int(tc_off[t0, c])
            b = (int(tc_off[t1, c]) if t1 < T
                 else int(tc_off[t1 - 1, c] + n_tc[t1 - 1, c]))
            npad = ((b - a + 127) // 128) * 128
            assert a + npad <= int(sec_off[c + 1]), "section slack exceeded"
            sub_id = len(subs)
            subs.append((c, a, npad))
            for t in range(t0, t1):
                lo = int(tc_off[t, c]) - a
                hi = lo + int(n_tc[t, c])
                s = lo
                while s < hi:
                    bl = s // 128
                    e = min(hi, (bl + 1) * 128)
                    runs_all.append((nblk_em + bl, t, s - bl * 128,
                                     e - bl * 128, a + s))
                    tile_ops[t].append((sub_id, bl, len(runs_all) - 1))
                    s = e
            nblk_em += npad // 128
        schunks.append(dict(t0=t0, t1=t1, subs=subs, tile_ops=tile_ops,
                            r0=r0, r1=len(runs_all), b0=b0, b1=nblk_em))
    nruns_em = len(runs_all)
    mrun = max(1, max(s["r1"] - s["r0"] for s in schunks))
    mblk = max(1, max(s["b1"] - s["b0"] for s in schunks))

    # per-core data tables in emission layout
    idx16s, dstpns, ewbs, ewbns = [], [], [], []
    for me in range(NCORES):
        dl, sp, ww, sc = percore[me]
        order = np.lexsort((dl, sc))
        dls, sps, wws, scs = (a[order] for a in (dl, sp, ww, sc))
        idx = np.zeros(S, np.int16)
        dstp_sl = np.full(S, 255.0, np.float32)
        ew_sl = np.zeros(S, np.float32)
        ptr = 0
        for c in range(NCORES):
            for t in range(T):
                k = int(cnt[me, t, c])
                if k:
                    o = int(tc_off[t, c])
                    sl = slice(ptr, ptr + k)
                    idx[o:o + k] = (sps[sl] % rows).astype(np.int16)
                    dstp_sl[o:o + k] = (dls[sl] % PART).astype(np.float32)
                    ew_sl[o:o + k] = wws[sl]
                    ptr += k
        assert ptr == dls.shape[0]
        dstpn = np.full((PART, max(nruns_em, 1)), -255.0, np.float32)
        for r, (be, t, lo, hi, abs_lo) in enumerate(runs_all):
            dstpn[lo:hi, r] = -dstp_sl[abs_lo:abs_lo + (hi - lo)]
        # per-emission-block ew columns
        ewb = np.zeros((PART, max(nblk_em, 1)), np.float32)
        for sch in schunks:
            be = sch["b0"]
            for (c, a, npad) in sch["subs"]:
                for bl in range(npad // 128):
                    ewb[:, be] = ew_sl[a + bl * 128:a + (bl + 1) * 128]
                    be += 1
        idx16s.append(_wrap16(idx))
        dstpns.append(dstpn)
        ewbs.append(ewb)
        ewbns.append(-ewb)
    meta = dict(S=S, nblk=nblk_em, nruns=nruns_em, schunks=schunks,
                runs_all=runs_all, mrun=mrun, mblk=mblk)
    return meta, idx16s, dstpns, ewbs, ewbns


def _cover_stream(cl_loc, src_pos, rows1, rows2, T2, pad_local):
    """Dst-aligned per-src-core-bucketed CSR for cover sum+max.

    Returns meta (k_tc [T2, NCORES], chunks, col layout) + per-core idx16."""
    cnt_pc = np.zeros((NCORES, T2 * PART, NCORES), np.int32)
    percore = []
    for me in range(NCORES):
        cl, sp = cl_loc[me], src_pos[me]
        sc = sp // rows1
        np.add.at(cnt_pc[me], (cl, sc), 1)
        percore.append((cl, sp, sc))
    # k per (tile, src core): max over partitions and cores
    k_tc = cnt_pc.reshape(NCORES, T2, PART, NCORES).max(axis=(0, 2))  # [T2, C]
    sec_cols = k_tc.sum(axis=0)  # columns per section
    col_off = np.zeros((T2, NCORES), np.int64)
    acc = 0
    sec_col0 = np.zeros(NCORES + 1, np.int64)
    for c in range(NCORES):
        sec_col0[c] = acc
        for t in range(T2):
            col_off[t, c] = acc
            acc += int(k_tc[t, c])
    sec_col0[NCORES] = acc
    Scols = acc
    S = Scols * 128

    # tile-major super-chunks: per tile range, one sub-gather per src core
    # (every per-core column span must fit one NIDX_MAX gather)
    maxcols = NIDX_MAX // 128
    assert int(k_tc.max()) <= maxcols, "single (t,c) exceeds one gather"
    tranges = []
    t0 = 0
    while t0 < T2:
        t1 = t0 + 1
        while (t1 < T2 and t1 - t0 < EPI_CH
               and int(k_tc[t0:t1 + 1].sum(axis=0).max()) <= maxcols):
            t1 += 1
        tranges.append((t0, t1))
        t0 = t1

    idx16s = []
    for me in range(NCORES):
        cl, sp, sc = percore[me]
        idx_cols = np.full((PART, Scols), -1, np.int64)
        for c in range(NCORES):
            m = sc == c
            cls, sps = cl[m], sp[m]
            order = np.argsort(cls, kind="stable")
            cls, sps = cls[order], sps[order]
            ccount = np.bincount(cls, minlength=T2 * PART)
            first = np.concatenate([[0], np.cumsum(ccount)[:-1]])
            rank = np.arange(cls.shape[0]) - first[cls]
            tt = cls // PART
            p = cls % PART
            col = col_off[tt, c] + rank
            idx_cols[p, col] = sps % rows1
            # pads for this section -> core-local zero row
            secsl = slice(int(sec_col0[c]), int(sec_col0[c + 1]))
            sub = idx_cols[:, secsl]
            sub[sub < 0] = pad_local[c]
            idx_cols[:, secsl] = sub
        # slot stream: col-major (slot i = col*128 + p)
        flat = idx_cols.T.reshape(-1).astype(np.int16)
        idx16s.append(_wrap16(flat))
    meta = dict(k_tc=k_tc, col_off=col_off, tranges=tranges, Scols=Scols,
                S=S)
    return meta, idx16s


def _prep(inputs):
    f32 = np.float32
    x = np.asarray(inputs["x"], f32)
    ei = np.asarray(inputs["edge_index"], np.int64)
    wts = np.asarray(inputs["weights"], f32)
    batch = np.asarray(inputs["batch"], np.int64)
    cover_n = np.asarray(inputs["cover_n"], np.int64)
    cover_c = np.asarray(inputs["cover_c"], np.int64)
    ei2 = np.asarray(inputs["edge_index2"], np.int64)
    wts2 = np.asarray(inputs["weights2"], f32)
    batch2 = np.asarray(inputs["batch2"], np.int64)
    N = x.shape[0]
    C = batch2.shape[0]
    B = int(batch.max()) + 1 if batch.size else 1
    B = max(B, int(batch2.max()) + 1)

    indeg = np.bincount(ei[1], minlength=N)
    perm1, pos1, rows1, tg1 = _shard_items(batch, indeg, B)
    covercnt = np.bincount(cover_c, minlength=C)
    perm2, pos2, rows2, tg2 = _shard_items(batch2, covercnt, B)
    T1, T2 = rows1 // PART, rows2 // PART

    # per-core pad (zero x1) local rows
    pad_local = []
    for c in range(NCORES):
        pads = np.nonzero(perm1[c] < 0)[0]
        assert pads.size, f"core {c} has no pad node row"
        pad_local.append(int(pads[0]))

    # mp level 1: edges grouped by dst core
    dpos = pos1[ei[1]]
    spos = pos1[ei[0]]
    dl1, sp1, w1 = [], [], []
    for me in range(NCORES):
        m = (dpos // rows1) == me
        dl1.append(dpos[m] % rows1)
        sp1.append(spos[m])
        w1.append(wts[m])
    mp1, idx16_1, dstp_1, ewb_1, ewbn_1 = _mp_stream(dl1, sp1, w1, rows1, T1)

    # mp level 2
    dpos2 = pos2[ei2[1]]
    spos2 = pos2[ei2[0]]
    dl2, sp2, w2 = [], [], []
    for me in range(NCORES):
        m = (dpos2 // rows2) == me
        dl2.append(dpos2[m] % rows2)
        sp2.append(spos2[m])
        w2.append(wts2[m])
    mp2, idx16_2, dstp_2, ewb_2, ewbn_2 = _mp_stream(dl2, sp2, w2, rows2, T2)

    # cover
    cpos = pos2[cover_c]
    npos = pos1[cover_n]
    clc, spc = [], []
    for me in range(NCORES):
        m = (cpos // rows2) == me
        clc.append(cpos[m] % rows2)
        spc.append(npos[m])
    cov, idx16_c = _cover_stream(clc, spc, rows1, rows2, T2, pad_local)

    # per-core dense transposed inputs (f32) and masks
    xTs, m1s, m2s = [], [], []
    for c in range(NCORES):
        pc = perm1[c]
        xc = np.zeros((rows1, x.shape[1]), f32)
        xc[pc >= 0] = x[pc[pc >= 0]]
        xTs.append(np.ascontiguousarray(xc.T))
        m1s.append(np.ascontiguousarray(
            (pc >= 0).astype(f32).reshape(T1, PART).T))
        p2 = perm2[c]
        m2s.append(np.ascontiguousarray(
            (p2 >= 0).astype(f32).reshape(T2, PART).T))

    # degree tables (host-side: exact f32) -> dis = rsqrt(deg+1)*mask
    deg1 = np.zeros(NCORES * rows1, f32)
    np.add.at(deg1, dpos, wts)
    deg2 = np.zeros(NCORES * rows2, f32)
    np.add.at(deg2, dpos2, wts2)
    dis1s, dis2s = [], []
    for c in range(NCORES):
        d1 = 1.0 / np.sqrt(deg1[c * rows1:(c + 1) * rows1] + 1.0)
        d1 = d1.reshape(T1, PART).T * m1s[c]
        dis1s.append(np.ascontiguousarray(d1).astype(f32))
        d2 = 1.0 / np.sqrt(deg2[c * rows2:(c + 1) * rows2] + 1.0)
        d2 = d2.reshape(T2, PART).T * m2s[c]
        dis2s.append(np.ascontiguousarray(d2).astype(f32))

    meta = dict(B=B, T1=T1, T2=T2, rows1=rows1, rows2=rows2,
                mp1=mp1, mp2=mp2, cov=cov, tg1=tg1, tg2=tg2, FIN=x.shape[1])

    rep = lambda v: np.ascontiguousarray(
        np.broadcast_to(np.asarray(v, f32).reshape(1, -1), (PART, v.shape[-1])))
    g = np.asarray(inputs["bn_gamma"], f32)
    bb = np.asarray(inputs["bn_beta"], f32)
    l1w = np.asarray(inputs["lin1_W"], f32)
    H = np.asarray(inputs["W_in0"], f32).shape[1]
    selS = np.r_[0:H, 2 * H:3 * H]
    selM = np.r_[H:2 * H, 3 * H:4 * H]
    shared = {
        "W_in0": np.asarray(inputs["W_in0"], f32),
        "W_in1": np.asarray(inputs["W_in1"], f32),
        "Wl_in": np.asarray(inputs["Wl_in"], f32),
        "W_b0": np.asarray(inputs["W_b0"], f32),
        "W_b1": np.asarray(inputs["W_b1"], f32),
        "Wl_b": np.asarray(inputs["Wl_b"], f32),
        "b_in0": rep(inputs["b_in0"]), "b_in1": rep(inputs["b_in1"]),
        "bl_in": rep(inputs["bl_in"]), "b_b0": rep(inputs["b_b0"]),
        "b_b1": rep(inputs["b_b1"]), "bl_b": rep(inputs["bl_b"]),
        "gammaS": np.ascontiguousarray(g[selS].reshape(PART, 1)),
        "gammaM": np.ascontiguousarray(g[selM].reshape(PART, 1)),
        "betaS": np.ascontiguousarray(bb[selS].reshape(PART, 1)),
        "betaM": np.ascontiguousarray(bb[selM].reshape(PART, 1)),
        "l1WS": np.ascontiguousarray(l1w[selS]),
        "l1WM": np.ascontiguousarray(l1w[selM]),
        "l1b": rep(inputs["lin1_b"]),
        "l2W": np.asarray(inputs["lin2_W"], f32),
        "l2b": rep(inputs["lin2_b"]),
        "iota": np.ascontiguousarray(
            np.broadcast_to(np.arange(PART, dtype=f32)[None, :],
                            (PART, PART))),
        "iotan": np.ascontiguousarray(
            np.broadcast_to(-np.arange(PART, dtype=f32)[None, :],
                            (PART, PART))),
    }
    in_maps = []
    for c in range(NCORES):
        m = dict(shared)
        m["x_cT"] = xTs[c]
        m["mask1"] = m1s[c]
        m["mask2"] = m2s[c]
        m["dis1"] = dis1s[c]
        m["dis2"] = dis2s[c]
        m["idx16_1"] = idx16_1[c]
        m["dstp_1"] = dstp_1[c]
        m["ewb_1"] = ewb_1[c]
        m["ewbn_1"] = ewbn_1[c]
        m["idx16_2"] = idx16_2[c]
        m["dstp_2"] = dstp_2[c]
        m["ewb_2"] = ewb_2[c]
        m["ewbn_2"] = ewbn_2[c]
        m["idx16_c"] = idx16_c[c]
        in_maps.append(m)
    return meta, in_maps


# ------------------------------------------------------------- device kernel

def _build(meta, NCLS=10, H=64):
    import concourse.bass as bass
    import concourse.bacc as bacc
    import concourse.mybir as mybir
    import concourse.tile as tile
    from concourse.masks import make_identity
    from concourse import library_config

    f32 = mybir.dt.float32
    bf16 = mybir.dt.bfloat16
    i16 = mybir.dt.int16
    ALU = mybir.AluOpType
    ACTF = mybir.ActivationFunctionType
    AX = mybir.AxisListType

    B = meta["B"]
    T1, T2 = meta["T1"], meta["T2"]
    rows1, rows2 = meta["rows1"], meta["rows2"]
    FIN = meta["FIN"]
    mp1, mp2, cov = meta["mp1"], meta["mp2"], meta["cov"]
    RG = [list(range(NCORES))]

    nc = bacc.Bacc("TRN2", target_bir_lowering=False, debug=False,
                   num_devices=NCORES, num_swdge_queues=4)

    ein = lambda n, s, d=f32: nc.dram_tensor(n, s, d, kind="ExternalInput")
    x_cT = ein("x_cT", [FIN, rows1])
    mask1 = ein("mask1", [PART, T1]); mask2 = ein("mask2", [PART, T2])
    dis1_d = ein("dis1", [PART, T1]); dis2_d = ein("dis2", [PART, T2])
    idx16_1 = ein("idx16_1", [PART, mp1["S"] // 16], i16)
    dstp_1 = ein("dstp_1", [PART, max(mp1["nruns"], 1)])
    ewb_1 = ein("ewb_1", [PART, max(mp1["nblk"], 1)])
    ewbn_1 = ein("ewbn_1", [PART, max(mp1["nblk"], 1)])
    idx16_2 = ein("idx16_2", [PART, mp2["S"] // 16], i16)
    dstp_2 = ein("dstp_2", [PART, max(mp2["nruns"], 1)])
    ewb_2 = ein("ewb_2", [PART, max(mp2["nblk"], 1)])
    ewbn_2 = ein("ewbn_2", [PART, max(mp2["nblk"], 1)])
    idx16_c = ein("idx16_c", [PART, cov["S"] // 16], i16)
    iota_d = ein("iota", [PART, PART])
    iotan_d = ein("iotan", [PART, PART])
    wshapes = {"W_in0": [FIN, H], "W_in1": [H, H], "Wl_in": [2 * H, H],
               "W_b0": [2 * H, H], "W_b1": [H, H], "Wl_b": [2 * H, H]}
    Ws = {n: ein(n, s) for n, s in wshapes.items()}
    bs = {n: ein(n, [PART, H]) for n in
          ("b_in0", "b_in1", "bl_in", "b_b0", "b_b1", "bl_b")}
    gammaS = ein("gammaS", [PART, 1]); gammaM = ein("gammaM", [PART, 1])
    betaS = ein("betaS", [PART, 1]); betaM = ein("betaM", [PART, 1])
    l1WS = ein("l1WS", [PART, H]); l1WM = ein("l1WM", [PART, H])
    l1b = ein("l1b", [PART, H])
    l2W = ein("l2W", [H, NCLS]); l2b = ein("l2b", [PART, NCLS])
    out_ext = nc.dram_tensor("out", [B, NCLS], f32, kind="ExternalOutput")

    # internal DRAM: wide bf16 tables (upper half junk, never read)
    hs_c1 = nc.dram_tensor("hs_c1", [rows1, WIDE], bf16)
    hs_full1 = nc.dram_tensor("hs_full1", [NCORES * rows1, WIDE], bf16, addr_space="Shared")
    hs_c1b = nc.dram_tensor("hs_c1b", [rows1, WIDE], bf16)
    hs_full1b = nc.dram_tensor("hs_full1b", [NCORES * rows1, WIDE], bf16, addr_space="Shared")
    x1_c = nc.dram_tensor("x1_c", [rows1, WIDE], bf16)
    x1_full = nc.dram_tensor("x1_full", [NCORES * rows1, WIDE], bf16, addr_space="Shared")
    hs_c2 = nc.dram_tensor("hs_c2", [rows2, WIDE], bf16)
    hs_full2 = nc.dram_tensor("hs_full2", [NCORES * rows2, WIDE], bf16, addr_space="Shared")
    hs_c2b = nc.dram_tensor("hs_c2b", [rows2, WIDE], bf16)
    hs_full2b = nc.dram_tensor("hs_full2b", [NCORES * rows2, WIDE], bf16, addr_space="Shared")
    arS_in = nc.dram_tensor("arS_in", [PART, B], f32)
    arS_out = nc.dram_tensor("arS_out", [PART, B], f32, addr_space="Shared")
    arM_in = nc.dram_tensor("arM_in", [PART, B], f32)
    arM_out = nc.dram_tensor("arM_out", [PART, B], f32, addr_space="Shared")

    with tile.TileContext(nc) as tc:
        nc.gpsimd.load_library(library_config.mlp)
        with (tc.tile_pool(name="const", bufs=1) as cpool,
              tc.tile_pool(name="res", bufs=1) as rpool,
              tc.tile_pool(name="gtp", bufs=3) as gtpool,
              tc.tile_pool(name="stg", bufs=3) as stgpool,
              tc.tile_pool(name="work", bufs=2) as wpool,
              tc.tile_pool(name="ps", bufs=3, space="PSUM") as pspool,
              tc.tile_pool(name="psacc", bufs=1, space="PSUM") as papool):

            ident = cpool.tile([PART, PART], f32, tag="ident")
            make_identity(nc, ident[:])

            def load2d(dram, shape, dt=f32, tag=None):
                t = cpool.tile(list(shape), dt, tag=tag or dram.name)
                nc.sync.dma_start(t[:], dram[:, :])
                return t

            identB = cpool.tile([PART, PART], bf16, tag="identB")
            make_identity(nc, identB[:])
            mask1_s = load2d(mask1, (PART, T1))
            mask2_s = load2d(mask2, (PART, T2))
            dis1 = load2d(dis1_d, (PART, T1), tag="dis1s")
            dis2 = load2d(dis2_d, (PART, T2), tag="dis2s")
            iota_s = load2d(iota_d, (PART, PART))
            iotan_s = load2d(iotan_d, (PART, PART), tag="iotan")
            W_s = {n: load2d(Ws[n], Ws[n].shape) for n in Ws}
            b_s = {n: load2d(bs[n], (PART, H)) for n in bs}
            l1WS_s = load2d(l1WS, (PART, H)); l1WM_s = load2d(l1WM, (PART, H))
            l1b_s = load2d(l1b, (PART, H))
            l2W_s = load2d(l2W, (H, NCLS)); l2b_s = load2d(l2b, (PART, NCLS))
            gS_s = load2d(gammaS, (PART, 1)); gM_s = load2d(gammaM, (PART, 1))
            bS_s = load2d(betaS, (PART, 1)); bM_s = load2d(betaM, (PART, 1))

            # f32 activation accumulators: [..., 0, :] = layer a / cover sum,
            # [..., 1, :] = layer b / cover max
            acc1 = rpool.tile([PART, T1, 2, H], f32, tag="acc1")
            acc2 = rpool.tile([PART, T2, 2, H], f32, tag="acc2")
            hs1_sb = rpool.tile([PART, T1, H], bf16, tag="hs1_sb")
            hs2_sb = rpool.tile([PART, T2, H], bf16, tag="hs2_sb")
            rm1 = rpool.tile([PART, B, H], f32, tag="rm1")
            rm2 = rpool.tile([PART, B, H], f32, tag="rm2")
            oneh = rpool.tile([PART, B, B], f32, tag="oneh")
            nc.vector.memset(rm1[:], 0.0)
            nc.vector.memset(rm2[:], 0.0)
            nc.vector.memset(oneh[:], 0.0)
            for g in range(B):
                nc.vector.memset(oneh[:, g, g:g + 1], 1.0)

            def bc_mid(ap2d, G):
                a = ap2d.ap
                return bass.AP(ap2d.tensor, ap2d.offset,
                               [a[0], [0, G], a[-1]])

            ps_sum1 = papool.tile([B, H], f32, tag="sum1")
            ps_sum2 = papool.tile([B, H], f32, tag="sum2")

            def stripes(T, step):
                return [(s, min(s + step, T)) for s in range(0, T, step)]

            # ---- dense matmul phase: hs = dis * (act @ W) -> SBUF + DRAM ----
            def mm_phase(lhsT_fn, Tn, W, dis_t, hs_sb, hs_dram):
                hsd = hs_dram.ap().rearrange("(t p) f -> p t f", p=PART)
                for (s0, s1) in stripes(Tn, STRIPE):
                    for t in range(s0, s1):
                        lhsT = lhsT_fn(t)
                        mm = pspool.tile([PART, H], f32, tag="mm")
                        nc.tensor.matmul(out=mm[:], lhsT=lhsT, rhs=W[:],
                                         start=True, stop=True)
                        nc.vector.tensor_scalar(
                            out=hs_sb[:, t, :], in0=mm[:],
                            scalar1=dis_t[:, t:t + 1], scalar2=None,
                            op0=ALU.mult)
                    nc.sync.dma_start(hsd[:, s0:s1, 0:H], hs_sb[:, s0:s1, :])

            def lhsT_transpose(src_fn, kdim):
                def fn(t):
                    tp = pspool.tile([PART, PART], f32, tag="tp")
                    nc.tensor.transpose(tp[:kdim, :], src_fn(t), ident[:])
                    tsb = wpool.tile([PART, PART], f32, tag="tsb", bufs=5)
                    nc.scalar.copy(out=tsb[:kdim, :], in_=tp[:kdim, :])
                    return tsb[:kdim, :]
                return fn

            xTv = x_cT.ap()
            _xc = {}

            def lhsT_x(t):
                s0 = (t // STRIPE_X) * STRIPE_X
                if s0 not in _xc:
                    xstg = stgpool.tile([FIN, STRIPE_X * PART], f32,
                                        tag="xstg")
                    s1 = min(s0 + STRIPE_X, T1)
                    nc.sync.dma_start(xstg[:, :(s1 - s0) * PART],
                                      xTv[:, s0 * PART:s1 * PART])
                    _xc[s0] = xstg
                return _xc[s0][:, (t - s0) * PART:(t - s0 + 1) * PART]

            def allgather(src, dst):
                nc.gpsimd.collective_compute(
                    "AllGather", ALU.bypass, ins=[src.ap().opt()],
                    outs=[dst.ap().opt()], replica_groups=RG)

            # ---- mp phase: tile-major super-chunks; per tile one PSUM
            # accumulation group (self matmul + one matmul per run), fused
            # epilogue acc = relu((sum + hs_self)*dis + bias) ----
            _qrot = [0]

            def mp_phase(mp, hs_full, idx16_d, dstp_d, ewb_d, ewbn_d, rows,
                         acc, half, hs_sb, dis_t, bias, tile_cb=None):
                runs_all = mp["runs_all"]
                mrun, mblk = mp["mrun"], mp["mblk"]
                mctr = 0
                for sch in mp["schunks"]:
                    r0, r1 = sch["r0"], sch["r1"]
                    b0, b1 = sch["b0"], sch["b1"]
                    dst_t = stgpool.tile([PART, mrun], f32, tag="dstpstg")
                    nc.sync.dma_start(dst_t[:, :r1 - r0], dstp_d[:, r0:r1])
                    ew_t = stgpool.tile([PART, mblk], f32, tag="ewstg")
                    nc.sync.dma_start(ew_t[:, :b1 - b0], ewb_d[:, b0:b1])
                    ewn_t = stgpool.tile([PART, mblk], f32, tag="ewnstg")
                    nc.sync.dma_start(ewn_t[:, :b1 - b0], ewbn_d[:, b0:b1])
                    gts = []
                    for (c, a, npad) in sch["subs"]:
                        idxt = stgpool.tile([PART, NIDX_MAX // 16], i16,
                                            tag="idxstg", name="idxt",
                                            bufs=12)
                        nc.sync.dma_start(
                            idxt[:, :npad // 16],
                            idx16_d[:, a // 16:(a + npad) // 16])
                        gt = gtpool.tile([PART, NIDX_MAX // 128, WIDE],
                                         bf16, tag="gt", name="gt", bufs=8)
                        nc.gpsimd.dma_gather(
                            gt[:, :npad // 128, :],
                            hs_full[c * rows:(c + 1) * rows, :],
                            idxt[:, :npad // 16], npad, npad, WIDE,
                            queue_num=_qrot[0] % 4)
                        _qrot[0] += 1
                        gts.append(gt)
                    for t in range(sch["t0"], sch["t1"]):
                        ops = sch["tile_ops"][t]
                        ps = pspool.tile([PART, H], f32, tag="mm")
                        nc.tensor.matmul(out=ps[:], lhsT=identB[:],
                                         rhs=hs_sb[:, t, :], start=True,
                                         stop=(len(ops) == 0),
                                         skip_group_check=True)
                        for j, (sub_id, bl, rid) in enumerate(ops):
                            be = runs_all[rid][0]
                            M = wpool.tile([PART, PART], bf16, tag="M", bufs=6)
                            if mctr % 2 == 0:
                                nc.vector.tensor_scalar(
                                    out=M[:], in0=iotan_s[:],
                                    scalar1=dst_t[:, rid - r0:rid - r0 + 1],
                                    scalar2=ew_t[:, be - b0:be - b0 + 1],
                                    op0=ALU.is_equal, op1=ALU.mult)
                            else:
                                msq = wpool.tile([PART, PART], f32,
                                                 tag="msq", bufs=2)
                                nc.scalar.activation(
                                    msq[:], iota_s[:], ACTF.Square,
                                    bias=dst_t[:, rid - r0:rid - r0 + 1])
                                nc.scalar.activation(
                                    M[:], msq[:], ACTF.Relu,
                                    bias=ew_t[:, be - b0:be - b0 + 1],
                                    scale=ewn_t[:, be - b0:be - b0 + 1])
                            mctr += 1
                            nc.tensor.matmul(out=ps[:], lhsT=M[:],
                                             rhs=gts[sub_id][:, bl, 0:H],
                                             start=False,
                                             stop=(j == len(ops) - 1),
                                             skip_group_check=True)
                        ept = wpool.tile([PART, H], f32, tag="ept", bufs=5)
                        nc.vector.tensor_scalar(
                            out=ept[:], in0=ps[:],
                            scalar1=dis_t[:, t:t + 1], scalar2=None,
                            op0=ALU.mult)
                        nc.vector.tensor_tensor(out=ept[:], in0=ept[:],
                                                in1=bias[:], op=ALU.add)
                        nc.scalar.activation(acc[:, t, half, :], ept[:],
                                             ACTF.Relu)
                    if tile_cb is not None:
                        tile_cb(sch["t0"], sch["t1"])

            # per-tile mm work folded into a preceding mp phase (emission
            # interleaving hides the dense chains under gather drains)
            def mm_tile_cb(lhsT_fn, W, dis_t, hs_sb, hs_dram):
                hsd = hs_dram.ap().rearrange("(t p) f -> p t f", p=PART)

                def cb(t0, t1):
                    for t in range(t0, t1):
                        lhsT = lhsT_fn(t)
                        mm = pspool.tile([PART, H], f32, tag="mm")
                        nc.tensor.matmul(out=mm[:], lhsT=lhsT, rhs=W[:],
                                         start=True, stop=True,
                                         skip_group_check=True)
                        nc.vector.tensor_scalar(
                            out=hs_sb[:, t, :], in0=mm[:],
                            scalar1=dis_t[:, t:t + 1], scalar2=None,
                            op0=ALU.mult)
                    nc.sync.dma_start(hsd[:, t0:t1, 0:H], hs_sb[:, t0:t1, :])
                return cb

            def jk_tile_cb(acc, Tn, Wl, bias, mask_s, tg, ps_sum, rm,
                           x_dram):
                lfn = lhsT_transpose(
                    lambda t: acc[:, t, :, :].rearrange("p a b -> p (a b)"),
                    PART)
                xd = (x_dram.ap().rearrange("(t p) f -> p t f", p=PART)
                      if x_dram is not None else None)

                def cb(t0, t1):
                    stg = (stgpool.tile([PART, EPI_CH, H], bf16, tag="x1stg",
                                        name="stg")
                           if xd is not None else None)
                    for t in range(t0, t1):
                        lhsT = lfn(t)
                        mm = pspool.tile([PART, H], f32, tag="mm")
                        nc.tensor.matmul(out=mm[:], lhsT=lhsT, rhs=Wl[:],
                                         start=True, stop=True,
                                         skip_group_check=True)
                        xt = wpool.tile([PART, H], f32, tag="xt", bufs=5)
                        nc.vector.tensor_tensor(out=xt[:], in0=mm[:],
                                                in1=bias[:], op=ALU.add)
                        nc.scalar.activation(xt[:], xt[:], ACTF.Relu,
                                             scale=mask_s[:, t:t + 1])
                        g = int(tg[t])
                        nc.tensor.matmul(out=ps_sum[:], lhsT=oneh[:, g, :],
                                         rhs=xt[:], start=(t == 0),
                                         stop=(t == Tn - 1),
                                         skip_group_check=True)
                        nc.vector.tensor_tensor(out=rm[:, g, :],
                                                in0=rm[:, g, :],
                                                in1=xt[:], op=ALU.max)
                        if stg is not None:
                            nc.scalar.copy(out=stg[:, t - t0, :], in_=xt[:])
                    if stg is not None:
                        nc.sync.dma_start(xd[:, t0:t1, 0:H],
                                          stg[:, :t1 - t0, :])
                return cb

            # ---- jk: cat(a,b) @ Wl + bias, relu*mask, readouts ----
            def jk_phase(acc, Tn, Wl, bias, mask_s, tg, ps_sum, rm, x_dram):
                lfn = lhsT_transpose(
                    lambda t: acc[:, t, :, :].rearrange("p a b -> p (a b)"),
                    PART)
                xd = (x_dram.ap().rearrange("(t p) f -> p t f", p=PART)
                      if x_dram is not None else None)
                for (s0, s1) in stripes(Tn, EPI_CH):
                    stg = (stgpool.tile([PART, EPI_CH, H], bf16, tag="x1stg",
                                        name="stg")
                           if x_dram is not None else None)
                    for t in range(s0, s1):
                        lhsT = lfn(t)
                        mm = pspool.tile([PART, H], f32, tag="mm")
                        nc.tensor.matmul(out=mm[:], lhsT=lhsT, rhs=Wl[:],
                                         start=True, stop=True)
                        xt = wpool.tile([PART, H], f32, tag="xt", bufs=5)
                        nc.vector.tensor_tensor(out=xt[:], in0=mm[:],
                                                in1=bias[:], op=ALU.add)
                        nc.scalar.activation(xt[:], xt[:], ACTF.Relu,
                                             scale=mask_s[:, t:t + 1])
                        g = int(tg[t])
                        nc.tensor.matmul(out=ps_sum[:], lhsT=oneh[:, g, :],
                                         rhs=xt[:], start=(t == 0),
                                         stop=(t == Tn - 1),
                                         skip_group_check=True)
                        nc.vector.tensor_tensor(out=rm[:, g, :],
                                                in0=rm[:, g, :],
                                                in1=xt[:], op=ALU.max)
                        if stg is not None:
                            nc.scalar.copy(out=stg[:, t - s0, :], in_=xt[:])
                    if stg is not None:
                        nc.sync.dma_start(xd[:, s0:s1, 0:H],
                                          stg[:, :s1 - s0, :])

            # ================= pipeline =================
            with nc.named_scope("mm1a"):
                mm_phase(lhsT_x, T1, W_s["W_in0"], dis1, hs1_sb, hs_c1)
            with nc.named_scope("ag1a"):
                allgather(hs_c1, hs_full1)
            with nc.named_scope("mp1a"):
                mp_phase(mp1, hs_full1, idx16_1, dstp_1, ewb_1, ewbn_1,
                         rows1, acc1, 0, hs1_sb, dis1, b_s["b_in0"],
                         tile_cb=mm_tile_cb(
                             lhsT_transpose(lambda t: acc1[:, t, 0, :], H),
                             W_s["W_in1"], dis1, hs1_sb, hs_c1b))
            with nc.named_scope("ag1b"):
                allgather(hs_c1b, hs_full1b)
            with nc.named_scope("mp1b"):
                mp_phase(mp1, hs_full1b, idx16_1, dstp_1, ewb_1, ewbn_1,
                         rows1, acc1, 1, hs1_sb, dis1, b_s["b_in1"],
                         tile_cb=jk_tile_cb(acc1, T1, W_s["Wl_in"],
                                            b_s["bl_in"], mask1_s,
                                            meta["tg1"], ps_sum1, rm1,
                                            x1_c))
            with nc.named_scope("agx1"):
                allgather(x1_c, x1_full)

            # ---------- cover pooling (tile-major): sum -> acc1[...,0],
            # max -> [...,1]; mm2a per-tile work folded in ----------
            with nc.named_scope("cover"):
                k_tc = cov["k_tc"]; col_off = cov["col_off"]
                mm2a_cb = mm_tile_cb(
                    lhsT_transpose(
                        lambda t: acc1[:, t, :, :].rearrange(
                            "p a b -> p (a b)"), PART),
                    W_s["W_b0"], dis2, hs2_sb, hs_c2)
                for (t0, t1) in cov["tranges"]:
                    gts = []
                    for c in range(NCORES):
                        a = int(col_off[t0, c])
                        ncols = int(k_tc[t0:t1, c].sum())
                        if ncols == 0:
                            gts.append(None)
                            continue
                        n = ncols * 128
                        idxt = stgpool.tile([PART, NIDX_MAX // 16], i16,
                                            tag="idxstg", name="idxt",
                                            bufs=12)
                        nc.sync.dma_start(idxt[:, :n // 16],
                                          idx16_c[:, a * 8:(a + ncols) * 8])
                        gt = gtpool.tile([PART, NIDX_MAX // 128, WIDE],
                                         bf16, tag="gt", name="gt", bufs=8)
                        nc.gpsimd.dma_gather(
                            gt[:, :ncols, :],
                            x1_full[c * rows1:(c + 1) * rows1, :],
                            idxt[:, :n // 16], n, n, WIDE,
                            queue_num=_qrot[0] % 4)
                        _qrot[0] += 1
                        gts.append(gt)
                    for t in range(t0, t1):
                        first = True
                        for c in range(NCORES):
                            k = int(k_tc[t, c])
                            if k == 0:
                                continue
                            ca = int(col_off[t, c]) - int(col_off[t0, c])
                            view = gts[c][:, ca:ca + k, 0:H].rearrange(
                                "p k f -> p f k")
                            zs = wpool.tile([PART, H], f32, tag="zs",
                                            bufs=5)
                            nc.vector.tensor_reduce(out=zs[:], in_=view,
                                                    axis=AX.X, op=ALU.add)
                            zm = wpool.tile([PART, H], f32, tag="zm",
                                            bufs=5)
                            nc.vector.tensor_reduce(out=zm[:], in_=view,
                                                    axis=AX.X, op=ALU.max)
                            if first:
                                nc.vector.tensor_copy(acc1[:, t, 0, :],
                                                      zs[:])
                                nc.vector.tensor_copy(acc1[:, t, 1, :],
                                                      zm[:])
                                first = False
                            else:
                                nc.vector.tensor_tensor(
                                    out=acc1[:, t, 0, :],
                                    in0=acc1[:, t, 0, :], in1=zs[:],
                                    op=ALU.add)
                                nc.vector.tensor_tensor(
                                    out=acc1[:, t, 1, :],
                                    in0=acc1[:, t, 1, :], in1=zm[:],
                                    op=ALU.max)
                        if first:
                            nc.vector.memset(acc1[:, t, 0, :], 0.0)
                            nc.vector.memset(acc1[:, t, 1, :], 0.0)
                    mm2a_cb(t0, t1)
            with nc.named_scope("ag2a"):
                allgather(hs_c2, hs_full2)
            with nc.named_scope("mp2a"):
                mp_phase(mp2, hs_full2, idx16_2, dstp_2, ewb_2, ewbn_2,
                         rows2, acc2, 0, hs2_sb, dis2, b_s["b_b0"],
                         tile_cb=mm_tile_cb(
                             lhsT_transpose(lambda t: acc2[:, t, 0, :], H),
                             W_s["W_b1"], dis2, hs2_sb, hs_c2b))
            with nc.named_scope("ag2b"):
                allgather(hs_c2b, hs_full2b)
            with nc.named_scope("mp2b"):
                mp_phase(mp2, hs_full2b, idx16_2, dstp_2, ewb_2, ewbn_2,
                         rows2, acc2, 1, hs2_sb, dis2, b_s["b_b1"],
                         tile_cb=jk_tile_cb(acc2, T2, W_s["Wl_b"],
                                            b_s["bl_b"], mask2_s,
                                            meta["tg2"], ps_sum2, rm2,
                                            None))

            # ---------- readout combine + head ----------
            sc_head = nc.named_scope("head"); sc_head.__enter__()
            sum1_sb = wpool.tile([B, H], f32, tag="s1sb")
            nc.scalar.copy(out=sum1_sb[:], in_=ps_sum1[:])
            sum2_sb = wpool.tile([B, H], f32, tag="s2sb")
            nc.scalar.copy(out=sum2_sb[:], in_=ps_sum2[:])
            sT = pspool.tile([H, B], f32, tag="tp")
            nc.tensor.matmul(out=sT[:], lhsT=sum1_sb[:], rhs=ident[:B, :B],
                             start=True, stop=True)
            sT1 = wpool.tile([H, B], f32, tag="sT1")
            nc.scalar.copy(out=sT1[:], in_=sT[:])
            sT_2 = pspool.tile([H, B], f32, tag="tp")
            nc.tensor.matmul(out=sT_2[:], lhsT=sum2_sb[:], rhs=ident[:B, :B],
                             start=True, stop=True)
            sT2 = wpool.tile([H, B], f32, tag="sT2")
            nc.scalar.copy(out=sT2[:], in_=sT_2[:])
            nc.sync.dma_start(arS_in[0:H, :], sT1[:])
            nc.sync.dma_start(arS_in[H:2 * H, :], sT2[:])

            mT1 = wpool.tile([H, B], f32, tag="mT1")
            mT2 = wpool.tile([H, B], f32, tag="mT2")
            for g in range(B):
                for rm, mt in ((rm1, mT1), (rm2, mT2)):
                    tpm = pspool.tile([H, PART], f32, tag="tp")
                    nc.tensor.transpose(tpm[:], rm[:, g, :], ident[:])
                    msb = wpool.tile([H, PART], f32, tag="msb")
                    nc.scalar.copy(out=msb[:], in_=tpm[:])
                    nc.vector.tensor_reduce(out=mt[:, g:g + 1], in_=msb[:],
                                            axis=AX.X, op=ALU.max)
            nc.sync.dma_start(arM_in[0:H, :], mT1[:])
            nc.sync.dma_start(arM_in[H:2 * H, :], mT2[:])

            nc.gpsimd.collective_compute(
                "AllReduce", ALU.add, ins=[arS_in.ap().opt()],
                outs=[arS_out.ap().opt()], replica_groups=RG)
            nc.gpsimd.collective_compute(
                "AllReduce", ALU.max, ins=[arM_in.ap().opt()],
                outs=[arM_out.ap().opt()], replica_groups=RG)

            S_sb = wpool.tile([PART, B], f32, tag="Ssb")
            M_sb = wpool.tile([PART, B], f32, tag="Msb")
            nc.sync.dma_start(S_sb[:], arS_out[:, :])
            nc.sync.dma_start(M_sb[:], arM_out[:, :])

            def bn(t_sb, gam, bet):
                mu = wpool.tile([PART, 1], f32, tag="mu")
                nc.vector.tensor_reduce(out=mu[:], in_=t_sb[:], axis=AX.X,
                                        op=ALU.add)
                nc.vector.tensor_scalar_mul(mu[:], mu[:], 1.0 / B)
                nc.vector.tensor_scalar(out=t_sb[:], in0=t_sb[:],
                                        scalar1=mu[:], scalar2=None,
                                        op0=ALU.subtract)
                sq = wpool.tile([PART, B], f32, tag="sq")
                nc.vector.tensor_tensor(out=sq[:], in0=t_sb[:], in1=t_sb[:],
                                        op=ALU.mult)
                var = wpool.tile([PART, 1], f32, tag="var")
                nc.vector.tensor_reduce(out=var[:], in_=sq[:], axis=AX.X,
                                        op=ALU.add)
                nc.vector.tensor_scalar(out=var[:], in0=var[:],
                                        scalar1=1.0 / B, scalar2=EPS,
                                        op0=ALU.mult, op1=ALU.add)
                nc.scalar.activation(var[:], var[:], ACTF.Sqrt)
                nc.vector.reciprocal(var[:], var[:])
                nc.vector.tensor_scalar(out=t_sb[:], in0=t_sb[:],
                                        scalar1=var[:], scalar2=gam[:],
                                        op0=ALU.mult, op1=ALU.mult)
                nc.vector.tensor_scalar(out=t_sb[:], in0=t_sb[:],
                                        scalar1=bet[:], scalar2=None,
                                        op0=ALU.add)

            bn(S_sb, gS_s, bS_s)
            bn(M_sb, gM_s, bM_s)

            pl1 = pspool.tile([B, H], f32, tag="mm")
            nc.tensor.matmul(out=pl1[:], lhsT=S_sb[:], rhs=l1WS_s[:],
                             start=True, stop=False)
            nc.tensor.matmul(out=pl1[:], lhsT=M_sb[:], rhs=l1WM_s[:],
                             start=False, stop=True)
            y = wpool.tile([B, H], f32, tag="y")
            nc.vector.tensor_tensor(out=y[:], in0=pl1[:], in1=l1b_s[:B, :],
                                    op=ALU.add)
            nc.scalar.activation(y[:], y[:], ACTF.Relu)
            yT_ps = pspool.tile([H, B], f32, tag="tp")
            nc.tensor.matmul(out=yT_ps[:], lhsT=y[:], rhs=ident[:B, :B],
                             start=True, stop=True)
            yT = wpool.tile([H, B], f32, tag="yTs")
            nc.scalar.copy(out=yT[:], in_=yT_ps[:])
            pl2 = pspool.tile([B, NCLS], f32, tag="mm")
            nc.tensor.matmul(out=pl2[:], lhsT=yT[:], rhs=l2W_s[:],
                             start=True, stop=True)
            z = wpool.tile([B, NCLS], f32, tag="z")
            nc.vector.tensor_tensor(out=z[:], in0=pl2[:], in1=l2b_s[:B, :],
                                    op=ALU.add)
            zmax = wpool.tile([B, 1], f32, tag="zmax")
            nc.vector.tensor_reduce(out=zmax[:], in_=z[:], axis=AX.X,
                                    op=ALU.max)
            nc.vector.tensor_scalar(out=z[:], in0=z[:], scalar1=zmax[:],
                                    scalar2=None, op0=ALU.subtract)
            nc.scalar.activation(z[:], z[:], ACTF.Exp)
            zsum = wpool.tile([B, 1], f32, tag="zsum")
            nc.vector.tensor_reduce(out=zsum[:], in_=z[:], axis=AX.X,
                                    op=ALU.add)
            nc.vector.reciprocal(zsum[:], zsum[:])
            nc.vector.tensor_scalar(out=z[:], in0=z[:], scalar1=zsum[:],
                                    scalar2=None, op0=ALU.mult)
            nc.sync.dma_start(out_ext[:, :], z[:])
            sc_head.__exit__(None, None, None)

    nc.compile()
    return nc


def kernel(**inputs):
    from concourse import bass_utils
    meta, in_maps = _prep(inputs)
    nc = _build(meta)
    res = bass_utils.run_bass_kernel_spmd(
        nc, in_maps, core_ids=list(range(NCORES)))
    return np.asarray(res.results[0]["out"])


# revision 49
# speedup vs baseline: 1.6126x; 1.0085x over previous
"""Bass/Trainium2 kernel for nn_KPlexPool (GCN blocks + cover pooling), 8 NeuronCores.

v3: message-passing gathers use dma_gather (one SWDGE instruction per <=4096
slots instead of one per 128 rows), with slot streams sorted by (src_core,
dst) so each instruction's int16 indices fit one core's 32K-row window of the
AllGathered bf16 table.  Segment sums are done by TensorE: per 128-slot block
a one-hot matrix M[slot, dst_partition] = (iota==dstp)*ew is built in one DVE
tensor_scalar op and matmul'd against the gathered rows, accumulating into a
f32 SBUF accumulator per tile.  Cover pooling keeps dst-aligned CSR (needed
for max) bucketed by source core, gathered with dma_gather + DVE reduces.
Activations stay f32 in SBUF (dense matmuls fp32); only the gathered hs/x1
tables, their AllGathers, and the one-hot ew are bf16.  The program is
SPMD-uniform: all slot counts are padded to per-(tile, src_core) maxima over
cores; per-core data (indices, dstp, ew) carries the differences.
"""

import sys
import numpy as np
import ml_dtypes

sys.path.insert(0, "/opt/trn_rl_repo")

PART = 128
NCORES = 8
EPS = 1e-5
NIDX_MAX = 1024      # slots per dma_gather (ring-limited: 2048+ wedges SWDGE)
EPI_CH = 16          # tiles per epilogue chunk
STRIPE = 32          # tiles per hs DMA stripe
STRIPE_X = 8         # tiles per x lhsT load stripe

BF16 = ml_dtypes.bfloat16
WIDE = 128           # gathered-table row width (bf16 -> 256B rows)


# ----------------------------------------------------------------- host prep

def _shard_items(batch, sortkey, B):
    n = batch.shape[0]
    counts = np.bincount(batch, minlength=B)
    starts = np.concatenate([[0], np.cumsum(counts)[:-1]])
    perm_cores = [[] for _ in range(NCORES)]
    tile_graph = []
    for g in range(B):
        cnt = int(counts[g])
        st = int(starts[g])
        base, rem = divmod(cnt, NCORES)
        sizes = [base + (1 if c < rem else 0) for c in range(NCORES)]
        Tg = max(1, -(-max(sizes) // PART))
        tile_graph += [g] * Tg
        off = st
        for c in range(NCORES):
            s = sizes[c]
            ids = np.arange(off, off + s)
            off += s
            order = np.argsort(-sortkey[ids], kind="stable")
            padded = np.full(Tg * PART, -1, dtype=np.int64)
            padded[:s] = ids[order]
            perm_cores[c].append(padded)
    perm = [np.concatenate(p) for p in perm_cores]
    rows = perm[0].shape[0]
    pos = np.full(n, -1, dtype=np.int64)
    for c in range(NCORES):
        real = perm[c] >= 0
        pos[perm[c][real]] = c * rows + np.nonzero(real)[0]
    return perm, pos, rows, np.asarray(tile_graph)


def _wrap16(flat):
    """idx stream [n] -> dma_gather idx layout [128, n/16] int16."""
    n = flat.shape[0]
    assert n % 16 == 0
    w = np.zeros((16, n // 16), dtype=np.int16)
    w[np.arange(n) % 16, np.arange(n) // 16] = flat
    return np.tile(w, (8, 1))


def _mp_stream(dst_loc, src_pos, w, rows, T):
    """(src_core, dst)-sorted slot stream, chunked into tile-major
    super-chunks of 8 per-core sub-gathers so each tile's segment-sum is one
    PSUM accumulation group.

    Returns program-uniform meta + per-core data (idx16, dstpn, ewb, ewbn).
    """
    cnt = np.zeros((NCORES, T, NCORES), np.int64)
    percore = []
    for me in range(NCORES):
        dl, sp, ww = dst_loc[me], src_pos[me], w[me]
        sc = sp // rows
        np.add.at(cnt[me], (dl // PART, sc), 1)
        percore.append((dl, sp, ww, sc))
    n_tc = cnt.max(axis=0)  # [T, NCORES]
    n_tc = ((n_tc + 15) // 16) * 16          # x16 so idx slices stay aligned

    sec_len = n_tc.sum(axis=0) + 128         # +128 slack for x128 rounding
    sec_len_p = ((sec_len + 127) // 128) * 128
    sec_off = np.concatenate([[0], np.cumsum(sec_len_p)])
    S = int(sec_off[-1])

    tc_off = np.zeros((T, NCORES), np.int64)
    for c in range(NCORES):
        off = int(sec_off[c])
        for t in range(T):
            tc_off[t, c] = off
            off += int(n_tc[t, c])

    # super-chunks: tile ranges where every core's sub-range <= NIDX_MAX-127
    lim = NIDX_MAX - 127
    tranges = []
    t0 = 0
    while t0 < T:
        t1 = t0 + 1
        while (t1 < T and t1 - t0 < EPI_CH
               and int(n_tc[t0:t1 + 1].sum(axis=0).max()) <= lim):
            t1 += 1
        tranges.append((t0, t1))
        t0 = t1

    # emission: per super-chunk, per core: sub-gather (a, npad); blocks are
    # instruction-local; runs = (sub, blk_local, tile, lo, hi) -> emission ids
    schunks = []
    nblk_em = 0
    runs_all = []  # run id -> (blk_em, tile, lo, hi, abs_lo)
    for (t0, t1) in tranges:
        subs = []
        tile_ops = {t: [] for t in range(t0, t1)}
        r0, b0 = len(runs_all), nblk_em
        for c in range(NCORES):
            a = int(tc_off[t0, c])
            b = (int(tc_off[t1, c]) if t1 < T
                 else int(tc_off[t1 - 1, c] + n_tc[t1 - 1, c]))
            npad = ((b - a + 127) // 128) * 128
            assert a + npad <= int(sec_off[c + 1]), "section slack exceeded"
            sub_id = len(subs)
            subs.append((c, a, npad))
            for t in range(t0, t1):
                lo = int(tc_off[t, c]) - a
                hi = lo + int(n_tc[t, c])
                s = lo
                while s < hi:
                    bl = s // 128
                    e = min(hi, (bl + 1) * 128)
                    runs_all.append((nblk_em + bl, t, s - bl * 128,
                                     e - bl * 128, a + s))
                    tile_ops[t].append((sub_id, bl, len(runs_all) - 1))
                    s = e
            nblk_em += npad // 128
        schunks.append(dict(t0=t0, t1=t1, subs=subs, tile_ops=tile_ops,
                            r0=r0, r1=len(runs_all), b0=b0, b1=nblk_em))
    nruns_em = len(runs_all)
    mrun = max(1, max(s["r1"] - s["r0"] for s in schunks))
    mblk = max(1, max(s["b1"] - s["b0"] for s in schunks))

    # per-core data tables in emission layout
    idx16s, dstpns, ewbs, ewbns = [], [], [], []
    for me in range(NCORES):
        dl, sp, ww, sc = percore[me]
        order = np.lexsort((dl, sc))
        dls, sps, wws, scs = (a[order] for a in (dl, sp, ww, sc))
        idx = np.zeros(S, np.int16)
        dstp_sl = np.full(S, 255.0, np.float32)
        ew_sl = np.zeros(S, np.float32)
        ptr = 0
        for c in range(NCORES):
            for t in range(T):
                k = int(cnt[me, t, c])
                if k:
                    o = int(tc_off[t, c])
                    sl = slice(ptr, ptr + k)
                    idx[o:o + k] = (sps[sl] % rows).astype(np.int16)
                    dstp_sl[o:o + k] = (dls[sl] % PART).astype(np.float32)
                    ew_sl[o:o + k] = wws[sl]
                    ptr += k
        assert ptr == dls.shape[0]
        dstpn = np.full((PART, max(nruns_em, 1)), -255.0, np.float32)
        for r, (be, t, lo, hi, abs_lo) in enumerate(runs_all):
            dstpn[lo:hi, r] = -dstp_sl[abs_lo:abs_lo + (hi - lo)]
        # per-emission-block ew columns
        ewb = np.zeros((PART, max(nblk_em, 1)), np.float32)
        for sch in schunks:
            be = sch["b0"]
            for (c, a, npad) in sch["subs"]:
                for bl in range(npad // 128):
                    ewb[:, be] = ew_sl[a + bl * 128:a + (bl + 1) * 128]
                    be += 1
        idx16s.append(_wrap16(idx))
        dstpns.append(dstpn)
        ewbs.append(ewb)
        ewbns.append(-ewb)
    meta = dict(S=S, nblk=nblk_em, nruns=nruns_em, schunks=schunks,
                runs_all=runs_all, mrun=mrun, mblk=mblk)
    return meta, idx16s, dstpns, ewbs, ewbns


def _cover_stream(cl_loc, src_pos, rows1, rows2, T2, pad_local):
    """Dst-aligned per-src-core-bucketed CSR for cover sum+max.

    Returns meta (k_tc [T2, NCORES], chunks, col layout) + per-core idx16."""
    cnt_pc = np.zeros((NCORES, T2 * PART, NCORES), np.int32)
    percore = []
    for me in range(NCORES):
        cl, sp = cl_loc[me], src_pos[me]
        sc = sp // rows1
        np.add.at(cnt_pc[me], (cl, sc), 1)
        percore.append((cl, sp, sc))
    # k per (tile, src core): max over partitions and cores
    k_tc = cnt_pc.reshape(NCORES, T2, PART, NCORES).max(axis=(0, 2))  # [T2, C]
    sec_cols = k_tc.sum(axis=0)  # columns per section
    col_off = np.zeros((T2, NCORES), np.int64)
    acc = 0
    sec_col0 = np.zeros(NCORES + 1, np.int64)
    for c in range(NCORES):
        sec_col0[c] = acc
        for t in range(T2):
            col_off[t, c] = acc
            acc += int(k_tc[t, c])
    sec_col0[NCORES] = acc
    Scols = acc
    S = Scols * 128

    # tile-major super-chunks: per tile range, one sub-gather per src core
    # (every per-core column span must fit one NIDX_MAX gather)
    maxcols = NIDX_MAX // 128
    assert int(k_tc.max()) <= maxcols, "single (t,c) exceeds one gather"
    tranges = []
    t0 = 0
    while t0 < T2:
        t1 = t0 + 1
        while (t1 < T2 and t1 - t0 < EPI_CH
               and int(k_tc[t0:t1 + 1].sum(axis=0).max()) <= maxcols):
            t1 += 1
        tranges.append((t0, t1))
        t0 = t1

    idx16s = []
    for me in range(NCORES):
        cl, sp, sc = percore[me]
        idx_cols = np.full((PART, Scols), -1, np.int64)
        for c in range(NCORES):
            m = sc == c
            cls, sps = cl[m], sp[m]
            order = np.argsort(cls, kind="stable")
            cls, sps = cls[order], sps[order]
            ccount = np.bincount(cls, minlength=T2 * PART)
            first = np.concatenate([[0], np.cumsum(ccount)[:-1]])
            rank = np.arange(cls.shape[0]) - first[cls]
            tt = cls // PART
            p = cls % PART
            col = col_off[tt, c] + rank
            idx_cols[p, col] = sps % rows1
            # pads for this section -> core-local zero row
            secsl = slice(int(sec_col0[c]), int(sec_col0[c + 1]))
            sub = idx_cols[:, secsl]
            sub[sub < 0] = pad_local[c]
            idx_cols[:, secsl] = sub
        # slot stream: col-major (slot i = col*128 + p)
        flat = idx_cols.T.reshape(-1).astype(np.int16)
        idx16s.append(_wrap16(flat))
    meta = dict(k_tc=k_tc, col_off=col_off, tranges=tranges, Scols=Scols,
                S=S)
    return meta, idx16s


def _prep(inputs):
    f32 = np.float32
    x = np.asarray(inputs["x"], f32)
    ei = np.asarray(inputs["edge_index"], np.int64)
    wts = np.asarray(inputs["weights"], f32)
    batch = np.asarray(inputs["batch"], np.int64)
    cover_n = np.asarray(inputs["cover_n"], np.int64)
    cover_c = np.asarray(inputs["cover_c"], np.int64)
    ei2 = np.asarray(inputs["edge_index2"], np.int64)
    wts2 = np.asarray(inputs["weights2"], f32)
    batch2 = np.asarray(inputs["batch2"], np.int64)
    N = x.shape[0]
    C = batch2.shape[0]
    B = int(batch.max()) + 1 if batch.size else 1
    B = max(B, int(batch2.max()) + 1)

    indeg = np.bincount(ei[1], minlength=N)
    perm1, pos1, rows1, tg1 = _shard_items(batch, indeg, B)
    covercnt = np.bincount(cover_c, minlength=C)
    perm2, pos2, rows2, tg2 = _shard_items(batch2, covercnt, B)
    T1, T2 = rows1 // PART, rows2 // PART

    # per-core pad (zero x1) local rows
    pad_local = []
    for c in range(NCORES):
        pads = np.nonzero(perm1[c] < 0)[0]
        assert pads.size, f"core {c} has no pad node row"
        pad_local.append(int(pads[0]))

    # mp level 1: edges grouped by dst core
    dpos = pos1[ei[1]]
    spos = pos1[ei[0]]
    dl1, sp1, w1 = [], [], []
    for me in range(NCORES):
        m = (dpos // rows1) == me
        dl1.append(dpos[m] % rows1)
        sp1.append(spos[m])
        w1.append(wts[m])
    mp1, idx16_1, dstp_1, ewb_1, ewbn_1 = _mp_stream(dl1, sp1, w1, rows1, T1)

    # mp level 2
    dpos2 = pos2[ei2[1]]
    spos2 = pos2[ei2[0]]
    dl2, sp2, w2 = [], [], []
    for me in range(NCORES):
        m = (dpos2 // rows2) == me
        dl2.append(dpos2[m] % rows2)
        sp2.append(spos2[m])
        w2.append(wts2[m])
    mp2, idx16_2, dstp_2, ewb_2, ewbn_2 = _mp_stream(dl2, sp2, w2, rows2, T2)

    # cover
    cpos = pos2[cover_c]
    npos = pos1[cover_n]
    clc, spc = [], []
    for me in range(NCORES):
        m = (cpos // rows2) == me
        clc.append(cpos[m] % rows2)
        spc.append(npos[m])
    cov, idx16_c = _cover_stream(clc, spc, rows1, rows2, T2, pad_local)

    # per-core dense transposed inputs (f32) and masks
    xTs, m1s, m2s = [], [], []
    for c in range(NCORES):
        pc = perm1[c]
        xc = np.zeros((rows1, x.shape[1]), f32)
        xc[pc >= 0] = x[pc[pc >= 0]]
        xTs.append(np.ascontiguousarray(xc.T))
        m1s.append(np.ascontiguousarray(
            (pc >= 0).astype(f32).reshape(T1, PART).T))
        p2 = perm2[c]
        m2s.append(np.ascontiguousarray(
            (p2 >= 0).astype(f32).reshape(T2, PART).T))

    # degree tables (host-side: exact f32) -> dis = rsqrt(deg+1)*mask
    deg1 = np.zeros(NCORES * rows1, f32)
    np.add.at(deg1, dpos, wts)
    deg2 = np.zeros(NCORES * rows2, f32)
    np.add.at(deg2, dpos2, wts2)
    dis1s, dis2s = [], []
    for c in range(NCORES):
        d1 = 1.0 / np.sqrt(deg1[c * rows1:(c + 1) * rows1] + 1.0)
        d1 = d1.reshape(T1, PART).T * m1s[c]
        dis1s.append(np.ascontiguousarray(d1).astype(f32))
        d2 = 1.0 / np.sqrt(deg2[c * rows2:(c + 1) * rows2] + 1.0)
        d2 = d2.reshape(T2, PART).T * m2s[c]
        dis2s.append(np.ascontiguousarray(d2).astype(f32))

    meta = dict(B=B, T1=T1, T2=T2, rows1=rows1, rows2=rows2,
                mp1=mp1, mp2=mp2, cov=cov, tg1=tg1, tg2=tg2, FIN=x.shape[1])

    rep = lambda v: np.ascontiguousarray(
        np.broadcast_to(np.asarray(v, f32).reshape(1, -1), (PART, v.shape[-1])))
    g = np.asarray(inputs["bn_gamma"], f32)
    bb = np.asarray(inputs["bn_beta"], f32)
    l1w = np.asarray(inputs["lin1_W"], f32)
    H = np.asarray(inputs["W_in0"], f32).shape[1]
    selS = np.r_[0:H, 2 * H:3 * H]
    selM = np.r_[H:2 * H, 3 * H:4 * H]
    shared = {
        "W_in0": np.asarray(inputs["W_in0"], f32),
        "W_in1": np.asarray(inputs["W_in1"], f32),
        "Wl_in": np.asarray(inputs["Wl_in"], f32),
        "W_b0": np.asarray(inputs["W_b0"], f32),
        "W_b1": np.asarray(inputs["W_b1"], f32),
        "Wl_b": np.asarray(inputs["Wl_b"], f32),
        "b_in0": rep(inputs["b_in0"]), "b_in1": rep(inputs["b_in1"]),
        "bl_in": rep(inputs["bl_in"]), "b_b0": rep(inputs["b_b0"]),
        "b_b1": rep(inputs["b_b1"]), "bl_b": rep(inputs["bl_b"]),
        "gammaS": np.ascontiguousarray(g[selS].reshape(PART, 1)),
        "gammaM": np.ascontiguousarray(g[selM].reshape(PART, 1)),
        "betaS": np.ascontiguousarray(bb[selS].reshape(PART, 1)),
        "betaM": np.ascontiguousarray(bb[selM].reshape(PART, 1)),
        "l1WS": np.ascontiguousarray(l1w[selS]),
        "l1WM": np.ascontiguousarray(l1w[selM]),
        "l1b": rep(inputs["lin1_b"]),
        "l2W": np.asarray(inputs["lin2_W"], f32),
        "l2b": rep(inputs["lin2_b"]),
        "iota": np.ascontiguousarray(
            np.broadcast_to(np.arange(PART, dtype=f32)[None, :],
                            (PART, PART))).astype(BF16),
        "iotan": np.ascontiguousarray(
            np.broadcast_to(-np.arange(PART, dtype=f32)[None, :],
                            (PART, PART))).astype(BF16),
    }
    in_maps = []
    for c in range(NCORES):
        m = dict(shared)
        m["x_cT"] = xTs[c]
        m["mask1"] = m1s[c]
        m["mask2"] = m2s[c]
        m["dis1"] = dis1s[c]
        m["dis2"] = dis2s[c]
        m["idx16_1"] = idx16_1[c]
        m["dstp_1"] = dstp_1[c]
        m["ewb_1"] = ewb_1[c]
        m["ewbn_1"] = ewbn_1[c]
        m["idx16_2"] = idx16_2[c]
        m["dstp_2"] = dstp_2[c]
        m["ewb_2"] = ewb_2[c]
        m["ewbn_2"] = ewbn_2[c]
        m["idx16_c"] = idx16_c[c]
        in_maps.append(m)
    return meta, in_maps


# ------------------------------------------------------------- device kernel

def _build(meta, NCLS=10, H=64):
    import concourse.bass as bass
    import concourse.bacc as bacc
    import concourse.mybir as mybir
    import concourse.tile as tile
    from concourse.masks import make_identity
    from concourse import library_config

    f32 = mybir.dt.float32
    bf16 = mybir.dt.bfloat16
    i16 = mybir.dt.int16
    ALU = mybir.AluOpType
    ACTF = mybir.ActivationFunctionType
    AX = mybir.AxisListType

    B = meta["B"]
    T1, T2 = meta["T1"], meta["T2"]
    rows1, rows2 = meta["rows1"], meta["rows2"]
    FIN = meta["FIN"]
    mp1, mp2, cov = meta["mp1"], meta["mp2"], meta["cov"]
    RG = [list(range(NCORES))]

    nc = bacc.Bacc("TRN2", target_bir_lowering=False, debug=False,
                   num_devices=NCORES, num_swdge_queues=4)

    ein = lambda n, s, d=f32: nc.dram_tensor(n, s, d, kind="ExternalInput")
    x_cT = ein("x_cT", [FIN, rows1])
    mask1 = ein("mask1", [PART, T1]); mask2 = ein("mask2", [PART, T2])
    dis1_d = ein("dis1", [PART, T1]); dis2_d = ein("dis2", [PART, T2])
    idx16_1 = ein("idx16_1", [PART, mp1["S"] // 16], i16)
    dstp_1 = ein("dstp_1", [PART, max(mp1["nruns"], 1)])
    ewb_1 = ein("ewb_1", [PART, max(mp1["nblk"], 1)])
    ewbn_1 = ein("ewbn_1", [PART, max(mp1["nblk"], 1)])
    idx16_2 = ein("idx16_2", [PART, mp2["S"] // 16], i16)
    dstp_2 = ein("dstp_2", [PART, max(mp2["nruns"], 1)])
    ewb_2 = ein("ewb_2", [PART, max(mp2["nblk"], 1)])
    ewbn_2 = ein("ewbn_2", [PART, max(mp2["nblk"], 1)])
    idx16_c = ein("idx16_c", [PART, cov["S"] // 16], i16)
    iota_d = ein("iota", [PART, PART], bf16)
    iotan_d = ein("iotan", [PART, PART], bf16)
    wshapes = {"W_in0": [FIN, H], "W_in1": [H, H], "Wl_in": [2 * H, H],
               "W_b0": [2 * H, H], "W_b1": [H, H], "Wl_b": [2 * H, H]}
    Ws = {n: ein(n, s) for n, s in wshapes.items()}
    bs = {n: ein(n, [PART, H]) for n in
          ("b_in0", "b_in1", "bl_in", "b_b0", "b_b1", "bl_b")}
    gammaS = ein("gammaS", [PART, 1]); gammaM = ein("gammaM", [PART, 1])
    betaS = ein("betaS", [PART, 1]); betaM = ein("betaM", [PART, 1])
    l1WS = ein("l1WS", [PART, H]); l1WM = ein("l1WM", [PART, H])
    l1b = ein("l1b", [PART, H])
    l2W = ein("l2W", [H, NCLS]); l2b = ein("l2b", [PART, NCLS])
    out_ext = nc.dram_tensor("out", [B, NCLS], f32, kind="ExternalOutput")

    # internal DRAM: wide bf16 tables (upper half junk, never read)
    hs_c1 = nc.dram_tensor("hs_c1", [rows1, WIDE], bf16)
    hs_full1 = nc.dram_tensor("hs_full1", [NCORES * rows1, WIDE], bf16, addr_space="Shared")
    hs_c1b = nc.dram_tensor("hs_c1b", [rows1, WIDE], bf16)
    hs_full1b = nc.dram_tensor("hs_full1b", [NCORES * rows1, WIDE], bf16, addr_space="Shared")
    x1_c = nc.dram_tensor("x1_c", [rows1, WIDE], bf16)
    x1_full = nc.dram_tensor("x1_full", [NCORES * rows1, WIDE], bf16, addr_space="Shared")
    hs_c2 = nc.dram_tensor("hs_c2", [rows2, WIDE], bf16)
    hs_full2 = nc.dram_tensor("hs_full2", [NCORES * rows2, WIDE], bf16, addr_space="Shared")
    hs_c2b = nc.dram_tensor("hs_c2b", [rows2, WIDE], bf16)
    hs_full2b = nc.dram_tensor("hs_full2b", [NCORES * rows2, WIDE], bf16, addr_space="Shared")
    arS_in = nc.dram_tensor("arS_in", [PART, B], f32)
    arS_out = nc.dram_tensor("arS_out", [PART, B], f32, addr_space="Shared")
    arM_in = nc.dram_tensor("arM_in", [PART, B], f32)
    arM_out = nc.dram_tensor("arM_out", [PART, B], f32, addr_space="Shared")

    with tile.TileContext(nc) as tc:
        nc.gpsimd.load_library(library_config.mlp)
        with (tc.tile_pool(name="const", bufs=1) as cpool,
              tc.tile_pool(name="res", bufs=1) as rpool,
              tc.tile_pool(name="gtp", bufs=3) as gtpool,
              tc.tile_pool(name="stg", bufs=3) as stgpool,
              tc.tile_pool(name="work", bufs=2) as wpool,
              tc.tile_pool(name="ps", bufs=3, space="PSUM") as pspool,
              tc.tile_pool(name="psacc", bufs=1, space="PSUM") as papool):

            ident = cpool.tile([PART, PART], f32, tag="ident")
            make_identity(nc, ident[:])

            def load2d(dram, shape, dt=f32, tag=None):
                t = cpool.tile(list(shape), dt, tag=tag or dram.name)
                nc.sync.dma_start(t[:], dram[:, :])
                return t

            identB = cpool.tile([PART, PART], bf16, tag="identB")
            make_identity(nc, identB[:])
            mask1_s = load2d(mask1, (PART, T1))
            mask2_s = load2d(mask2, (PART, T2))
            dis1 = load2d(dis1_d, (PART, T1), tag="dis1s")
            dis2 = load2d(dis2_d, (PART, T2), tag="dis2s")
            iota_s = load2d(iota_d, (PART, PART), bf16)
            iotan_s = load2d(iotan_d, (PART, PART), bf16, tag="iotan")
            W_s = {n: load2d(Ws[n], Ws[n].shape) for n in Ws}
            b_s = {n: load2d(bs[n], (PART, H)) for n in bs}
            l1WS_s = load2d(l1WS, (PART, H)); l1WM_s = load2d(l1WM, (PART, H))
            l1b_s = load2d(l1b, (PART, H))
            l2W_s = load2d(l2W, (H, NCLS)); l2b_s = load2d(l2b, (PART, NCLS))
            gS_s = load2d(gammaS, (PART, 1)); gM_s = load2d(gammaM, (PART, 1))
            bS_s = load2d(betaS, (PART, 1)); bM_s = load2d(betaM, (PART, 1))

            # f32 activation accumulators: [..., 0, :] = layer a / cover sum,
            # [..., 1, :] = layer b / cover max
            acc1 = rpool.tile([PART, T1, 2, H], f32, tag="acc1")
            acc2 = rpool.tile([PART, T2, 2, H], f32, tag="acc2")
            hs1_sb = rpool.tile([PART, T1, H], bf16, tag="hs1_sb")
            hs2_sb = rpool.tile([PART, T2, H], bf16, tag="hs2_sb")
            rm1 = rpool.tile([PART, B, H], f32, tag="rm1")
            rm2 = rpool.tile([PART, B, H], f32, tag="rm2")
            oneh = rpool.tile([PART, B, B], f32, tag="oneh")
            nc.vector.memset(rm1[:], 0.0)
            nc.vector.memset(rm2[:], 0.0)
            nc.vector.memset(oneh[:], 0.0)
            for g in range(B):
                nc.vector.memset(oneh[:, g, g:g + 1], 1.0)

            def bc_mid(ap2d, G):
                a = ap2d.ap
                return bass.AP(ap2d.tensor, ap2d.offset,
                               [a[0], [0, G], a[-1]])

            ps_sum1 = papool.tile([B, H], f32, tag="sum1")
            ps_sum2 = papool.tile([B, H], f32, tag="sum2")

            def stripes(T, step):
                return [(s, min(s + step, T)) for s in range(0, T, step)]

            # ---- dense matmul phase: hs = dis * (act @ W) -> SBUF + DRAM ----
            def mm_phase(lhsT_fn, Tn, W, dis_t, hs_sb, hs_dram):
                hsd = hs_dram.ap().rearrange("(t p) f -> p t f", p=PART)
                for (s0, s1) in stripes(Tn, STRIPE):
                    for t in range(s0, s1):
                        lhsT = lhsT_fn(t)
                        mm = pspool.tile([PART, H], f32, tag="mm")
                        nc.tensor.matmul(out=mm[:], lhsT=lhsT, rhs=W[:],
                                         start=True, stop=True)
                        nc.vector.tensor_scalar(
                            out=hs_sb[:, t, :], in0=mm[:],
                            scalar1=dis_t[:, t:t + 1], scalar2=None,
                            op0=ALU.mult)
                    nc.sync.dma_start(hsd[:, s0:s1, 0:H], hs_sb[:, s0:s1, :])

            def lhsT_transpose(src_fn, kdim):
                def fn(t):
                    tp = pspool.tile([PART, PART], f32, tag="tp")
                    nc.tensor.transpose(tp[:kdim, :], src_fn(t), ident[:])
                    tsb = wpool.tile([PART, PART], f32, tag="tsb", bufs=5)
                    nc.scalar.copy(out=tsb[:kdim, :], in_=tp[:kdim, :])
                    return tsb[:kdim, :]
                return fn

            xTv = x_cT.ap()
            _xc = {}

            def lhsT_x(t):
                s0 = (t // STRIPE_X) * STRIPE_X
                if s0 not in _xc:
                    xstg = stgpool.tile([FIN, STRIPE_X * PART], f32,
                                        tag="xstg")
                    s1 = min(s0 + STRIPE_X, T1)
                    nc.sync.dma_start(xstg[:, :(s1 - s0) * PART],
                                      xTv[:, s0 * PART:s1 * PART])
                    _xc[s0] = xstg
                return _xc[s0][:, (t - s0) * PART:(t - s0 + 1) * PART]

            def allgather(src, dst):
                nc.gpsimd.collective_compute(
                    "AllGather", ALU.bypass, ins=[src.ap().opt()],
                    outs=[dst.ap().opt()], replica_groups=RG)

            # ---- mp phase: tile-major super-chunks; per tile one PSUM
            # accumulation group (self matmul + one matmul per run), fused
            # epilogue acc = relu((sum + hs_self)*dis + bias) ----
            _qrot = [0]

            def mp_phase(mp, hs_full, idx16_d, dstp_d, ewb_d, ewbn_d, rows,
                         acc, half, hs_sb, dis_t, bias, tile_cb=None):
                runs_all = mp["runs_all"]
                mrun, mblk = mp["mrun"], mp["mblk"]
                mctr = 0
                for sch in mp["schunks"]:
                    r0, r1 = sch["r0"], sch["r1"]
                    b0, b1 = sch["b0"], sch["b1"]
                    dst_t = stgpool.tile([PART, mrun], f32, tag="dstpstg")
                    nc.sync.dma_start(dst_t[:, :r1 - r0], dstp_d[:, r0:r1])
                    ew_t = stgpool.tile([PART, mblk], f32, tag="ewstg")
                    nc.sync.dma_start(ew_t[:, :b1 - b0], ewb_d[:, b0:b1])
                    ewn_t = stgpool.tile([PART, mblk], f32, tag="ewnstg")
                    nc.sync.dma_start(ewn_t[:, :b1 - b0], ewbn_d[:, b0:b1])
                    gts = []
                    for (c, a, npad) in sch["subs"]:
                        idxt = stgpool.tile([PART, NIDX_MAX // 16], i16,
                                            tag="idxstg", name="idxt",
                                            bufs=12)
                        nc.sync.dma_start(
                            idxt[:, :npad // 16],
                            idx16_d[:, a // 16:(a + npad) // 16])
                        gt = gtpool.tile([PART, NIDX_MAX // 128, WIDE],
                                         bf16, tag="gt", name="gt", bufs=8)
                        nc.gpsimd.dma_gather(
                            gt[:, :npad // 128, :],
                            hs_full[c * rows:(c + 1) * rows, :],
                            idxt[:, :npad // 16], npad, npad, WIDE,
                            queue_num=_qrot[0] % 4)
                        _qrot[0] += 1
                        gts.append(gt)
                    for t in range(sch["t0"], sch["t1"]):
                        ops = sch["tile_ops"][t]
                        ps = pspool.tile([PART, H], f32, tag="mm")
                        nc.tensor.matmul(out=ps[:], lhsT=identB[:],
                                         rhs=hs_sb[:, t, :], start=True,
                                         stop=(len(ops) == 0),
                                         skip_group_check=True)
                        for j, (sub_id, bl, rid) in enumerate(ops):
                            be = runs_all[rid][0]
                            M = wpool.tile([PART, PART], bf16, tag="M", bufs=6)
                            if mctr % 2 == 0:
                                nc.vector.tensor_scalar(
                                    out=M[:], in0=iotan_s[:],
                                    scalar1=dst_t[:, rid - r0:rid - r0 + 1],
                                    scalar2=ew_t[:, be - b0:be - b0 + 1],
                                    op0=ALU.is_equal, op1=ALU.mult)
                            else:
                                msq = wpool.tile([PART, PART], bf16,
                                                 tag="msq", bufs=2)
                                nc.scalar.activation(
                                    msq[:], iota_s[:], ACTF.Square,
                                    bias=dst_t[:, rid - r0:rid - r0 + 1])
                                nc.scalar.activation(
                                    M[:], msq[:], ACTF.Relu,
                                    bias=ew_t[:, be - b0:be - b0 + 1],
                                    scale=ewn_t[:, be - b0:be - b0 + 1])
                            mctr += 1
                            nc.tensor.matmul(out=ps[:], lhsT=M[:],
                                             rhs=gts[sub_id][:, bl, 0:H],
                                             start=False,
                                             stop=(j == len(ops) - 1),
                                             skip_group_check=True)
                        ept = wpool.tile([PART, H], f32, tag="ept", bufs=5)
                        nc.vector.tensor_scalar(
                            out=ept[:], in0=ps[:],
                            scalar1=dis_t[:, t:t + 1], scalar2=None,
                            op0=ALU.mult)
                        nc.vector.tensor_tensor(out=ept[:], in0=ept[:],
                                                in1=bias[:], op=ALU.add)
                        nc.scalar.activation(acc[:, t, half, :], ept[:],
                                             ACTF.Relu)
                    if tile_cb is not None:
                        tile_cb(sch["t0"], sch["t1"])

            # per-tile mm work folded into a preceding mp phase (emission
            # interleaving hides the dense chains under gather drains)
            def mm_tile_cb(lhsT_fn, W, dis_t, hs_sb, hs_dram):
                hsd = hs_dram.ap().rearrange("(t p) f -> p t f", p=PART)

                def cb(t0, t1):
                    for t in range(t0, t1):
                        lhsT = lhsT_fn(t)
                        mm = pspool.tile([PART, H], f32, tag="mm")
                        nc.tensor.matmul(out=mm[:], lhsT=lhsT, rhs=W[:],
                                         start=True, stop=True,
                                         skip_group_check=True)
                        nc.vector.tensor_scalar(
                            out=hs_sb[:, t, :], in0=mm[:],
                            scalar1=dis_t[:, t:t + 1], scalar2=None,
                            op0=ALU.mult)
                    nc.sync.dma_start(hsd[:, t0:t1, 0:H], hs_sb[:, t0:t1, :])
                return cb

            def jk_tile_cb(acc, Tn, Wl, bias, mask_s, tg, ps_sum, rm,
                           x_dram):
                lfn = lhsT_transpose(
                    lambda t: acc[:, t, :, :].rearrange("p a b -> p (a b)"),
                    PART)
                xd = (x_dram.ap().rearrange("(t p) f -> p t f", p=PART)
                      if x_dram is not None else None)

                def cb(t0, t1):
                    stg = (stgpool.tile([PART, EPI_CH, H], bf16, tag="x1stg",
                                        name="stg")
                           if xd is not None else None)
                    for t in range(t0, t1):
                        lhsT = lfn(t)
                        mm = pspool.tile([PART, H], f32, tag="mm")
                        nc.tensor.matmul(out=mm[:], lhsT=lhsT, rhs=Wl[:],
                                         start=True, stop=True,
                                         skip_group_check=True)
                        xt = wpool.tile([PART, H], f32, tag="xt", bufs=5)
                        nc.vector.tensor_tensor(out=xt[:], in0=mm[:],
                                                in1=bias[:], op=ALU.add)
                        nc.scalar.activation(xt[:], xt[:], ACTF.Relu,
                                             scale=mask_s[:, t:t + 1])
                        g = int(tg[t])
                        nc.tensor.matmul(out=ps_sum[:], lhsT=oneh[:, g, :],
                                         rhs=xt[:], start=(t == 0),
                                         stop=(t == Tn - 1),
                                         skip_group_check=True)
                        nc.vector.tensor_tensor(out=rm[:, g, :],
                                                in0=rm[:, g, :],
                                                in1=xt[:], op=ALU.max)
                        if stg is not None:
                            nc.scalar.copy(out=stg[:, t - t0, :], in_=xt[:])
                    if stg is not None:
                        nc.sync.dma_start(xd[:, t0:t1, 0:H],
                                          stg[:, :t1 - t0, :])
                return cb

            # ---- jk: cat(a,b) @ Wl + bias, relu*mask, readouts ----
            def jk_phase(acc, Tn, Wl, bias, mask_s, tg, ps_sum, rm, x_dram):
                lfn = lhsT_transpose(
                    lambda t: acc[:, t, :, :].rearrange("p a b -> p (a b)"),
                    PART)
                xd = (x_dram.ap().rearrange("(t p) f -> p t f", p=PART)
                      if x_dram is not None else None)
                for (s0, s1) in stripes(Tn, EPI_CH):
                    stg = (stgpool.tile([PART, EPI_CH, H], bf16, tag="x1stg",
                                        name="stg")
                           if x_dram is not None else None)
                    for t in range(s0, s1):
                        lhsT = lfn(t)
                        mm = pspool.tile([PART, H], f32, tag="mm")
                        nc.tensor.matmul(out=mm[:], lhsT=lhsT, rhs=Wl[:],
                                         start=True, stop=True)
                        xt = wpool.tile([PART, H], f32, tag="xt", bufs=5)
                        nc.vector.tensor_tensor(out=xt[:], in0=mm[:],
                                                in1=bias[:], op=ALU.add)
                        nc.scalar.activation(xt[:], xt[:], ACTF.Relu,
                                             scale=mask_s[:, t:t + 1])
                        g = int(tg[t])
                        nc.tensor.matmul(out=ps_sum[:], lhsT=oneh[:, g, :],
                                         rhs=xt[:], start=(t == 0),
                                         stop=(t == Tn - 1),
                                         skip_group_check=True)
                        nc.vector.tensor_tensor(out=rm[:, g, :],
                                                in0=rm[:, g, :],
                                                in1=xt[:], op=ALU.max)
                        if stg is not None:
                            nc.scalar.copy(out=stg[:, t - s0, :], in_=xt[:])
                    if stg is not None:
                        nc.sync.dma_start(xd[:, s0:s1, 0:H],
                                          stg[:, :s1 - s0, :])

            # ================= pipeline =================
            with nc.named_scope("mm1a"):
                mm_phase(lhsT_x, T1, W_s["W_in0"], dis1, hs1_sb, hs_c1)
            with nc.named_scope("ag1a"):
                allgather(hs_c1, hs_full1)
            with nc.named_scope("mp1a"):
                mp_phase(mp1, hs_full1, idx16_1, dstp_1, ewb_1, ewbn_1,
                         rows1, acc1, 0, hs1_sb, dis1, b_s["b_in0"],
                         tile_cb=mm_tile_cb(
                             lhsT_transpose(lambda t: acc1[:, t, 0, :], H),
                             W_s["W_in1"], dis1, hs1_sb, hs_c1b))
            with nc.named_scope("ag1b"):
                allgather(hs_c1b, hs_full1b)
            with nc.named_scope("mp1b"):
                mp_phase(mp1, hs_full1b, idx16_1, dstp_1, ewb_1, ewbn_1,
                         rows1, acc1, 1, hs1_sb, dis1, b_s["b_in1"],
                         tile_cb=jk_tile_cb(acc1, T1, W_s["Wl_in"],
                                            b_s["bl_in"], mask1_s,
                                            meta["tg1"], ps_sum1, rm1,
                                            x1_c))
            with nc.named_scope("agx1"):
                allgather(x1_c, x1_full)

            # ---------- cover pooling (tile-major): sum -> acc1[...,0],
            # max -> [...,1]; mm2a per-tile work folded in ----------
            with nc.named_scope("cover"):
                k_tc = cov["k_tc"]; col_off = cov["col_off"]
                mm2a_cb = mm_tile_cb(
                    lhsT_transpose(
                        lambda t: acc1[:, t, :, :].rearrange(
                            "p a b -> p (a b)"), PART),
                    W_s["W_b0"], dis2, hs2_sb, hs_c2)
                for (t0, t1) in cov["tranges"]:
                    gts = []
                    for c in range(NCORES):
                        a = int(col_off[t0, c])
                        ncols = int(k_tc[t0:t1, c].sum())
                        if ncols == 0:
                            gts.append(None)
                            continue
                        n = ncols * 128
                        idxt = stgpool.tile([PART, NIDX_MAX // 16], i16,
                                            tag="idxstg", name="idxt",
                                            bufs=12)
                        nc.sync.dma_start(idxt[:, :n // 16],
                                          idx16_c[:, a * 8:(a + ncols) * 8])
                        gt = gtpool.tile([PART, NIDX_MAX // 128, WIDE],
                                         bf16, tag="gt", name="gt", bufs=8)
                        nc.gpsimd.dma_gather(
                            gt[:, :ncols, :],
                            x1_full[c * rows1:(c + 1) * rows1, :],
                            idxt[:, :n // 16], n, n, WIDE,
                            queue_num=_qrot[0] % 4)
                        _qrot[0] += 1
                        gts.append(gt)
                    for t in range(t0, t1):
                        first = True
                        for c in range(NCORES):
                            k = int(k_tc[t, c])
                            if k == 0:
                                continue
                            ca = int(col_off[t, c]) - int(col_off[t0, c])
                            view = gts[c][:, ca:ca + k, 0:H].rearrange(
                                "p k f -> p f k")
                            zs = wpool.tile([PART, H], f32, tag="zs",
                                            bufs=5)
                            nc.vector.tensor_reduce(out=zs[:], in_=view,
                                                    axis=AX.X, op=ALU.add)
                            zm = wpool.tile([PART, H], f32, tag="zm",
                                            bufs=5)
                            nc.vector.tensor_reduce(out=zm[:], in_=view,
                                                    axis=AX.X, op=ALU.max)
                            if first:
                                nc.vector.tensor_copy(acc1[:, t, 0, :],
                                                      zs[:])
                                nc.vector.tensor_copy(acc1[:, t, 1, :],
                                                      zm[:])
                                first = False
                            else:
                                nc.vector.tensor_tensor(
                                    out=acc1[:, t, 0, :],
                                    in0=acc1[:, t, 0, :], in1=zs[:],
                                    op=ALU.add)
                                nc.vector.tensor_tensor(
                                    out=acc1[:, t, 1, :],
                                    in0=acc1[:, t, 1, :], in1=zm[:],
                                    op=ALU.max)
                        if first:
                            nc.vector.memset(acc1[:, t, 0, :], 0.0)
                            nc.vector.memset(acc1[:, t, 1, :], 0.0)
                    mm2a_cb(t0, t1)
            with nc.named_scope("ag2a"):
                allgather(hs_c2, hs_full2)
            with nc.named_scope("mp2a"):
                mp_phase(mp2, hs_full2, idx16_2, dstp_2, ewb_2, ewbn_2,
                         rows2, acc2, 0, hs2_sb, dis2, b_s["b_b0"],
                         tile_cb=mm_tile_cb(
                             lhsT_transpose(lambda t: acc2[:, t, 0, :], H),
                             W_s["W_b1"], dis2, hs2_sb, hs_c2b))
            with nc.named_scope("ag2b"):
                allgather(hs_c2b, hs_full2b)
            with nc.named_scope("mp2b"):
                mp_phase(mp2, hs_full2b, idx16_2, dstp_2, ewb_2, ewbn_2,
                         rows2, acc2, 1, hs2_sb, dis2, b_s["b_b1"],
                         tile_cb=jk_tile_cb(acc2, T2, W_s["Wl_b"],
                                            b_s["bl_b"], mask2_s,
                                            meta["tg2"], ps_sum2, rm2,
                                            None))

            # ---------- readout combine + head ----------
            sc_head = nc.named_scope("head"); sc_head.__enter__()
            sum1_sb = wpool.tile([B, H], f32, tag="s1sb")
            nc.scalar.copy(out=sum1_sb[:], in_=ps_sum1[:])
            sum2_sb = wpool.tile([B, H], f32, tag="s2sb")
            nc.scalar.copy(out=sum2_sb[:], in_=ps_sum2[:])
            sT = pspool.tile([H, B], f32, tag="tp")
            nc.tensor.matmul(out=sT[:], lhsT=sum1_sb[:], rhs=ident[:B, :B],
                             start=True, stop=True)
            sT1 = wpool.tile([H, B], f32, tag="sT1")
            nc.scalar.copy(out=sT1[:], in_=sT[:])
            sT_2 = pspool.tile([H, B], f32, tag="tp")
            nc.tensor.matmul(out=sT_2[:], lhsT=sum2_sb[:], rhs=ident[:B, :B],
                             start=True, stop=True)
            sT2 = wpool.tile([H, B], f32, tag="sT2")
            nc.scalar.copy(out=sT2[:], in_=sT_2[:])
            nc.sync.dma_start(arS_in[0:H, :], sT1[:])
            nc.sync.dma_start(arS_in[H:2 * H, :], sT2[:])

            mT1 = wpool.tile([H, B], f32, tag="mT1")
            mT2 = wpool.tile([H, B], f32, tag="mT2")
            for g in range(B):
                for rm, mt in ((rm1, mT1), (rm2, mT2)):
                    tpm = pspool.tile([H, PART], f32, tag="tp")
                    nc.tensor.transpose(tpm[:], rm[:, g, :], ident[:])
                    msb = wpool.tile([H, PART], f32, tag="msb")
                    nc.scalar.copy(out=msb[:], in_=tpm[:])
                    nc.vector.tensor_reduce(out=mt[:, g:g + 1], in_=msb[:],
                                            axis=AX.X, op=ALU.max)
            nc.sync.dma_start(arM_in[0:H, :], mT1[:])
            nc.sync.dma_start(arM_in[H:2 * H, :], mT2[:])

            nc.gpsimd.collective_compute(
                "AllReduce", ALU.add, ins=[arS_in.ap().opt()],
                outs=[arS_out.ap().opt()], replica_groups=RG)
            nc.gpsimd.collective_compute(
                "AllReduce", ALU.max, ins=[arM_in.ap().opt()],
                outs=[arM_out.ap().opt()], replica_groups=RG)

            S_sb = wpool.tile([PART, B], f32, tag="Ssb")
            M_sb = wpool.tile([PART, B], f32, tag="Msb")
            nc.sync.dma_start(S_sb[:], arS_out[:, :])
            nc.sync.dma_start(M_sb[:], arM_out[:, :])

            def bn(t_sb, gam, bet):
                mu = wpool.tile([PART, 1], f32, tag="mu")
                nc.vector.tensor_reduce(out=mu[:], in_=t_sb[:], axis=AX.X,
                                        op=ALU.add)
                nc.vector.tensor_scalar_mul(mu[:], mu[:], 1.0 / B)
                nc.vector.tensor_scalar(out=t_sb[:], in0=t_sb[:],
                                        scalar1=mu[:], scalar2=None,
                                        op0=ALU.subtract)
                sq = wpool.tile([PART, B], f32, tag="sq")
                nc.vector.tensor_tensor(out=sq[:], in0=t_sb[:], in1=t_sb[:],
                                        op=ALU.mult)
                var = wpool.tile([PART, 1], f32, tag="var")
                nc.vector.tensor_reduce(out=var[:], in_=sq[:], axis=AX.X,
                                        op=ALU.add)
                nc.vector.tensor_scalar(out=var[:], in0=var[:],
                                        scalar1=1.0 / B, scalar2=EPS,
                                        op0=ALU.mult, op1=ALU.add)
                nc.scalar.activation(var[:], var[:], ACTF.Sqrt)
                nc.vector.reciprocal(var[:], var[:])
                nc.vector.tensor_scalar(out=t_sb[:], in0=t_sb[:],
                                        scalar1=var[:], scalar2=gam[:],
                                        op0=ALU.mult, op1=ALU.mult)
                nc.vector.tensor_scalar(out=t_sb[:], in0=t_sb[:],
                                        scalar1=bet[:], scalar2=None,
                                        op0=ALU.add)

            bn(S_sb, gS_s, bS_s)
            bn(M_sb, gM_s, bM_s)

            pl1 = pspool.tile([B, H], f32, tag="mm")
            nc.tensor.matmul(out=pl1[:], lhsT=S_sb[:], rhs=l1WS_s[:],
                             start=True, stop=False)
            nc.tensor.matmul(out=pl1[:], lhsT=M_sb[:], rhs=l1WM_s[:],
                             start=False, stop=True)
            y = wpool.tile([B, H], f32, tag="y")
            nc.vector.tensor_tensor(out=y[:], in0=pl1[:], in1=l1b_s[:B, :],
                                    op=ALU.add)
            nc.scalar.activation(y[:], y[:], ACTF.Relu)
            yT_ps = pspool.tile([H, B], f32, tag="tp")
            nc.tensor.matmul(out=yT_ps[:], lhsT=y[:], rhs=ident[:B, :B],
                             start=True, stop=True)
            yT = wpool.tile([H, B], f32, tag="yTs")
            nc.scalar.copy(out=yT[:], in_=yT_ps[:])
            pl2 = pspool.tile([B, NCLS], f32, tag="mm")
            nc.tensor.matmul(out=pl2[:], lhsT=yT[:], rhs=l2W_s[:],
                             start=True, stop=True)
            z = wpool.tile([B, NCLS], f32, tag="z")
            nc.vector.tensor_tensor(out=z[:], in0=pl2[:], in1=l2b_s[:B, :],
                                    op=ALU.add)
            zmax = wpool.tile([B, 1], f32, tag="zmax")
            nc.vector.tensor_reduce(out=zmax[:], in_=z[:], axis=AX.X,
                                    op=ALU.max)
            nc.vector.tensor_scalar(out=z[:], in0=z[:], scalar1=zmax[:],
                                    scalar2=None, op0=ALU.subtract)
            nc.scalar.activation(z[:], z[:], ACTF.Exp)
            zsum = wpool.tile([B, 1], f32, tag="zsum")
            nc.vector.tensor_reduce(out=zsum[:], in_=z[:], axis=AX.X,
                                    op=ALU.add)
            nc.vector.reciprocal(zsum[:], zsum[:])
            nc.vector.tensor_scalar(out=z[:], in0=z[:], scalar1=zsum[:],
                                    scalar2=None, op0=ALU.mult)
            nc.sync.dma_start(out_ext[:, :], z[:])
            sc_head.__exit__(None, None, None)

    nc.compile()
    return nc


def kernel(**inputs):
    from concourse import bass_utils
    meta, in_maps = _prep(inputs)
    nc = _build(meta)
    res = bass_utils.run_bass_kernel_spmd(
        nc, in_maps, core_ids=list(range(NCORES)))
    return np.asarray(res.results[0]["out"])


# revision 52
# speedup vs baseline: 1.6655x; 1.0328x over previous
"""Bass/Trainium2 kernel for nn_KPlexPool (GCN blocks + cover pooling), 8 NeuronCores.

v3: message-passing gathers use dma_gather (one SWDGE instruction per <=4096
slots instead of one per 128 rows), with slot streams sorted by (src_core,
dst) so each instruction's int16 indices fit one core's 32K-row window of the
AllGathered bf16 table.  Segment sums are done by TensorE: per 128-slot block
a one-hot matrix M[slot, dst_partition] = (iota==dstp)*ew is built in one DVE
tensor_scalar op and matmul'd against the gathered rows, accumulating into a
f32 SBUF accumulator per tile.  Cover pooling keeps dst-aligned CSR (needed
for max) bucketed by source core, gathered with dma_gather + DVE reduces.
Activations stay f32 in SBUF (dense matmuls fp32); only the gathered hs/x1
tables, their AllGathers, and the one-hot ew are bf16.  The program is
SPMD-uniform: all slot counts are padded to per-(tile, src_core) maxima over
cores; per-core data (indices, dstp, ew) carries the differences.
"""

import sys
import numpy as np
import ml_dtypes

sys.path.insert(0, "/opt/trn_rl_repo")

PART = 128
NCORES = 8
EPS = 1e-5
NIDX_MAX = 1024      # slots per dma_gather (ring-limited: 2048+ wedges SWDGE)
EPI_CH = 16          # tiles per epilogue chunk
STRIPE = 32          # tiles per hs DMA stripe
STRIPE_X = 8         # tiles per x lhsT load stripe

BF16 = ml_dtypes.bfloat16
WIDE = 128           # gathered-table row width (bf16 -> 256B rows)


# ----------------------------------------------------------------- host prep

def _shard_items(batch, sortkey, B):
    n = batch.shape[0]
    counts = np.bincount(batch, minlength=B)
    starts = np.concatenate([[0], np.cumsum(counts)[:-1]])
    perm_cores = [[] for _ in range(NCORES)]
    tile_graph = []
    for g in range(B):
        cnt = int(counts[g])
        st = int(starts[g])
        base, rem = divmod(cnt, NCORES)
        sizes = [base + (1 if c < rem else 0) for c in range(NCORES)]
        Tg = max(1, -(-max(sizes) // PART))
        tile_graph += [g] * Tg
        off = st
        for c in range(NCORES):
            s = sizes[c]
            ids = np.arange(off, off + s)
            off += s
            order = np.argsort(-sortkey[ids], kind="stable")
            padded = np.full(Tg * PART, -1, dtype=np.int64)
            padded[:s] = ids[order]
            perm_cores[c].append(padded)
    perm = [np.concatenate(p) for p in perm_cores]
    rows = perm[0].shape[0]
    pos = np.full(n, -1, dtype=np.int64)
    for c in range(NCORES):
        real = perm[c] >= 0
        pos[perm[c][real]] = c * rows + np.nonzero(real)[0]
    return perm, pos, rows, np.asarray(tile_graph)


def _wrap16(flat):
    """idx stream [n] -> dma_gather idx layout [128, n/16] int16."""
    n = flat.shape[0]
    assert n % 16 == 0
    w = np.zeros((16, n // 16), dtype=np.int16)
    w[np.arange(n) % 16, np.arange(n) // 16] = flat
    return np.tile(w, (8, 1))


def _mp_stream(dst_loc, src_pos, w, rows, T):
    """(src_core, dst)-sorted slot stream, chunked into tile-major
    super-chunks of 8 per-core sub-gathers so each tile's segment-sum is one
    PSUM accumulation group.

    Returns program-uniform meta + per-core data (idx16, dstpn, ewb, ewbn).
    """
    cnt = np.zeros((NCORES, T, NCORES), np.int64)
    percore = []
    for me in range(NCORES):
        dl, sp, ww = dst_loc[me], src_pos[me], w[me]
        sc = sp // rows
        np.add.at(cnt[me], (dl // PART, sc), 1)
        percore.append((dl, sp, ww, sc))
    n_tc = cnt.max(axis=0)  # [T, NCORES]
    n_tc = ((n_tc + 15) // 16) * 16          # x16 so idx slices stay aligned

    sec_len = n_tc.sum(axis=0) + 128         # +128 slack for x128 rounding
    sec_len_p = ((sec_len + 127) // 128) * 128
    sec_off = np.concatenate([[0], np.cumsum(sec_len_p)])
    S = int(sec_off[-1])

    tc_off = np.zeros((T, NCORES), np.int64)
    for c in range(NCORES):
        off = int(sec_off[c])
        for t in range(T):
            tc_off[t, c] = off
            off += int(n_tc[t, c])

    # super-chunks: tile ranges where every core's sub-range <= NIDX_MAX-127
    lim = NIDX_MAX - 127
    tranges = []
    t0 = 0
    while t0 < T:
        t1 = t0 + 1
        while (t1 < T and t1 - t0 < EPI_CH
               and int(n_tc[t0:t1 + 1].sum(axis=0).max()) <= lim):
            t1 += 1
        tranges.append((t0, t1))
        t0 = t1

    # emission: per super-chunk, per core: sub-gather (a, npad); blocks are
    # instruction-local; runs = (sub, blk_local, tile, lo, hi) -> emission ids
    schunks = []
    nblk_em = 0
    runs_all = []  # run id -> (blk_em, tile, lo, hi, abs_lo)
    for (t0, t1) in tranges:
        subs = []
        tile_ops = {t: [] for t in range(t0, t1)}
        r0, b0 = len(runs_all), nblk_em
        for c in range(NCORES):
            a = int(tc_off[t0, c])
            b = (int(tc_off[t1, c]) if t1 < T
                 else int(tc_off[t1 - 1, c] + n_tc[t1 - 1, c]))
            npad = ((b - a + 127) // 128) * 128
            assert a + npad <= int(sec_off[c + 1]), "section slack exceeded"
            sub_id = len(subs)
            subs.append((c, a, npad))
            for t in range(t0, t1):
                lo = int(tc_off[t, c]) - a
                hi = lo + int(n_tc[t, c])
                s = lo
                while s < hi:
                    bl = s // 128
                    e = min(hi, (bl + 1) * 128)
                    runs_all.append((nblk_em + bl, t, s - bl * 128,
                                     e - bl * 128, a + s))
                    tile_ops[t].append((sub_id, bl, len(runs_all) - 1))
                    s = e
            nblk_em += npad // 128
        schunks.append(dict(t0=t0, t1=t1, subs=subs, tile_ops=tile_ops,
                            r0=r0, r1=len(runs_all), b0=b0, b1=nblk_em))
    nruns_em = len(runs_all)
    mrun = max(1, max(s["r1"] - s["r0"] for s in schunks))
    mblk = max(1, max(s["b1"] - s["b0"] for s in schunks))

    # per-core data tables in emission layout
    idx16s, dstpns, ewbs, ewbns = [], [], [], []
    for me in range(NCORES):
        dl, sp, ww, sc = percore[me]
        order = np.lexsort((dl, sc))
        dls, sps, wws, scs = (a[order] for a in (dl, sp, ww, sc))
        idx = np.zeros(S, np.int16)
        dstp_sl = np.full(S, 255.0, np.float32)
        ew_sl = np.zeros(S, np.float32)
        ptr = 0
        for c in range(NCORES):
            for t in range(T):
                k = int(cnt[me, t, c])
                if k:
                    o = int(tc_off[t, c])
                    sl = slice(ptr, ptr + k)
                    idx[o:o + k] = (sps[sl] % rows).astype(np.int16)
                    dstp_sl[o:o + k] = (dls[sl] % PART).astype(np.float32)
                    ew_sl[o:o + k] = wws[sl]
                    ptr += k
        assert ptr == dls.shape[0]
        dstpn = np.full((PART, max(nruns_em, 1)), -255.0, np.float32)
        for r, (be, t, lo, hi, abs_lo) in enumerate(runs_all):
            dstpn[lo:hi, r] = -dstp_sl[abs_lo:abs_lo + (hi - lo)]
        # per-emission-block ew columns
        ewb = np.zeros((PART, max(nblk_em, 1)), np.float32)
        for sch in schunks:
            be = sch["b0"]
            for (c, a, npad) in sch["subs"]:
                for bl in range(npad // 128):
                    ewb[:, be] = ew_sl[a + bl * 128:a + (bl + 1) * 128]
                    be += 1
        idx16s.append(_wrap16(idx))
        dstpns.append(dstpn)
        ewbs.append(ewb)
        ewbns.append(-ewb)
    meta = dict(S=S, nblk=nblk_em, nruns=nruns_em, schunks=schunks,
                runs_all=runs_all, mrun=mrun, mblk=mblk)
    return meta, idx16s, dstpns, ewbs, ewbns


def _cover_stream(cl_loc, src_pos, rows1, rows2, T2, pad_local):
    """Dst-aligned per-src-core-bucketed CSR for cover sum+max.

    Returns meta (k_tc [T2, NCORES], chunks, col layout) + per-core idx16."""
    cnt_pc = np.zeros((NCORES, T2 * PART, NCORES), np.int32)
    percore = []
    for me in range(NCORES):
        cl, sp = cl_loc[me], src_pos[me]
        sc = sp // rows1
        np.add.at(cnt_pc[me], (cl, sc), 1)
        percore.append((cl, sp, sc))
    # k per (tile, src core): max over partitions and cores
    k_tc = cnt_pc.reshape(NCORES, T2, PART, NCORES).max(axis=(0, 2))  # [T2, C]
    sec_cols = k_tc.sum(axis=0)  # columns per section
    col_off = np.zeros((T2, NCORES), np.int64)
    acc = 0
    sec_col0 = np.zeros(NCORES + 1, np.int64)
    for c in range(NCORES):
        sec_col0[c] = acc
        for t in range(T2):
            col_off[t, c] = acc
            acc += int(k_tc[t, c])
    sec_col0[NCORES] = acc
    Scols = acc
    S = Scols * 128

    # tile-major super-chunks: per tile range, one sub-gather per src core
    # (every per-core column span must fit one NIDX_MAX gather)
    maxcols = NIDX_MAX // 128
    assert int(k_tc.max()) <= maxcols, "single (t,c) exceeds one gather"
    tranges = []
    t0 = 0
    while t0 < T2:
        t1 = t0 + 1
        while (t1 < T2 and t1 - t0 < EPI_CH
               and int(k_tc[t0:t1 + 1].sum(axis=0).max()) <= maxcols):
            t1 += 1
        tranges.append((t0, t1))
        t0 = t1

    idx16s = []
    for me in range(NCORES):
        cl, sp, sc = percore[me]
        idx_cols = np.full((PART, Scols), -1, np.int64)
        for c in range(NCORES):
            m = sc == c
            cls, sps = cl[m], sp[m]
            order = np.argsort(cls, kind="stable")
            cls, sps = cls[order], sps[order]
            ccount = np.bincount(cls, minlength=T2 * PART)
            first = np.concatenate([[0], np.cumsum(ccount)[:-1]])
            rank = np.arange(cls.shape[0]) - first[cls]
            tt = cls // PART
            p = cls % PART
            col = col_off[tt, c] + rank
            idx_cols[p, col] = sps % rows1
            # pads for this section -> core-local zero row
            secsl = slice(int(sec_col0[c]), int(sec_col0[c + 1]))
            sub = idx_cols[:, secsl]
            sub[sub < 0] = pad_local[c]
            idx_cols[:, secsl] = sub
        # slot stream: col-major (slot i = col*128 + p)
        flat = idx_cols.T.reshape(-1).astype(np.int16)
        idx16s.append(_wrap16(flat))
    meta = dict(k_tc=k_tc, col_off=col_off, tranges=tranges, Scols=Scols,
                S=S)
    return meta, idx16s


def _prep(inputs):
    f32 = np.float32
    x = np.asarray(inputs["x"], f32)
    ei = np.asarray(inputs["edge_index"], np.int64)
    wts = np.asarray(inputs["weights"], f32)
    batch = np.asarray(inputs["batch"], np.int64)
    cover_n = np.asarray(inputs["cover_n"], np.int64)
    cover_c = np.asarray(inputs["cover_c"], np.int64)
    ei2 = np.asarray(inputs["edge_index2"], np.int64)
    wts2 = np.asarray(inputs["weights2"], f32)
    batch2 = np.asarray(inputs["batch2"], np.int64)
    N = x.shape[0]
    C = batch2.shape[0]
    B = int(batch.max()) + 1 if batch.size else 1
    B = max(B, int(batch2.max()) + 1)

    indeg = np.bincount(ei[1], minlength=N)
    perm1, pos1, rows1, tg1 = _shard_items(batch, indeg, B)
    covercnt = np.bincount(cover_c, minlength=C)
    perm2, pos2, rows2, tg2 = _shard_items(batch2, covercnt, B)
    T1, T2 = rows1 // PART, rows2 // PART

    # per-core pad (zero x1) local rows
    pad_local = []
    for c in range(NCORES):
        pads = np.nonzero(perm1[c] < 0)[0]
        assert pads.size, f"core {c} has no pad node row"
        pad_local.append(int(pads[0]))

    # mp level 1: edges grouped by dst core
    dpos = pos1[ei[1]]
    spos = pos1[ei[0]]
    dl1, sp1, w1 = [], [], []
    for me in range(NCORES):
        m = (dpos // rows1) == me
        dl1.append(dpos[m] % rows1)
        sp1.append(spos[m])
        w1.append(wts[m])
    mp1, idx16_1, dstp_1, ewb_1, ewbn_1 = _mp_stream(dl1, sp1, w1, rows1, T1)

    # mp level 2
    dpos2 = pos2[ei2[1]]
    spos2 = pos2[ei2[0]]
    dl2, sp2, w2 = [], [], []
    for me in range(NCORES):
        m = (dpos2 // rows2) == me
        dl2.append(dpos2[m] % rows2)
        sp2.append(spos2[m])
        w2.append(wts2[m])
    mp2, idx16_2, dstp_2, ewb_2, ewbn_2 = _mp_stream(dl2, sp2, w2, rows2, T2)

    # cover
    cpos = pos2[cover_c]
    npos = pos1[cover_n]
    clc, spc = [], []
    for me in range(NCORES):
        m = (cpos // rows2) == me
        clc.append(cpos[m] % rows2)
        spc.append(npos[m])
    cov, idx16_c = _cover_stream(clc, spc, rows1, rows2, T2, pad_local)

    # per-core dense transposed inputs (f32) and masks
    xTs, m1s, m2s = [], [], []
    for c in range(NCORES):
        pc = perm1[c]
        xc = np.zeros((rows1, x.shape[1]), f32)
        xc[pc >= 0] = x[pc[pc >= 0]]
        xTs.append(np.ascontiguousarray(xc.T))
        m1s.append(np.ascontiguousarray(
            (pc >= 0).astype(f32).reshape(T1, PART).T))
        p2 = perm2[c]
        m2s.append(np.ascontiguousarray(
            (p2 >= 0).astype(f32).reshape(T2, PART).T))

    # degree tables (host-side: exact f32) -> dis = rsqrt(deg+1)*mask
    deg1 = np.zeros(NCORES * rows1, f32)
    np.add.at(deg1, dpos, wts)
    deg2 = np.zeros(NCORES * rows2, f32)
    np.add.at(deg2, dpos2, wts2)
    dis1s, dis2s = [], []
    for c in range(NCORES):
        d1 = 1.0 / np.sqrt(deg1[c * rows1:(c + 1) * rows1] + 1.0)
        d1 = d1.reshape(T1, PART).T * m1s[c]
        dis1s.append(np.ascontiguousarray(d1).astype(f32))
        d2 = 1.0 / np.sqrt(deg2[c * rows2:(c + 1) * rows2] + 1.0)
        d2 = d2.reshape(T2, PART).T * m2s[c]
        dis2s.append(np.ascontiguousarray(d2).astype(f32))

    meta = dict(B=B, T1=T1, T2=T2, rows1=rows1, rows2=rows2,
                mp1=mp1, mp2=mp2, cov=cov, tg1=tg1, tg2=tg2, FIN=x.shape[1])

    rep = lambda v: np.ascontiguousarray(
        np.broadcast_to(np.asarray(v, f32).reshape(1, -1), (PART, v.shape[-1])))
    g = np.asarray(inputs["bn_gamma"], f32)
    bb = np.asarray(inputs["bn_beta"], f32)
    l1w = np.asarray(inputs["lin1_W"], f32)
    H = np.asarray(inputs["W_in0"], f32).shape[1]
    selS = np.r_[0:H, 2 * H:3 * H]
    selM = np.r_[H:2 * H, 3 * H:4 * H]
    shared = {
        "W_in0": np.asarray(inputs["W_in0"], f32),
        "W_in1": np.asarray(inputs["W_in1"], f32),
        "Wl_in": np.asarray(inputs["Wl_in"], f32),
        "W_b0": np.asarray(inputs["W_b0"], f32),
        "W_b1": np.asarray(inputs["W_b1"], f32),
        "Wl_b": np.asarray(inputs["Wl_b"], f32),
        "b_in0": rep(inputs["b_in0"]), "b_in1": rep(inputs["b_in1"]),
        "bl_in": rep(inputs["bl_in"]), "b_b0": rep(inputs["b_b0"]),
        "b_b1": rep(inputs["b_b1"]), "bl_b": rep(inputs["bl_b"]),
        "gammaS": np.ascontiguousarray(g[selS].reshape(PART, 1)),
        "gammaM": np.ascontiguousarray(g[selM].reshape(PART, 1)),
        "betaS": np.ascontiguousarray(bb[selS].reshape(PART, 1)),
        "betaM": np.ascontiguousarray(bb[selM].reshape(PART, 1)),
        "l1WS": np.ascontiguousarray(l1w[selS]),
        "l1WM": np.ascontiguousarray(l1w[selM]),
        "l1b": rep(inputs["lin1_b"]),
        "l2W": np.asarray(inputs["lin2_W"], f32),
        "l2b": rep(inputs["lin2_b"]),
        "iota": np.ascontiguousarray(
            np.broadcast_to(np.arange(PART, dtype=f32)[None, :],
                            (PART, PART))).astype(BF16),
        "iotan": np.ascontiguousarray(
            np.broadcast_to(-np.arange(PART, dtype=f32)[None, :],
                            (PART, PART))).astype(BF16),
    }
    in_maps = []
    for c in range(NCORES):
        m = dict(shared)
        m["x_cT"] = xTs[c]
        m["mask1"] = m1s[c]
        m["mask2"] = m2s[c]
        m["dis1"] = dis1s[c]
        m["dis2"] = dis2s[c]
        m["idx16_1"] = idx16_1[c]
        m["dstp_1"] = dstp_1[c]
        m["ewb_1"] = ewb_1[c]
        m["ewbn_1"] = ewbn_1[c]
        m["idx16_2"] = idx16_2[c]
        m["dstp_2"] = dstp_2[c]
        m["ewb_2"] = ewb_2[c]
        m["ewbn_2"] = ewbn_2[c]
        m["idx16_c"] = idx16_c[c]
        in_maps.append(m)
    return meta, in_maps


# ------------------------------------------------------------- device kernel

def _build(meta, NCLS=10, H=64):
    import concourse.bass as bass
    import concourse.bacc as bacc
    import concourse.mybir as mybir
    import concourse.tile as tile
    from concourse.masks import make_identity
    from concourse import library_config

    f32 = mybir.dt.float32
    bf16 = mybir.dt.bfloat16
    i16 = mybir.dt.int16
    ALU = mybir.AluOpType
    ACTF = mybir.ActivationFunctionType
    AX = mybir.AxisListType

    B = meta["B"]
    T1, T2 = meta["T1"], meta["T2"]
    rows1, rows2 = meta["rows1"], meta["rows2"]
    FIN = meta["FIN"]
    mp1, mp2, cov = meta["mp1"], meta["mp2"], meta["cov"]
    RG = [list(range(NCORES))]

    nc = bacc.Bacc("TRN2", target_bir_lowering=False, debug=False,
                   num_devices=NCORES, num_swdge_queues=4)

    ein = lambda n, s, d=f32: nc.dram_tensor(n, s, d, kind="ExternalInput")
    x_cT = ein("x_cT", [FIN, rows1])
    mask1 = ein("mask1", [PART, T1]); mask2 = ein("mask2", [PART, T2])
    dis1_d = ein("dis1", [PART, T1]); dis2_d = ein("dis2", [PART, T2])
    idx16_1 = ein("idx16_1", [PART, mp1["S"] // 16], i16)
    dstp_1 = ein("dstp_1", [PART, max(mp1["nruns"], 1)])
    ewb_1 = ein("ewb_1", [PART, max(mp1["nblk"], 1)])
    ewbn_1 = ein("ewbn_1", [PART, max(mp1["nblk"], 1)])
    idx16_2 = ein("idx16_2", [PART, mp2["S"] // 16], i16)
    dstp_2 = ein("dstp_2", [PART, max(mp2["nruns"], 1)])
    ewb_2 = ein("ewb_2", [PART, max(mp2["nblk"], 1)])
    ewbn_2 = ein("ewbn_2", [PART, max(mp2["nblk"], 1)])
    idx16_c = ein("idx16_c", [PART, cov["S"] // 16], i16)
    iota_d = ein("iota", [PART, PART], bf16)
    iotan_d = ein("iotan", [PART, PART], bf16)
    wshapes = {"W_in0": [FIN, H], "W_in1": [H, H], "Wl_in": [2 * H, H],
               "W_b0": [2 * H, H], "W_b1": [H, H], "Wl_b": [2 * H, H]}
    Ws = {n: ein(n, s) for n, s in wshapes.items()}
    bs = {n: ein(n, [PART, H]) for n in
          ("b_in0", "b_in1", "bl_in", "b_b0", "b_b1", "bl_b")}
    gammaS = ein("gammaS", [PART, 1]); gammaM = ein("gammaM", [PART, 1])
    betaS = ein("betaS", [PART, 1]); betaM = ein("betaM", [PART, 1])
    l1WS = ein("l1WS", [PART, H]); l1WM = ein("l1WM", [PART, H])
    l1b = ein("l1b", [PART, H])
    l2W = ein("l2W", [H, NCLS]); l2b = ein("l2b", [PART, NCLS])
    out_ext = nc.dram_tensor("out", [B, NCLS], f32, kind="ExternalOutput")

    # internal DRAM: wide bf16 tables (upper half junk, never read)
    hs_c1 = nc.dram_tensor("hs_c1", [rows1, WIDE], bf16)
    hs_full1 = nc.dram_tensor("hs_full1", [NCORES * rows1, WIDE], bf16, addr_space="Shared")
    hs_c1b = nc.dram_tensor("hs_c1b", [rows1, WIDE], bf16)
    hs_full1b = nc.dram_tensor("hs_full1b", [NCORES * rows1, WIDE], bf16, addr_space="Shared")
    x1_c = nc.dram_tensor("x1_c", [rows1, WIDE], bf16)
    x1_full = nc.dram_tensor("x1_full", [NCORES * rows1, WIDE], bf16, addr_space="Shared")
    hs_c2 = nc.dram_tensor("hs_c2", [rows2, WIDE], bf16)
    hs_full2 = nc.dram_tensor("hs_full2", [NCORES * rows2, WIDE], bf16, addr_space="Shared")
    hs_c2b = nc.dram_tensor("hs_c2b", [rows2, WIDE], bf16)
    hs_full2b = nc.dram_tensor("hs_full2b", [NCORES * rows2, WIDE], bf16, addr_space="Shared")
    arS_in = nc.dram_tensor("arS_in", [PART, B], f32)
    arS_out = nc.dram_tensor("arS_out", [PART, B], f32, addr_space="Shared")
    arM_in = nc.dram_tensor("arM_in", [PART, B], f32)
    arM_out = nc.dram_tensor("arM_out", [PART, B], f32, addr_space="Shared")

    with tile.TileContext(nc) as tc:
        nc.gpsimd.load_library(library_config.mlp)
        with (tc.tile_pool(name="const", bufs=1) as cpool,
              tc.tile_pool(name="res", bufs=1) as rpool,
              tc.tile_pool(name="gtp", bufs=3) as gtpool,
              tc.tile_pool(name="stg", bufs=3) as stgpool,
              tc.tile_pool(name="work", bufs=2) as wpool,
              tc.tile_pool(name="ps", bufs=3, space="PSUM") as pspool,
              tc.tile_pool(name="psacc", bufs=1, space="PSUM") as papool):

            ident = cpool.tile([PART, PART], f32, tag="ident")
            make_identity(nc, ident[:])

            def load2d(dram, shape, dt=f32, tag=None):
                t = cpool.tile(list(shape), dt, tag=tag or dram.name)
                nc.sync.dma_start(t[:], dram[:, :])
                return t

            identB = cpool.tile([PART, PART], bf16, tag="identB")
            make_identity(nc, identB[:])
            mask1_s = load2d(mask1, (PART, T1))
            mask2_s = load2d(mask2, (PART, T2))
            dis1 = load2d(dis1_d, (PART, T1), tag="dis1s")
            dis2 = load2d(dis2_d, (PART, T2), tag="dis2s")
            iota_s = load2d(iota_d, (PART, PART), bf16)
            iotan_s = load2d(iotan_d, (PART, PART), bf16, tag="iotan")
            W_s = {n: load2d(Ws[n], Ws[n].shape) for n in Ws}
            b_s = {n: load2d(bs[n], (PART, H)) for n in bs}
            l1WS_s = load2d(l1WS, (PART, H)); l1WM_s = load2d(l1WM, (PART, H))
            l1b_s = load2d(l1b, (PART, H))
            l2W_s = load2d(l2W, (H, NCLS)); l2b_s = load2d(l2b, (PART, NCLS))
            gS_s = load2d(gammaS, (PART, 1)); gM_s = load2d(gammaM, (PART, 1))
            bS_s = load2d(betaS, (PART, 1)); bM_s = load2d(betaM, (PART, 1))

            # f32 activation accumulators: [..., 0, :] = layer a / cover sum,
            # [..., 1, :] = layer b / cover max
            acc1 = rpool.tile([PART, T1, 2, H], f32, tag="acc1")
            acc2 = rpool.tile([PART, T2, 2, H], f32, tag="acc2")
            hs1_sb = rpool.tile([PART, T1, H], bf16, tag="hs1_sb")
            hs2_sb = rpool.tile([PART, T2, H], bf16, tag="hs2_sb")
            rm1 = rpool.tile([PART, B, H], f32, tag="rm1")
            rm2 = rpool.tile([PART, B, H], f32, tag="rm2")
            oneh = rpool.tile([PART, B, B], f32, tag="oneh")
            nc.vector.memset(rm1[:], 0.0)
            nc.vector.memset(rm2[:], 0.0)
            nc.vector.memset(oneh[:], 0.0)
            for g in range(B):
                nc.vector.memset(oneh[:, g, g:g + 1], 1.0)

            def bc_mid(ap2d, G):
                a = ap2d.ap
                return bass.AP(ap2d.tensor, ap2d.offset,
                               [a[0], [0, G], a[-1]])

            ps_sum1 = papool.tile([B, H], f32, tag="sum1")
            ps_sum2 = papool.tile([B, H], f32, tag="sum2")

            def stripes(T, step):
                return [(s, min(s + step, T)) for s in range(0, T, step)]

            # ---- dense matmul phase: hs = dis * (act @ W) -> SBUF + DRAM ----
            def mm_phase(lhsT_fn, Tn, W, dis_t, hs_sb, hs_dram):
                hsd = hs_dram.ap().rearrange("(t p) f -> p t f", p=PART)
                for (s0, s1) in stripes(Tn, STRIPE):
                    for t in range(s0, s1):
                        lhsT = lhsT_fn(t)
                        mm = pspool.tile([PART, H], f32, tag="mm")
                        nc.tensor.matmul(out=mm[:], lhsT=lhsT, rhs=W[:],
                                         start=True, stop=True)
                        nc.vector.tensor_scalar(
                            out=hs_sb[:, t, :], in0=mm[:],
                            scalar1=dis_t[:, t:t + 1], scalar2=None,
                            op0=ALU.mult)
                    nc.sync.dma_start(hsd[:, s0:s1, 0:H], hs_sb[:, s0:s1, :])

            def lhsT_transpose(src_fn, kdim):
                def fn(t):
                    tp = pspool.tile([PART, PART], f32, tag="tp")
                    nc.tensor.transpose(tp[:kdim, :], src_fn(t), ident[:])
                    tsb = wpool.tile([PART, PART], f32, tag="tsb", bufs=4)
                    nc.scalar.copy(out=tsb[:kdim, :], in_=tp[:kdim, :])
                    return tsb[:kdim, :]
                return fn

            xTv = x_cT.ap()
            _xc = {}

            def lhsT_x(t):
                s0 = (t // STRIPE_X) * STRIPE_X
                if s0 not in _xc:
                    xstg = stgpool.tile([FIN, STRIPE_X * PART], f32,
                                        tag="xstg")
                    s1 = min(s0 + STRIPE_X, T1)
                    nc.sync.dma_start(xstg[:, :(s1 - s0) * PART],
                                      xTv[:, s0 * PART:s1 * PART])
                    _xc[s0] = xstg
                return _xc[s0][:, (t - s0) * PART:(t - s0 + 1) * PART]

            def allgather(src, dst):
                nc.gpsimd.collective_compute(
                    "AllGather", ALU.bypass, ins=[src.ap().opt()],
                    outs=[dst.ap().opt()], replica_groups=RG)

            # ---- mp phase: tile-major super-chunks; per tile one PSUM
            # accumulation group (self matmul + one matmul per run), fused
            # epilogue acc = relu((sum + hs_self)*dis + bias) ----
            _qrot = [0]

            def mp_phase(mp, hs_full, idx16_d, dstp_d, ewb_d, ewbn_d, rows,
                         acc, half, hs_sb, dis_t, bias, tile_cb=None):
                runs_all = mp["runs_all"]
                mrun, mblk = mp["mrun"], mp["mblk"]
                mctr = 0
                for sch in mp["schunks"]:
                    r0, r1 = sch["r0"], sch["r1"]
                    b0, b1 = sch["b0"], sch["b1"]
                    dst_t = stgpool.tile([PART, mrun], f32, tag="dstpstg")
                    nc.sync.dma_start(dst_t[:, :r1 - r0], dstp_d[:, r0:r1])
                    ew_t = stgpool.tile([PART, mblk], f32, tag="ewstg")
                    nc.sync.dma_start(ew_t[:, :b1 - b0], ewb_d[:, b0:b1])
                    ewn_t = stgpool.tile([PART, mblk], f32, tag="ewnstg")
                    nc.sync.dma_start(ewn_t[:, :b1 - b0], ewbn_d[:, b0:b1])
                    gts = []
                    for (c, a, npad) in sch["subs"]:
                        idxt = stgpool.tile([PART, NIDX_MAX // 16], i16,
                                            tag="idxstg", name="idxt",
                                            bufs=12)
                        nc.sync.dma_start(
                            idxt[:, :npad // 16],
                            idx16_d[:, a // 16:(a + npad) // 16])
                        gt = gtpool.tile([PART, NIDX_MAX // 128, WIDE],
                                         bf16, tag="gt", name="gt", bufs=10)
                        nc.gpsimd.dma_gather(
                            gt[:, :npad // 128, :],
                            hs_full[c * rows:(c + 1) * rows, :],
                            idxt[:, :npad // 16], npad, npad, WIDE,
                            queue_num=_qrot[0] % 4)
                        _qrot[0] += 1
                        gts.append(gt)
                    for t in range(sch["t0"], sch["t1"]):
                        ops = sch["tile_ops"][t]
                        ps = pspool.tile([PART, H], f32, tag="mm")
                        nc.tensor.matmul(out=ps[:], lhsT=identB[:],
                                         rhs=hs_sb[:, t, :], start=True,
                                         stop=(len(ops) == 0),
                                         skip_group_check=True)
                        for j, (sub_id, bl, rid) in enumerate(ops):
                            be = runs_all[rid][0]
                            M = wpool.tile([PART, PART], bf16, tag="M", bufs=6)
                            if mctr % 2 == 0:
                                nc.vector.tensor_scalar(
                                    out=M[:], in0=iotan_s[:],
                                    scalar1=dst_t[:, rid - r0:rid - r0 + 1],
                                    scalar2=ew_t[:, be - b0:be - b0 + 1],
                                    op0=ALU.is_equal, op1=ALU.mult)
                            else:
                                msq = wpool.tile([PART, PART], bf16,
                                                 tag="msq", bufs=2)
                                nc.scalar.activation(
                                    msq[:], iota_s[:], ACTF.Square,
                                    bias=dst_t[:, rid - r0:rid - r0 + 1])
                                nc.scalar.activation(
                                    M[:], msq[:], ACTF.Relu,
                                    bias=ew_t[:, be - b0:be - b0 + 1],
                                    scale=ewn_t[:, be - b0:be - b0 + 1])
                            mctr += 1
                            nc.tensor.matmul(out=ps[:], lhsT=M[:],
                                             rhs=gts[sub_id][:, bl, 0:H],
                                             start=False,
                                             stop=(j == len(ops) - 1),
                                             skip_group_check=True)
                        ept = wpool.tile([PART, H], f32, tag="ept", bufs=5)
                        nc.vector.tensor_scalar(
                            out=ept[:], in0=ps[:],
                            scalar1=dis_t[:, t:t + 1], scalar2=None,
                            op0=ALU.mult)
                        nc.vector.tensor_tensor(out=ept[:], in0=ept[:],
                                                in1=bias[:], op=ALU.add)
                        nc.scalar.activation(acc[:, t, half, :], ept[:],
                                             ACTF.Relu)
                    if tile_cb is not None:
                        tile_cb(sch["t0"], sch["t1"])

            # per-tile mm work folded into a preceding mp phase (emission
            # interleaving hides the dense chains under gather drains)
            def mm_tile_cb(lhsT_fn, W, dis_t, hs_sb, hs_dram):
                hsd = hs_dram.ap().rearrange("(t p) f -> p t f", p=PART)

                def cb(t0, t1):
                    for t in range(t0, t1):
                        lhsT = lhsT_fn(t)
                        mm = pspool.tile([PART, H], f32, tag="mm")
                        nc.tensor.matmul(out=mm[:], lhsT=lhsT, rhs=W[:],
                                         start=True, stop=True,
                                         skip_group_check=True)
                        nc.vector.tensor_scalar(
                            out=hs_sb[:, t, :], in0=mm[:],
                            scalar1=dis_t[:, t:t + 1], scalar2=None,
                            op0=ALU.mult)
                    nc.sync.dma_start(hsd[:, t0:t1, 0:H], hs_sb[:, t0:t1, :])
                return cb

            def jk_tile_cb(acc, Tn, Wl, bias, mask_s, tg, ps_sum, rm,
                           x_dram):
                lfn = lhsT_transpose(
                    lambda t: acc[:, t, :, :].rearrange("p a b -> p (a b)"),
                    PART)
                xd = (x_dram.ap().rearrange("(t p) f -> p t f", p=PART)
                      if x_dram is not None else None)

                def cb(t0, t1):
                    stg = (stgpool.tile([PART, EPI_CH, H], bf16, tag="x1stg",
                                        name="stg", bufs=2)
                           if xd is not None else None)
                    for t in range(t0, t1):
                        lhsT = lfn(t)
                        mm = pspool.tile([PART, H], f32, tag="mm")
                        nc.tensor.matmul(out=mm[:], lhsT=lhsT, rhs=Wl[:],
                                         start=True, stop=True,
                                         skip_group_check=True)
                        xt = wpool.tile([PART, H], f32, tag="xt", bufs=5)
                        nc.vector.tensor_tensor(out=xt[:], in0=mm[:],
                                                in1=bias[:], op=ALU.add)
                        nc.scalar.activation(xt[:], xt[:], ACTF.Relu,
                                             scale=mask_s[:, t:t + 1])
                        g = int(tg[t])
                        nc.tensor.matmul(out=ps_sum[:], lhsT=oneh[:, g, :],
                                         rhs=xt[:], start=(t == 0),
                                         stop=(t == Tn - 1),
                                         skip_group_check=True)
                        nc.vector.tensor_tensor(out=rm[:, g, :],
                                                in0=rm[:, g, :],
                                                in1=xt[:], op=ALU.max)
                        if stg is not None:
                            nc.scalar.copy(out=stg[:, t - t0, :], in_=xt[:])
                    if stg is not None:
                        nc.sync.dma_start(xd[:, t0:t1, 0:H],
                                          stg[:, :t1 - t0, :])
                return cb

            # ---- jk: cat(a,b) @ Wl + bias, relu*mask, readouts ----
            def jk_phase(acc, Tn, Wl, bias, mask_s, tg, ps_sum, rm, x_dram):
                lfn = lhsT_transpose(
                    lambda t: acc[:, t, :, :].rearrange("p a b -> p (a b)"),
                    PART)
                xd = (x_dram.ap().rearrange("(t p) f -> p t f", p=PART)
                      if x_dram is not None else None)
                for (s0, s1) in stripes(Tn, EPI_CH):
                    stg = (stgpool.tile([PART, EPI_CH, H], bf16, tag="x1stg",
                                        name="stg", bufs=2)
                           if x_dram is not None else None)
                    for t in range(s0, s1):
                        lhsT = lfn(t)
                        mm = pspool.tile([PART, H], f32, tag="mm")
                        nc.tensor.matmul(out=mm[:], lhsT=lhsT, rhs=Wl[:],
                                         start=True, stop=True)
                        xt = wpool.tile([PART, H], f32, tag="xt", bufs=5)
                        nc.vector.tensor_tensor(out=xt[:], in0=mm[:],
                                                in1=bias[:], op=ALU.add)
                        nc.scalar.activation(xt[:], xt[:], ACTF.Relu,
                                             scale=mask_s[:, t:t + 1])
                        g = int(tg[t])
                        nc.tensor.matmul(out=ps_sum[:], lhsT=oneh[:, g, :],
                                         rhs=xt[:], start=(t == 0),
                                         stop=(t == Tn - 1),
                                         skip_group_check=True)
                        nc.vector.tensor_tensor(out=rm[:, g, :],
                                                in0=rm[:, g, :],
                                                in1=xt[:], op=ALU.max)
                        if stg is not None:
                            nc.scalar.copy(out=stg[:, t - s0, :], in_=xt[:])
                    if stg is not None:
                        nc.sync.dma_start(xd[:, s0:s1, 0:H],
                                          stg[:, :s1 - s0, :])

            # ================= pipeline =================
            with nc.named_scope("mm1a"):
                mm_phase(lhsT_x, T1, W_s["W_in0"], dis1, hs1_sb, hs_c1)
            with nc.named_scope("ag1a"):
                allgather(hs_c1, hs_full1)
            with nc.named_scope("mp1a"):
                mp_phase(mp1, hs_full1, idx16_1, dstp_1, ewb_1, ewbn_1,
                         rows1, acc1, 0, hs1_sb, dis1, b_s["b_in0"],
                         tile_cb=mm_tile_cb(
                             lhsT_transpose(lambda t: acc1[:, t, 0, :], H),
                             W_s["W_in1"], dis1, hs1_sb, hs_c1b))
            with nc.named_scope("ag1b"):
                allgather(hs_c1b, hs_full1b)
            with nc.named_scope("mp1b"):
                mp_phase(mp1, hs_full1b, idx16_1, dstp_1, ewb_1, ewbn_1,
                         rows1, acc1, 1, hs1_sb, dis1, b_s["b_in1"],
                         tile_cb=jk_tile_cb(acc1, T1, W_s["Wl_in"],
                                            b_s["bl_in"], mask1_s,
                                            meta["tg1"], ps_sum1, rm1,
                                            x1_c))
            with nc.named_scope("agx1"):
                allgather(x1_c, x1_full)

            # ---------- cover pooling (tile-major): sum -> acc1[...,0],
            # max -> [...,1]; mm2a per-tile work folded in ----------
            with nc.named_scope("cover"):
                k_tc = cov["k_tc"]; col_off = cov["col_off"]
                mm2a_cb = mm_tile_cb(
                    lhsT_transpose(
                        lambda t: acc1[:, t, :, :].rearrange(
                            "p a b -> p (a b)"), PART),
                    W_s["W_b0"], dis2, hs2_sb, hs_c2)
                for (t0, t1) in cov["tranges"]:
                    gts = []
                    for c in range(NCORES):
                        a = int(col_off[t0, c])
                        ncols = int(k_tc[t0:t1, c].sum())
                        if ncols == 0:
                            gts.append(None)
                            continue
                        n = ncols * 128
                        idxt = stgpool.tile([PART, NIDX_MAX // 16], i16,
                                            tag="idxstg", name="idxt",
                                            bufs=12)
                        nc.sync.dma_start(idxt[:, :n // 16],
                                          idx16_c[:, a * 8:(a + ncols) * 8])
                        gt = gtpool.tile([PART, NIDX_MAX // 128, WIDE],
                                         bf16, tag="gt", name="gt", bufs=10)
                        nc.gpsimd.dma_gather(
                            gt[:, :ncols, :],
                            x1_full[c * rows1:(c + 1) * rows1, :],
                            idxt[:, :n // 16], n, n, WIDE,
                            queue_num=_qrot[0] % 4)
                        _qrot[0] += 1
                        gts.append(gt)
                    for t in range(t0, t1):
                        first = True
                        for c in range(NCORES):
                            k = int(k_tc[t, c])
                            if k == 0:
                                continue
                            ca = int(col_off[t, c]) - int(col_off[t0, c])
                            view = gts[c][:, ca:ca + k, 0:H].rearrange(
                                "p k f -> p f k")
                            zs = wpool.tile([PART, H], f32, tag="zs",
                                            bufs=5)
                            nc.vector.tensor_reduce(out=zs[:], in_=view,
                                                    axis=AX.X, op=ALU.add)
                            zm = wpool.tile([PART, H], f32, tag="zm",
                                            bufs=5)
                            nc.vector.tensor_reduce(out=zm[:], in_=view,
                                                    axis=AX.X, op=ALU.max)
                            if first:
                                nc.vector.tensor_copy(acc1[:, t, 0, :],
                                                      zs[:])
                                nc.vector.tensor_copy(acc1[:, t, 1, :],
                                                      zm[:])
                                first = False
                            else:
                                nc.vector.tensor_tensor(
                                    out=acc1[:, t, 0, :],
                                    in0=acc1[:, t, 0, :], in1=zs[:],
                                    op=ALU.add)
                                nc.vector.tensor_tensor(
                                    out=acc1[:, t, 1, :],
                                    in0=acc1[:, t, 1, :], in1=zm[:],
                                    op=ALU.max)
                        if first:
                            nc.vector.memset(acc1[:, t, 0, :], 0.0)
                            nc.vector.memset(acc1[:, t, 1, :], 0.0)
                    mm2a_cb(t0, t1)
            with nc.named_scope("ag2a"):
                allgather(hs_c2, hs_full2)
            with nc.named_scope("mp2a"):
                mp_phase(mp2, hs_full2, idx16_2, dstp_2, ewb_2, ewbn_2,
                         rows2, acc2, 0, hs2_sb, dis2, b_s["b_b0"],
                         tile_cb=mm_tile_cb(
                             lhsT_transpose(lambda t: acc2[:, t, 0, :], H),
                             W_s["W_b1"], dis2, hs2_sb, hs_c2b))
            with nc.named_scope("ag2b"):
                allgather(hs_c2b, hs_full2b)
            with nc.named_scope("mp2b"):
                mp_phase(mp2, hs_full2b, idx16_2, dstp_2, ewb_2, ewbn_2,
                         rows2, acc2, 1, hs2_sb, dis2, b_s["b_b1"],
                         tile_cb=jk_tile_cb(acc2, T2, W_s["Wl_b"],
                                            b_s["bl_b"], mask2_s,
                                            meta["tg2"], ps_sum2, rm2,
                                            None))

            # ---------- readout combine + head ----------
            sc_head = nc.named_scope("head"); sc_head.__enter__()
            sum1_sb = wpool.tile([B, H], f32, tag="s1sb")
            nc.scalar.copy(out=sum1_sb[:], in_=ps_sum1[:])
            sum2_sb = wpool.tile([B, H], f32, tag="s2sb")
            nc.scalar.copy(out=sum2_sb[:], in_=ps_sum2[:])
            sT = pspool.tile([H, B], f32, tag="tp")
            nc.tensor.matmul(out=sT[:], lhsT=sum1_sb[:], rhs=ident[:B, :B],
                             start=True, stop=True)
            sT1 = wpool.tile([H, B], f32, tag="sT1")
            nc.scalar.copy(out=sT1[:], in_=sT[:])
            sT_2 = pspool.tile([H, B], f32, tag="tp")
            nc.tensor.matmul(out=sT_2[:], lhsT=sum2_sb[:], rhs=ident[:B, :B],
                             start=True, stop=True)
            sT2 = wpool.tile([H, B], f32, tag="sT2")
            nc.scalar.copy(out=sT2[:], in_=sT_2[:])
            nc.sync.dma_start(arS_in[0:H, :], sT1[:])
            nc.sync.dma_start(arS_in[H:2 * H, :], sT2[:])

            mT1 = wpool.tile([H, B], f32, tag="mT1")
            mT2 = wpool.tile([H, B], f32, tag="mT2")
            for g in range(B):
                for rm, mt in ((rm1, mT1), (rm2, mT2)):
                    tpm = pspool.tile([H, PART], f32, tag="tp")
                    nc.tensor.transpose(tpm[:], rm[:, g, :], ident[:])
                    msb = wpool.tile([H, PART], f32, tag="msb")
                    nc.scalar.copy(out=msb[:], in_=tpm[:])
                    nc.vector.tensor_reduce(out=mt[:, g:g + 1], in_=msb[:],
                                            axis=AX.X, op=ALU.max)
            nc.sync.dma_start(arM_in[0:H, :], mT1[:])
            nc.sync.dma_start(arM_in[H:2 * H, :], mT2[:])

            nc.gpsimd.collective_compute(
                "AllReduce", ALU.add, ins=[arS_in.ap().opt()],
                outs=[arS_out.ap().opt()], replica_groups=RG)
            nc.gpsimd.collective_compute(
                "AllReduce", ALU.max, ins=[arM_in.ap().opt()],
                outs=[arM_out.ap().opt()], replica_groups=RG)

            S_sb = wpool.tile([PART, B], f32, tag="Ssb")
            M_sb = wpool.tile([PART, B], f32, tag="Msb")
            nc.sync.dma_start(S_sb[:], arS_out[:, :])
            nc.sync.dma_start(M_sb[:], arM_out[:, :])

            def bn(t_sb, gam, bet):
                mu = wpool.tile([PART, 1], f32, tag="mu")
                nc.vector.tensor_reduce(out=mu[:], in_=t_sb[:], axis=AX.X,
                                        op=ALU.add)
                nc.vector.tensor_scalar_mul(mu[:], mu[:], 1.0 / B)
                nc.vector.tensor_scalar(out=t_sb[:], in0=t_sb[:],
                                        scalar1=mu[:], scalar2=None,
                                        op0=ALU.subtract)
                sq = wpool.tile([PART, B], f32, tag="sq")
                nc.vector.tensor_tensor(out=sq[:], in0=t_sb[:], in1=t_sb[:],
                                        op=ALU.mult)
                var = wpool.tile([PART, 1], f32, tag="var")
                nc.vector.tensor_reduce(out=var[:], in_=sq[:], axis=AX.X,
                                        op=ALU.add)
                nc.vector.tensor_scalar(out=var[:], in0=var[:],
                                        scalar1=1.0 / B, scalar2=EPS,
                                        op0=ALU.mult, op1=ALU.add)
                nc.scalar.activation(var[:], var[:], ACTF.Sqrt)
                nc.vector.reciprocal(var[:], var[:])
                nc.vector.tensor_scalar(out=t_sb[:], in0=t_sb[:],
                                        scalar1=var[:], scalar2=gam[:],
                                        op0=ALU.mult, op1=ALU.mult)
                nc.vector.tensor_scalar(out=t_sb[:], in0=t_sb[:],
                                        scalar1=bet[:], scalar2=None,
                                        op0=ALU.add)

            bn(S_sb, gS_s, bS_s)
            bn(M_sb, gM_s, bM_s)

            pl1 = pspool.tile([B, H], f32, tag="mm")
            nc.tensor.matmul(out=pl1[:], lhsT=S_sb[:], rhs=l1WS_s[:],
                             start=True, stop=False)
            nc.tensor.matmul(out=pl1[:], lhsT=M_sb[:], rhs=l1WM_s[:],
                             start=False, stop=True)
            y = wpool.tile([B, H], f32, tag="y")
            nc.vector.tensor_tensor(out=y[:], in0=pl1[:], in1=l1b_s[:B, :],
                                    op=ALU.add)
            nc.scalar.activation(y[:], y[:], ACTF.Relu)
            yT_ps = pspool.tile([H, B], f32, tag="tp")
            nc.tensor.matmul(out=yT_ps[:], lhsT=y[:], rhs=ident[:B, :B],
                             start=True, stop=True)
            yT = wpool.tile([H, B], f32, tag="yTs")
            nc.scalar.copy(out=yT[:], in_=yT_ps[:])
            pl2 = pspool.tile([B, NCLS], f32, tag="mm")
            nc.tensor.matmul(out=pl2[:], lhsT=yT[:], rhs=l2W_s[:],
                             start=True, stop=True)
            z = wpool.tile([B, NCLS], f32, tag="z")
            nc.vector.tensor_tensor(out=z[:], in0=pl2[:], in1=l2b_s[:B, :],
                                    op=ALU.add)
            zmax = wpool.tile([B, 1], f32, tag="zmax")
            nc.vector.tensor_reduce(out=zmax[:], in_=z[:], axis=AX.X,
                                    op=ALU.max)
            nc.vector.tensor_scalar(out=z[:], in0=z[:], scalar1=zmax[:],
                                    scalar2=None, op0=ALU.subtract)
            nc.scalar.activation(z[:], z[:], ACTF.Exp)
            zsum = wpool.tile([B, 1], f32, tag="zsum")
            nc.vector.tensor_reduce(out=zsum[:], in_=z[:], axis=AX.X,
                                    op=ALU.add)
            nc.vector.reciprocal(zsum[:], zsum[:])
            nc.vector.tensor_scalar(out=z[:], in0=z[:], scalar1=zsum[:],
                                    scalar2=None, op0=ALU.mult)
            nc.sync.dma_start(out_ext[:, :], z[:])
            sc_head.__exit__(None, None, None)

    nc.compile()
    return nc


def kernel(**inputs):
    from concourse import bass_utils
    meta, in_maps = _prep(inputs)
    nc = _build(meta)
    res = bass_utils.run_bass_kernel_spmd(
        nc, in_maps, core_ids=list(range(NCORES)))
    return np.asarray(res.results[0]["out"])
